# revision 1
# baseline (speedup 1.0000x reference)
"""Trainium2 Bass kernel for grouped-attention MoE routing.

Math (derived from the nn.Module):
  gate  = softmax(mlp(maxpool(conv(x))) + mlp(avgpool(conv(x))))      (B,45)
  sel   = sorted(top22(mean_b gate))                                  (22,)
  Per expert e with u = x[:, sel[e], :]:
    energy[l,m] = (a_e*u_l + g_e) * u_m   (rank-1; scalars a,g from weights)
    attn = softmax_m(energy);  s_l = sum_m u_m attn[l,m]
    y_l  = P_e*s_l + Q_e;      A[:,sel[e],:] = y * gate[:,sel[e]]
  G = x * A (flat);  return (G, A_flat)

Strategy: pure data parallel over batch on 8 cores; two launches with the
45-float routing reduction mediated on host (equivalent of the all-reduce).
"""

import numpy as np
from contextlib import ExitStack

import bass_rust
import concourse.bass as bass
import concourse.mybir as mybir
import concourse.tile as tile
from concourse.bass_utils import run_bass_kernel_spmd

_MULTIWAIT_OK = ("InstNoOp", "InstAllEngineBarrier",
                 "InstEventSemaphore", "InstUnconditionalBranch")


def legalize_sync_waits(nc):
    """walrus codegen on this stack rejects >1 sync wait on most
    instructions; hoist extra waits onto same-engine NoOps."""
    for func in nc.m.functions:
        for block in func.blocks:
            il = block.instructions
            out = []
            for inst in il:
                tname = type(inst).__name__
                si = getattr(inst, "sync_info", None)
                waits = list(si.on_wait) if si is not None else []
                if tname not in _MULTIWAIT_OK and len(waits) > 1:
                    for k, w in enumerate(waits):
                        nop = mybir.InstNoOp(
                            name=f"{inst.name}-synop{k}", ins=[], outs=[])
                        nop.engine = inst.engine
                        nop.sync_info = bass_rust.SyncInfo(
                            on_wait=[w], on_update=[])
                        out.append(nop)
                    inst.sync_info = bass_rust.SyncInfo(
                        on_wait=[], on_update=list(inst.sync_info.on_update))
                out.append(inst)
            il.clear()
            il.extend(out)

B, C, L, E = 8192, 45, 21, 22
NCORES = 8
BC = B // NCORES          # rows per core
P = 128                   # SBUF partitions
NT = BC // P              # batch tiles per core
CL = C * L                # 945
F32 = mybir.dt.float32
BF16 = mybir.dt.bfloat16
AF = mybir.ActivationFunctionType
ALU = mybir.AluOpType
AX = mybir.AxisListType

# channel groups for the gating conv matmul: 8 groups of <=6 channels
GROUPS = [list(range(g, min(g + 6, C))) for g in range(0, C, 6)]
BIG_BUFS = 2


def _ap(base, extra_free):
    """Custom free-dim access pattern on an SBUF tile slice.

    base: AP from tile[:, a:b]; extra_free: list of [step,count] replacing
    the free dims (partition dim kept)."""
    return bass.AP(tensor=base.tensor, offset=base.offset,
                   ap=[base.ap[0]] + extra_free)


def build_gate_program(repeat=1):
    nc = bass.Bass()
    x = nc.declare_dram_parameter("x", [BC, CL], F32, isOutput=False)
    # per-group block-diag gc_w^T (rows: (i,l) pairs), bias rows separate
    wblk = nc.declare_dram_parameter("wblk", [126, len(GROUPS) * 126], F32,
                                     isOutput=False)
    wbias = nc.declare_dram_parameter("wbias", [1, len(GROUPS) * 126], F32,
                                      isOutput=False)
    w1mx = nc.declare_dram_parameter("w1mx", [C, 25], F32, isOutput=False)
    w1av = nc.declare_dram_parameter("w1av", [C, 25], F32, isOutput=False)
    b1r = nc.declare_dram_parameter("b1r", [1, 25], F32, isOutput=False)
    w2 = nc.declare_dram_parameter("w2", [25, C], F32, isOutput=False)
    b2r = nc.declare_dram_parameter("b2r", [1, C], F32, isOutput=False)
    ident = nc.declare_dram_parameter("ident", [P, P], F32, isOutput=False)
    gate_o = nc.declare_dram_parameter("gate", [BC, C], F32, isOutput=True)
    gsum_o = nc.declare_dram_parameter("gsum", [C, 1], F32, isOutput=True)

    with tile.TileContext(nc) as tc, ExitStack() as ctx:
        singles = ctx.enter_context(tc.tile_pool(name="singles", bufs=1))
        xs = ctx.enter_context(tc.tile_pool(name="xs", bufs=2))
        work = ctx.enter_context(tc.tile_pool(name="work", bufs=3))
        small = ctx.enter_context(tc.tile_pool(name="small", bufs=4))
        ps = ctx.enter_context(tc.tile_pool(name="ps", bufs=2, space="PSUM"))
        psm = ctx.enter_context(tc.tile_pool(name="psm", bufs=1, space="PSUM"))
        pst = ctx.enter_context(tc.tile_pool(name="pst", bufs=1, space="PSUM"))
        pss = ctx.enter_context(tc.tile_pool(name="pss", bufs=1, space="PSUM"))

        # All PE-read tensors funnel through DVE so every matmul needs at
        # most one sync wait (fp32 self-loading matmul ISA limit).
        def dve_const(dram, p, n):
            raw = singles.tile([p, n], F32, name="raw_" + dram.name)
            nc.sync.dma_start(out=raw, in_=dram[:, :])
            t = singles.tile([p, n], F32, name="sb_" + dram.name)
            nc.vector.tensor_copy(out=t, in_=raw)
            return t

        sb_id = dve_const(ident, P, P)
        sb_wblk = dve_const(wblk, 126, len(GROUPS) * 126)
        sb_wbias = dve_const(wbias, 1, len(GROUPS) * 126)
        sb_w1mx = dve_const(w1mx, C, 25)
        sb_w1av = dve_const(w1av, C, 25)
        sb_b1r = dve_const(b1r, 1, 25)
        sb_w2 = dve_const(w2, 25, C)
        sb_b2r = dve_const(b2r, 1, C)
        ones_col = singles.tile([P, 1], F32)
        nc.vector.memset(ones_col, 1.0)
        ones_row = singles.tile([1, P], F32)
        nc.vector.memset(ones_row, 1.0)
        # dummy PE op: advances PE's observed DVE clock past the consts
        warm_ps = pss.tile([1, P], F32)
        nc.tensor.transpose(warm_ps, ones_col, sb_id)

        gsum_ps = pss.tile([C, 1], F32)

        def mlp_branch(h_sb, w1_sb):
            """h_sb (P,45) -> tanh((tanh(h@w1+b1))@w2+b2) as (P,45) SBUF."""
            hT_ps = psm.tile([C, P], F32, tag="mlpT")
            nc.tensor.transpose(hT_ps, h_sb, sb_id)
            hT = work.tile([C, P], F32, tag="hT_sb")
            nc.vector.tensor_copy(out=hT, in_=hT_ps)
            p1 = psm.tile([P, 25], F32, tag="mlpP")
            nc.tensor.matmul(p1, hT, w1_sb, start=True, stop=False)
            nc.tensor.matmul(p1, ones_row, sb_b1r, start=False, stop=True)
            p1c = small.tile([P, 25], F32, tag="p1c")
            nc.vector.tensor_copy(out=p1c, in_=p1)
            t1 = small.tile([P, 25], F32, tag="t1")
            nc.scalar.activation(out=t1, in_=p1c, func=AF.Tanh)
            t1d = small.tile([P, 25], F32, tag="t1d")
            nc.vector.tensor_copy(out=t1d, in_=t1)
            t1T_ps = psm.tile([25, P], F32, tag="mlpT")
            nc.tensor.transpose(t1T_ps, t1d, sb_id)
            t1T = work.tile([25, P], F32, tag="t1T_sb")
            nc.vector.tensor_copy(out=t1T, in_=t1T_ps)
            p2 = psm.tile([P, C], F32, tag="mlpP")
            nc.tensor.matmul(p2, t1T, sb_w2, start=True, stop=False)
            nc.tensor.matmul(p2, ones_row, sb_b2r, start=False, stop=True)
            p2c = small.tile([P, C], F32, tag="p2c")
            nc.vector.tensor_copy(out=p2c, in_=p2)
            z = small.tile([P, C], F32, tag="z")
            nc.scalar.activation(out=z, in_=p2c, func=AF.Tanh)
            return z

        for t in range(NT):
            xt = xs.tile([P, CL], F32)
            nc.sync.dma_start(out=xt, in_=x[t * P:(t + 1) * P, :])

            for _r in range(repeat):
                # conv: per channel-group transpose + block-diag matmul
                temp_ps = [pst.tile([P, 504], F32, tag="tempA", name="tempA"),
                           pst.tile([P, 504], F32, tag="tempB", name="tempB")]
                for g, chans in enumerate(GROUPS):
                    w = len(chans) * L  # 126 or 63
                    xT_ps = ps.tile([126, P], F32, tag="xT")
                    nc.tensor.transpose(xT_ps[0:w, :],
                                        xt[:, chans[0] * L:chans[0] * L + w],
                                        sb_id)
                    lhs = work.tile([126, P], F32, tag="lhs")
                    nc.vector.tensor_copy(out=lhs[0:w, :], in_=xT_ps[0:w, :])
                    half, slot = divmod(g, 4)
                    dst = temp_ps[half][:, slot * 126:(slot + 1) * 126]
                    nc.tensor.matmul(dst, lhs[0:w, :],
                                     sb_wblk[0:w, g * 126:(g + 1) * 126],
                                     start=True, stop=False)
                    nc.tensor.matmul(dst, ones_row,
                                     sb_wbias[:, g * 126:(g + 1) * 126],
                                     start=False, stop=True)

                # mx/av pools over the 21 conv output channels
                mx = small.tile([P, 48], F32, tag="mx")
                av = small.tile([P, 48], F32, tag="av")
                for half in range(2):
                    src = _ap(temp_ps[half][:, 0:504], [[126, 4], [21, 6], [1, L]])
                    nc.vector.tensor_reduce(out=mx[:, half * 24:half * 24 + 24],
                                            in_=src, axis=AX.X, op=ALU.max)
                    nc.vector.tensor_reduce(out=av[:, half * 24:half * 24 + 24],
                                            in_=src, axis=AX.X, op=ALU.add)

                zmx = mlp_branch(mx[:, 0:C], sb_w1mx)
                zav = mlp_branch(av[:, 0:C], sb_w1av)
                z = small.tile([P, C], F32, tag="zsum")
                nc.vector.tensor_add(out=z, in0=zmx, in1=zav)

                # softmax over the 45 channels
                m1 = small.tile([P, 1], F32, tag="m1")
                nc.vector.tensor_reduce(out=m1, in_=z, axis=AX.X, op=ALU.max)
                nm = small.tile([P, 1], F32, tag="nm")
                nc.vector.tensor_scalar_mul(out=nm, in0=m1, scalar1=-1.0)
                eg = small.tile([P, C], F32, tag="eg")
                ssum = small.tile([P, 1], F32, tag="ssum")
                nc.scalar.activation(out=eg, in_=z, func=AF.Exp, bias=nm,
                                     accum_out=ssum)
                rs = small.tile([P, 1], F32, tag="rs")
                nc.vector.reciprocal(out=rs, in_=ssum)
                gt = small.tile([P, C], F32, tag="gt")
                nc.vector.tensor_scalar_mul(out=gt, in0=eg, scalar1=rs)
                nc.sync.dma_start(out=gate_o[t * P:(t + 1) * P, :], in_=gt)

                nc.tensor.matmul(gsum_ps, gt, ones_col,
                                 start=(t == 0), stop=(t == NT - 1))


        gs_sb = singles.tile([C, 1], F32)
        nc.vector.tensor_copy(out=gs_sb, in_=gsum_ps)
        nc.sync.dma_start(out=gsum_o[:, :], in_=gs_sb)
    legalize_sync_waits(nc)
    return nc


def build_attn_program(sel, repeat=1, pool_experts=0):
    """sel: sorted list of 22 selected channels (python ints, baked in).
    repeat>1 re-runs the compute body (same I/O) for timing isolation.
    pool_experts: how many experts' big elementwise muls run on GpSimd
    (pool) instead of DVE, to parallelize the two engines."""
    # runs of consecutive channels -> contiguous slices in both x and expert idx
    runs = []  # (chan0, e0, len)
    i = 0
    while i < E:
        j = i
        while j + 1 < E and sel[j + 1] == sel[j] + 1:
            j += 1
        runs.append((sel[i], i, j - i + 1))
        i = j + 1

    EL = E * L            # 462
    ELM = E * L * L       # 9702

    nc = bass.Bass()
    x = nc.declare_dram_parameter("x", [BC, CL], F32, isOutput=False)
    gsel = nc.declare_dram_parameter("gsel", [BC, E], F32, isOutput=False)
    avec = nc.declare_dram_parameter("avec", [EL], F32, isOutput=False)
    gvec = nc.declare_dram_parameter("gvec", [EL], F32, isOutput=False)
    pvec = nc.declare_dram_parameter("pvec", [E], F32, isOutput=False)
    qvec = nc.declare_dram_parameter("qvec", [E], F32, isOutput=False)
    a_o = nc.declare_dram_parameter("asel", [BC, EL], F32, isOutput=True)
    g_o = nc.declare_dram_parameter("gout", [BC, EL], F32, isOutput=True)

    with tile.TileContext(nc) as tc, ExitStack() as ctx:
        singles = ctx.enter_context(tc.tile_pool(name="singles", bufs=1))
        xs = ctx.enter_context(tc.tile_pool(name="xs", bufs=2))
        big = ctx.enter_context(tc.tile_pool(name="big", bufs=BIG_BUFS))
        mid = ctx.enter_context(tc.tile_pool(name="mid", bufs=2))
        outs = ctx.enter_context(tc.tile_pool(name="outs", bufs=2))

        def bconst(dram, n):
            base = dram[:]
            t = singles.tile([P, n], F32, name="bc_" + dram.name)
            nc.gpsimd.dma_start(
                out=t, in_=bass.AP(tensor=base.tensor, offset=base.offset,
                                   ap=[[0, P], [1, n]]))
            return t

        aB = bconst(avec, EL)
        gB = bconst(gvec, EL)
        pB = bconst(pvec, E)
        qB = bconst(qvec, E)

        for t in range(NT):
            xt = xs.tile([P, CL], F32)
            nc.sync.dma_start(out=xt, in_=x[t * P:(t + 1) * P, :])
            gs = xs.tile([P, E], F32, tag="gs")
            nc.sync.dma_start(out=gs, in_=gsel[t * P:(t + 1) * P, :])

            for _r in range(repeat):
                # gather the 22 selected channels once; all later ops contiguous
                u = mid.tile([P, EL], F32, tag="u")
                for (c0, e0, n) in runs:
                    nc.vector.tensor_copy(out=u[:, e0 * L:(e0 + n) * L],
                                          in_=xt[:, c0 * L:(c0 + n) * L])

                # kappa[b,(e,l)] = a_e * u[b,e,l] + g_e
                kap = mid.tile([P, EL], F32, tag="kap")
                nc.vector.tensor_mul(out=kap, in0=u, in1=aB)
                nc.gpsimd.tensor_add(out=kap, in0=kap, in1=gB)

                # energy[b,(e,l,m)] = kappa[b,e,l] * u[b,e,m]; exp in place
                en = big.tile([P, ELM], F32, tag="en")
                ed = E - pool_experts  # experts on DVE

                def en_split(op_dve, op_pool):
                    for eng_mul, e0, ne in ((op_dve, 0, ed),
                                            (op_pool, ed, E - ed)):
                        if ne == 0:
                            continue
                        o = _ap(en[:, e0 * L * L:(e0 + ne) * L * L],
                                [[L * L, ne], [L, L], [1, L]])
                        kl = _ap(kap[:, e0 * L:(e0 + ne) * L],
                                 [[L, ne], [1, L], [0, L]])
                        um = _ap(u[:, e0 * L:(e0 + ne) * L],
                                 [[L, ne], [0, L], [1, L]])
                        eng_mul(o, kl, um)

                def mul_en_kap(o, kl, um):
                    nc.vector.tensor_mul(out=o, in0=kl, in1=um)

                def mul_en_kap_pool(o, kl, um):
                    nc.gpsimd.tensor_mul(out=o, in0=kl, in1=um)

                en_split(mul_en_kap, mul_en_kap_pool)
                nc.scalar.activation(out=en, in_=en, func=AF.Exp)

                den = mid.tile([P, EL], F32, tag="den")
                nc.vector.tensor_reduce(
                    out=den, in_=_ap(en[:, 0:ELM], [[L, EL], [1, L]]),
                    axis=AX.X, op=ALU.add)

                # en <- en * u_m (numerator weights), then reduce
                def mul_num(e0, ne):
                    o = _ap(en[:, e0 * L * L:(e0 + ne) * L * L],
                            [[L * L, ne], [L, L], [1, L]])
                    um = _ap(u[:, e0 * L:(e0 + ne) * L],
                             [[L, ne], [0, L], [1, L]])
                    return o, um

                o, um = mul_num(0, ed)
                nc.vector.tensor_mul(out=o, in0=o, in1=um)
                if E - ed:
                    o, um = mul_num(ed, E - ed)
                    nc.gpsimd.tensor_mul(out=o, in0=o, in1=um)
                num = mid.tile([P, EL], F32, tag="num")
                nc.vector.tensor_reduce(
                    out=num, in_=_ap(en[:, 0:ELM], [[L, EL], [1, L]]),
                    axis=AX.X, op=ALU.add)

                nc.vector.reciprocal(out=den, in_=den)
                nc.gpsimd.tensor_mul(out=num, in0=num, in1=den)  # s

                # A = s * (gate*P)_rep + (gate*Q)_rep ; G = A * u
                gp = mid.tile([P, E], F32, tag="gp")
                nc.gpsimd.tensor_mul(out=gp, in0=gs, in1=pB)
                gq = mid.tile([P, E], F32, tag="gq")
                nc.gpsimd.tensor_mul(out=gq, in0=gs, in1=qB)
                at = outs.tile([P, EL], F32, tag="at")
                nc.gpsimd.tensor_mul(out=_ap(at[:, 0:EL], [[L, E], [1, L]]),
                                     in0=_ap(num[:, 0:EL], [[L, E], [1, L]]),
                                     in1=_ap(gp[:, 0:E], [[1, E], [0, L]]))
                nc.gpsimd.tensor_add(out=_ap(at[:, 0:EL], [[L, E], [1, L]]),
                                     in0=_ap(at[:, 0:EL], [[L, E], [1, L]]),
                                     in1=_ap(gq[:, 0:E], [[1, E], [0, L]]))
                gt = outs.tile([P, EL], F32, tag="gt")
                nc.vector.tensor_mul(out=gt, in0=at, in1=u)
                nc.sync.dma_start(out=a_o[t * P:(t + 1) * P, :], in_=at)
                nc.sync.dma_start(out=g_o[t * P:(t + 1) * P, :], in_=gt)
    legalize_sync_waits(nc)
    return nc


def _host_params(inputs):
    gc_w, gc_b = inputs["gc_w"], inputs["gc_b"]
    ng = len(GROUPS)
    wblk = np.zeros((126, ng * 126), np.float32)
    wbias = np.zeros((1, ng * 126), np.float32)
    for g, chans in enumerate(GROUPS):
        for k, _ in enumerate(chans):
            c0 = g * 126 + k * L
            wblk[k * L:(k + 1) * L, c0:c0 + L] = gc_w.T
            wbias[0, c0:c0 + L] = gc_b
    w1mx = inputs["w1"].T.astype(np.float32)
    w1av = (inputs["w1"].T / L).astype(np.float32)
    b1r = inputs["b1"][None, :].astype(np.float32)
    w2 = inputs["w2"].T.astype(np.float32)
    b2r = inputs["b2"][None, :].astype(np.float32)
    return wblk, wbias, w1mx, w1av, b1r, w2, b2r


_CACHE = {}


def kernel(**inputs):
    inputs = {k: np.ascontiguousarray(np.asarray(v)) for k, v in inputs.items()}
    x = inputs["x"].astype(np.float32).reshape(B, CL)
    wblk, wbias, w1mx, w1av, b1r, w2, b2r = _host_params(inputs)
    ident = np.eye(P, dtype=np.float32)
    cores = list(range(NCORES))

    if "gate" not in _CACHE:
        _CACHE["gate"] = build_gate_program()
    nc1 = _CACHE["gate"]
    maps1 = [{"x": x[i * BC:(i + 1) * BC], "wblk": wblk, "wbias": wbias,
              "w1mx": w1mx, "w1av": w1av, "b1r": b1r, "w2": w2, "b2r": b2r,
              "ident": ident} for i in cores]
    r1 = run_bass_kernel_spmd(nc1, maps1, cores).results
    gate = np.concatenate([r["gate"] for r in r1], 0)          # (B,45)
    mean_gate = np.sum([r["gsum"][:, 0] for r in r1], 0) / B   # (45,)
    sel = np.sort(np.argsort(-mean_gate, kind="stable")[:E])

    wq, bq = inputs["wq"], inputs["bq"]
    wk, bk = inputs["wk"], inputs["bk"]
    wv, bv = inputs["wv"], inputs["bv"]
    wo, bo = inputs["wo"], inputs["bo"]
    alpha = (wq * wk).sum(1).astype(np.float32)
    gamma = (bq * wk).sum(1).astype(np.float32)
    pv = (wo * wv).sum(1).astype(np.float32)
    qv = ((wo * bv).sum(1) + bo).astype(np.float32)
    avec = np.repeat(alpha, L)
    gvec = np.repeat(gamma, L)
    gsel = np.ascontiguousarray(gate[:, sel])

    key = tuple(sel.tolist())
    if _CACHE.get("attn_key") != key:
        _CACHE["attn"] = build_attn_program([int(s) for s in sel],
                                            pool_experts=8)
        _CACHE["attn_key"] = key
    nc2 = _CACHE["attn"]
    maps2 = [{"x": x[i * BC:(i + 1) * BC], "gsel": gsel[i * BC:(i + 1) * BC],
              "avec": avec, "gvec": gvec, "pvec": pv, "qvec": qv}
             for i in cores]
    r2 = run_bass_kernel_spmd(nc2, maps2, cores).results
    asel = np.concatenate([r["asel"] for r in r2], 0)          # (B,462)
    gout = np.concatenate([r["gout"] for r in r2], 0)

    cols = (np.repeat(sel * L, L) + np.tile(np.arange(L), E))  # (462,)
    A_full = np.zeros((B, CL), np.float32)
    G_full = np.zeros((B, CL), np.float32)
    A_full[:, cols] = asel
    G_full[:, cols] = gout
    return G_full, A_full



# revision 11
# speedup vs baseline: 2.5248x; 2.5248x over previous
"""Trainium2 Bass kernel for grouped-attention MoE routing.

Math (derived from the nn.Module):
  gate  = softmax(mlp(maxpool(conv(x))) + mlp(avgpool(conv(x))))      (B,45)
  sel   = sorted(top22(mean_b gate))                                  (22,)
  Per expert e with u = x[:, sel[e], :]:
    energy[l,m] = (a_e*u_l + g_e) * u_m   (rank-1; scalars a,g from weights)
    attn = softmax_m(energy);  s_l = sum_m u_m attn[l,m]
    y_l  = P_e*s_l + Q_e;      A[:,sel[e],:] = y * gate[:,sel[e]]
  G = x * A (flat);  return (G, A_flat)

Key optimization: with k = a*u_l + g, the softmax row sums are
  den_l = sum_m e^{k u_m},  num_l = sum_m u_m e^{k u_m}.
Approximating e^z by a degree-J Chebyshev fit P(z) = sum_j d_j z^j on the
realized z-range turns both into polynomials in k with power-sum
coefficients: den = sum_j d_j k^j S_j, num = sum_j d_j k^j S_{j+1}, where
S_j = sum_m u_m^j.  This replaces the O(L^2) energy tensor with O(L*J)
work (J=6 gives ~1e-3 end-to-end error vs the 2e-2 gate).

Layout: fp16 everywhere on the elementwise path with e (expert) innermost
so every scalar_tensor_tensor op hits the DVE 4x perf mode; power sums via
a pairwise tree reduce; Horner evaluation of num|den jointly (num and den
share the multiply-by-k steps and the immediate d_j coefficients once u is
pre-scaled by 1/2 on the host).

Strategy: pure data parallel over batch on 8 cores; two launches with the
45-float routing reduction mediated on host (equivalent of the all-reduce).
"""

import numpy as np
import ml_dtypes
from contextlib import ExitStack

import bass_rust
import concourse.bass as bass
import concourse.mybir as mybir
import concourse.tile as tile
from concourse.bass_utils import run_bass_kernel_spmd

_MULTIWAIT_OK = ("InstNoOp", "InstAllEngineBarrier",
                 "InstEventSemaphore", "InstUnconditionalBranch")


def legalize_sync_waits(nc):
    """walrus codegen on this stack rejects >1 sync wait on most
    instructions; hoist extra waits onto same-engine NoOps."""
    for func in nc.m.functions:
        for block in func.blocks:
            il = block.instructions
            out = []
            for inst in il:
                tname = type(inst).__name__
                si = getattr(inst, "sync_info", None)
                waits = list(si.on_wait) if si is not None else []
                if tname not in _MULTIWAIT_OK and len(waits) > 1:
                    for k, w in enumerate(waits):
                        nop = mybir.InstNoOp(
                            name=f"{inst.name}-synop{k}", ins=[], outs=[])
                        nop.engine = inst.engine
                        nop.sync_info = bass_rust.SyncInfo(
                            on_wait=[w], on_update=[])
                        out.append(nop)
                    inst.sync_info = bass_rust.SyncInfo(
                        on_wait=[], on_update=list(inst.sync_info.on_update))
                out.append(inst)
            il.clear()
            il.extend(out)


B, C, L, E = 8192, 45, 21, 22
NCORES = 8
BC = B // NCORES          # rows per core
P = 128                   # SBUF partitions
NT = BC // P              # batch tiles per core
CL = C * L                # 945
EL = E * L                # 462
J = 5                     # exp-approx polynomial degree
F32 = mybir.dt.float32
F16 = mybir.dt.float16
BF16 = mybir.dt.bfloat16
AF = mybir.ActivationFunctionType
ALU = mybir.AluOpType
AX = mybir.AxisListType
BYP = ALU.bypass

# channel groups for the gating conv matmul: 8 groups of <=6 channels
GROUPS = [list(range(g, min(g + 6, C))) for g in range(0, C, 6)]
NG = len(GROUPS)          # 8
GROWS = 127               # rows per chunk in the host-packed transposed x


def _ap(base, extra_free):
    """Custom free-dim access pattern on a tile slice: keep the partition
    dim of `base`, replace the free dims."""
    return bass.AP(tensor=base.tensor, offset=base.offset,
                   ap=[base.ap[0]] + extra_free)


def _dram_ap(dram, offset, ap):
    base = dram[:, :] if len(dram.shape) > 1 else dram[:]
    return bass.AP(tensor=base.tensor, offset=base.offset + offset, ap=ap)


def build_gate_program():
    """Gating network. x arrives host-transposed as 8 row-chunks of 127
    (6 channels x 21 taps + a ones row for bias), bf16.  Conv + avg-pool
    ride the PE as block-diagonal matmuls; max-pool on DVE/Pool; the MLP
    runs transposed (bias via per-partition activation bias) so only two
    PE transposes per branch are needed; softmax skips the max-subtract
    (|z| <= 2 by construction). Output gate in fp16."""
    nc = bass.Bass()
    xg = nc.declare_dram_parameter("xg", [NG * GROWS, BC], BF16,
                                   isOutput=False)
    wcat = nc.declare_dram_parameter("wcat", [GROWS, CL], BF16,
                                     isOutput=False)
    wav = nc.declare_dram_parameter("wav", [GROWS, C], BF16, isOutput=False)
    w1a = nc.declare_dram_parameter("w1a", [C, 25], BF16, isOutput=False)
    b1c = nc.declare_dram_parameter("b1c", [25, 1], F32, isOutput=False)
    w2a = nc.declare_dram_parameter("w2a", [25, C], BF16, isOutput=False)
    b2c = nc.declare_dram_parameter("b2c", [C, 1], F32, isOutput=False)
    identb = nc.declare_dram_parameter("identb", [P, P], BF16, isOutput=False)
    gate_o = nc.declare_dram_parameter("gate", [BC, C], F16, isOutput=True)

    # per-group geometry: (chunk row base, data rows, out-col base, n chans)
    geo = []
    cb = 0
    for g, chans in enumerate(GROUPS):
        nch = len(chans)
        geo.append((g * GROWS, nch * L, cb, nch))
        cb += nch * L

    with tile.TileContext(nc) as tc, ExitStack() as ctx:
        singles = ctx.enter_context(tc.tile_pool(name="singles", bufs=1))
        xs = ctx.enter_context(tc.tile_pool(name="xs", bufs=2))
        hw = ctx.enter_context(tc.tile_pool(name="hw", bufs=2))
        sm = ctx.enter_context(tc.tile_pool(name="sm", bufs=3))
        ps = ctx.enter_context(tc.tile_pool(name="ps", bufs=2, space="PSUM"))
        psm = ctx.enter_context(tc.tile_pool(name="psm", bufs=1, space="PSUM"))

        # PE-read consts funnel through DVE (one-wait matmul constraint);
        # warm-up transpose advances PE's observed DVE clock past them.
        def dve_const(dram, p, n, dt):
            raw = singles.tile([p, n], dt, name="raw_" + dram.name)
            nc.sync.dma_start(out=raw, in_=dram[:, :])
            t = singles.tile([p, n], dt, name="sb_" + dram.name)
            nc.vector.tensor_copy(out=t, in_=raw)
            return t

        sb_id = dve_const(identb, P, P, BF16)
        sb_wcat = dve_const(wcat, GROWS, CL, BF16)
        sb_wav = dve_const(wav, GROWS, C, BF16)
        sb_w1a = dve_const(w1a, C, 25, BF16)
        sb_w2a = dve_const(w2a, 25, C, BF16)
        sb_b1c = dve_const(b1c, 25, 1, F32)
        sb_b2c = dve_const(b2c, C, 1, F32)
        ones_col = singles.tile([P, 1], BF16)
        nc.vector.memset(ones_col, 1.0)
        warm_ps = psm.tile([C, P], BF16, tag="hT")
        nc.tensor.transpose(warm_ps[0:1, :], ones_col, sb_id)

        for t in range(NT):
            # one DMA: all 8 transposed chunks side by side (127, 8*128)
            xgt = xs.tile([GROWS, NG * P], BF16, tag="xgt")
            nc.sync.dma_start(
                out=xgt[:, :],
                in_=_dram_ap(xg, t * P,
                             [[BC, GROWS], [GROWS * BC, NG], [1, P]]))

            # conv: out (128, 21*nch) per group; halves 504 + 441 cols
            tp0 = ps.tile([P, 504], F32, tag="tp0")
            tp1 = ps.tile([P, 486], F32, tag="tp1")  # 441 conv + 45 avg cols
            av_ps = tp1[:, 441:486]
            for g, (rbase, rdata, cbase, nch) in enumerate(geo):
                half, dst = (0, tp0) if g < 4 else (1, tp1)
                c0 = cbase - (0 if g < 4 else 504)
                lhs = xgt[0:rdata + 1, g * P:(g + 1) * P]
                nc.tensor.matmul(dst[:, c0:c0 + nch * L], lhs,
                                 sb_wcat[0:rdata + 1, cbase:cbase + nch * L],
                                 start=True, stop=True)
                cav = sum(len(ch) for ch in GROUPS[:g])
                nc.tensor.matmul(tp1[:, 441 + cav:441 + cav + nch], lhs,
                                 sb_wav[0:rdata + 1, cav:cav + nch],
                                 start=True, stop=True)

            # max-pool over the 21 conv outputs per channel -> bf16
            mxh = sm.tile([P, 48], BF16, tag="mxh")
            nc.vector.tensor_reduce(
                out=mxh[:, 0:24], in_=_ap(tp0[:, 0:504], [[126, 4], [21, 6], [1, L]]),
                axis=AX.X, op=ALU.max)
            nc.vector.tensor_reduce(
                out=mxh[:, 24:42], in_=_ap(tp1[:, 0:378], [[126, 3], [21, 6], [1, L]]),
                axis=AX.X, op=ALU.max)
            nc.vector.tensor_reduce(
                out=mxh[:, 42:45], in_=_ap(tp1[:, 378:441], [[21, 3], [1, L]]),
                axis=AX.X, op=ALU.max)
            # avg-pool came out of the PE directly; copy+cast to bf16
            avh = sm.tile([P, C], BF16, tag="avh")
            nc.scalar.activation(out=avh, in_=tp1[:, 441:486], func=AF.Copy)

            # MLP, transposed: p1T = w1a^T . hT ; tanh(+b1); p2T; tanh(+b2)
            zTs = []
            for br, h in enumerate((mxh, avh)):
                hT_ps = psm.tile([C, P], BF16, tag="hT")
                nc.tensor.transpose(hT_ps, h[:, 0:C], sb_id)
                hT = hw.tile([C, P], BF16, tag=f"hT{br}")
                if br == 0:
                    nc.scalar.activation(out=hT, in_=hT_ps, func=AF.Copy)
                else:
                    nc.vector.tensor_copy(out=hT, in_=hT_ps)
                p1T_ps = psm.tile([25, P], F32, tag="p1T")
                nc.tensor.matmul(p1T_ps, sb_w1a, hT, start=True, stop=True)
                t1T = hw.tile([25, P], BF16, tag=f"t1T{br}")
                nc.scalar.activation(out=t1T, in_=p1T_ps, func=AF.Tanh,
                                     bias=sb_b1c)
                p2T_ps = psm.tile([C, P], F32, tag="p2T")
                nc.tensor.matmul(p2T_ps, sb_w2a, t1T, start=True, stop=True)
                zT = hw.tile([C, P], BF16, tag=f"zT{br}")
                nc.scalar.activation(out=zT, in_=p2T_ps, func=AF.Tanh,
                                     bias=sb_b2c)
                zTs.append(zT)

            # sum the branches in SBUF, then one transpose back (the PE
            # transpose path does not accumulate in PSUM)
            zTsum = hw.tile([C, P], BF16, tag="zTsum")
            nc.vector.tensor_add(out=zTsum, in0=zTs[0], in1=zTs[1])
            zsum_ps = psm.tile([P, C], BF16, tag="zsum")
            nc.tensor.transpose(zsum_ps, zTsum, sb_id[0:C, 0:C])

            # softmax over 45 channels; |z|<=2 so no max-subtract needed
            eg = sm.tile([P, C], F16, tag="eg")
            ssum = sm.tile([P, 1], F32, tag="ssum")
            nc.scalar.activation(out=eg, in_=zsum_ps, func=AF.Exp,
                                 accum_out=ssum)
            rs = sm.tile([P, 1], F32, tag="rs")
            nc.vector.reciprocal(out=rs, in_=ssum)
            gt = sm.tile([P, C], F16, tag="gt")
            nc.vector.tensor_scalar_mul(out=gt, in0=eg, scalar1=rs)
            nc.scalar.dma_start(out=gate_o[t * P:(t + 1) * P, :], in_=gt)
    legalize_sync_waits(nc)
    return nc


def build_attn_program(dj):
    """Rank-1 attention via the polynomial trick.  Inputs are fp16,
    l-major (element (l,e) at l*22+e), with u pre-scaled by 1/2 on the
    host so fp16 power sums cannot overflow; dj[j] = cheb_j * 2^j are the
    shared step immediates (the num half then computes num/2, folded
    into the 4*P_e gate coefficient together with the output-side 2x
    that the host strips off again).

    Engine notes: only plain TensorScalarPtr/TensorCopy get the DVE
    2x/4x modes (scalar_tensor_tensor gets none), and walrus limits TSP
    to 2 free dims, so the hot loop uses fp16 InstTensorTensor (2x_1p)
    with 3-free-dim access patterns:
    - power stack interleaved (l, j, e): the pairwise tree reduce over l
      runs on (l, row-contiguous) patterns and its last step lands
      S'_1..S'_7 directly in (j, e) order next to the memset S'_0;
    - per-step coefficient pairs [S'_j | S'_{j+1}]*dj are prebuilt by a
      single windowed TT against a repeated-dj constant; the num-half
      coefficients additionally absorb gate*4P_e so the Horner num
      output is already the gated numerator.
    J=5 keeps end-to-end error ~2e-3 against the 2e-2 gate."""
    nc = bass.Bass()
    xs16 = nc.declare_dram_parameter("xs16", [BC, EL], F16, isOutput=False)
    gs16 = nc.declare_dram_parameter("gs16", [BC, E], F16, isOutput=False)
    a2v = nc.declare_dram_parameter("a2v", [EL], F16, isOutput=False)
    gvv = nc.declare_dram_parameter("gvv", [EL], F16, isOutput=False)
    p2v = nc.declare_dram_parameter("p2v", [E], F16, isOutput=False)
    qvv = nc.declare_dram_parameter("qvv", [E], F16, isOutput=False)
    djv = nc.declare_dram_parameter("djv", [(J + 1) * 2 * E], F16,
                                    isOutput=False)
    o16 = nc.declare_dram_parameter("o16", [BC, 2 * EL], F16, isOutput=True)

    NJ = J + 1        # powers u'^1..u'^{J+1}
    ROW = NJ * E      # one l-row of the interleaved power stack

    with tile.TileContext(nc) as tc, ExitStack() as ctx:
        singles = ctx.enter_context(tc.tile_pool(name="singles", bufs=1))
        pstk = ctx.enter_context(tc.tile_pool(name="pstk", bufs=2))
        trp = ctx.enter_context(tc.tile_pool(name="trp", bufs=2))
        kp = ctx.enter_context(tc.tile_pool(name="kp", bufs=2))
        hp = ctx.enter_context(tc.tile_pool(name="hp", bufs=2))
        op = ctx.enter_context(tc.tile_pool(name="op", bufs=2))

        def bconst(dram, n):
            base = dram[:]
            t = singles.tile([P, n], F16, name="bc_" + dram.name)
            nc.gpsimd.dma_start(
                out=t, in_=bass.AP(tensor=base.tensor, offset=base.offset,
                                   ap=[[0, P], [1, n]]))
            return t

        a2B = bconst(a2v, EL)
        gB = bconst(gvv, EL)
        p2B = bconst(p2v, E)
        qB = bconst(qvv, E)
        djB = bconst(djv, (J + 1) * 2 * E)

        for t in range(NT):
            u0 = kp.tile([P, EL], F16, tag="u0")     # u' contiguous (l,e)
            nc.sync.dma_start(out=u0, in_=xs16[t * P:(t + 1) * P, :])
            gst = kp.tile([P, E], F16, tag="gst")
            nc.sync.dma_start(out=gst, in_=gs16[t * P:(t + 1) * P, :])

            # interleaved power stack (l, j, e); ACT copies u' to slot 1
            pst = pstk.tile([P, L * ROW], F16, tag="pst")

            def slot(j):          # (l, e) view of power j
                return _ap(pst[:, (j - 1) * E:], [[ROW, L], [1, E]])

            nc.scalar.activation(out=slot(1), in_=u0, func=AF.Copy)

            # kap = (2a)u' + g   (Pool)
            kt = kp.tile([P, EL], F16, tag="kt")
            nc.gpsimd.tensor_mul(out=kt, in0=u0, in1=a2B)
            kap = kp.tile([P, EL], F16, tag="kap")
            nc.gpsimd.tensor_add(out=kap, in0=kt, in1=gB)

            # powers u'^2..: squares on ACT, odd muls on DVE/Pool
            nc.scalar.activation(out=slot(2), in_=slot(1), func=AF.Square)
            nc.vector.tensor_mul(out=slot(3), in0=slot(2), in1=slot(1))
            nc.scalar.activation(out=slot(4), in_=slot(2), func=AF.Square)
            if NJ >= 5:
                nc.gpsimd.tensor_mul(out=slot(5), in0=slot(3), in1=slot(2))
            if NJ >= 6:
                nc.scalar.activation(out=slot(6), in_=slot(3), func=AF.Square)
            if NJ >= 7:
                nc.gpsimd.tensor_mul(out=slot(7), in0=slot(6), in1=slot(1))

            # pairwise tree over l; (j,e) stays contiguous throughout, so
            # the final step writes S'_1.. straight into the S-stack
            Sp = trp.tile([P, (NJ + 1) * E], F16, tag="Sp")
            nc.gpsimd.memset(Sp[:, 0:E], float(L))
            t1 = trp.tile([P, 10 * ROW], F16, tag="t1")
            t2 = trp.tile([P, 5 * ROW], F16, tag="t2")
            t3 = trp.tile([P, 2 * ROW], F16, tag="t3")
            t4 = trp.tile([P, ROW], F16, tag="t4")
            t5 = trp.tile([P, ROW], F16, tag="t5")

            pR = lambda l0, n: _ap(pst[:, l0 * ROW:], [[ROW, n], [1, ROW]])
            tR = lambda tl, l0, n: _ap(tl[:, l0 * ROW:], [[ROW, n], [1, ROW]])
            nc.vector.tensor_add(out=tR(t1, 0, 10), in0=pR(0, 10),
                                 in1=pR(10, 10))
            nc.vector.tensor_add(out=tR(t2, 0, 5), in0=tR(t1, 0, 5),
                                 in1=tR(t1, 5, 5))
            nc.vector.tensor_add(out=tR(t3, 0, 2), in0=tR(t2, 0, 2),
                                 in1=tR(t2, 2, 2))
            nc.vector.tensor_add(out=tR(t4, 0, 1), in0=tR(t3, 0, 1),
                                 in1=tR(t3, 1, 1))
            nc.vector.tensor_add(out=tR(t5, 0, 1), in0=tR(t4, 0, 1),
                                 in1=tR(t2, 4, 1))
            nc.vector.tensor_add(out=_ap(Sp[:, E:], [[1, ROW]]),
                                 in0=tR(t5, 0, 1), in1=pR(20, 1))

            # coefficient pairs SS_j = dj * [S'_j | S'_{j+1}] in one
            # windowed TT; then scale the num half by gate*4P_e
            SS = trp.tile([P, (J + 1) * 2 * E], F16, tag="SS")
            nc.vector.tensor_mul(
                out=_ap(SS[:, 0:], [[2 * E, J + 1], [1, 2 * E]]),
                in0=_ap(Sp[:, 0:], [[E, J + 1], [1, 2 * E]]),
                in1=_ap(djB[:, 0:], [[2 * E, J + 1], [1, 2 * E]]))
            gp = kp.tile([P, E], F16, tag="gp")
            nc.gpsimd.tensor_mul(out=gp, in0=gst, in1=p2B)
            nc.vector.tensor_mul(
                out=_ap(SS[:, E:], [[2 * E, J + 1], [1, E]]),
                in0=_ap(SS[:, E:], [[2 * E, J + 1], [1, E]]),
                in1=_ap(gp[:, 0:E], [[0, J + 1], [1, E]]))

            # joint Horner on [den | gated-num] with 3-free-dim TT ops
            ra = hp.tile([P, 2 * EL], F16, tag="ra")
            rb = hp.tile([P, 2 * EL], F16, tag="rb")
            f44 = lambda tl: _ap(tl[:, 0:], [[EL, 2], [E, L], [1, E]])
            cj = lambda j: _ap(SS[:, j * 2 * E:], [[E, 2], [0, L], [1, E]])
            kB = _ap(kap[:, 0:EL], [[0, 2], [E, L], [1, E]])
            cur, other = ra, rb
            nc.vector.tensor_mul(out=f44(cur), in0=cj(J), in1=kB)
            for j in range(J - 1, -1, -1):
                nc.vector.tensor_add(out=f44(other), in0=cj(j), in1=f44(cur))
                cur, other = other, cur
                if j > 0:
                    nc.vector.tensor_mul(out=f44(other), in0=f44(cur), in1=kB)
                    cur, other = other, cur

            # at2 = 2*A = gated-num/den + 2*gate*Q (host halves A on the
            # way out); gt = at2 * u' = A * x
            rd32 = op.tile([P, EL], F32, tag="rd32")
            nc.vector.reciprocal(out=rd32, in_=cur[:, 0:EL])
            atm = op.tile([P, EL], F16, tag="atm")
            nc.gpsimd.tensor_mul(out=atm, in0=cur[:, EL:2 * EL], in1=rd32)
            gq = kp.tile([P, E], F16, tag="gq")
            nc.gpsimd.tensor_mul(out=gq, in0=gst, in1=qB)
            ot = op.tile([P, 2 * EL], F16, tag="ot")
            gqB = _ap(gq[:, 0:E], [[0, L], [1, E]])
            nc.vector.tensor_add(out=ot[:, 0:EL], in0=atm, in1=gqB)
            nc.vector.tensor_mul(out=ot[:, EL:2 * EL], in0=ot[:, 0:EL],
                                 in1=u0)
            nc.scalar.dma_start(out=o16[t * P:(t + 1) * P, :], in_=ot)
    legalize_sync_waits(nc)
    return nc


def _gate_params(inputs):
    gc_w, gc_b = inputs["gc_w"], inputs["gc_b"]
    wbar = gc_w.mean(0)
    bbar = gc_b.mean()
    wcat = np.zeros((GROWS, CL), np.float32)
    wav = np.zeros((GROWS, C), np.float32)
    cb = 0
    for g, chans in enumerate(GROUPS):
        nch = len(chans)
        for k, c in enumerate(chans):
            wcat[k * L:(k + 1) * L, cb + k * L:cb + (k + 1) * L] = gc_w.T
            wav[k * L:(k + 1) * L, c] = wbar
            wav[nch * L, c] = bbar
        wcat[nch * L, cb:cb + nch * L] = np.tile(gc_b, nch)
        cb += nch * L
    bf = ml_dtypes.bfloat16
    return (wcat.astype(bf), wav.astype(bf),
            inputs["w1"].T.astype(bf).copy(),
            inputs["b1"][:, None].astype(np.float32).copy(),
            inputs["w2"].T.astype(bf).copy(),
            inputs["b2"][:, None].astype(np.float32).copy(),
            np.eye(P, dtype=np.float32).astype(bf))


_CACHE = {}


def kernel(**inputs):
    inputs = {k: np.ascontiguousarray(np.asarray(v)) for k, v in inputs.items()}
    x = inputs["x"].astype(np.float32)              # (B, C, L)
    bf = ml_dtypes.bfloat16
    cores = list(range(NCORES))

    # ---- launch 1: gate -------------------------------------------------
    wcat, wav, w1a, b1c, w2a, b2c, identb = _gate_params(inputs)
    # host-marshaled transposed x: 8 chunks of (6ch x 21 + ones row) x B
    xt = x.reshape(B, CL).T                          # (945, B)
    xg = np.zeros((NG * GROWS, B), np.float32)
    cb = 0
    for g, chans in enumerate(GROUPS):
        nch = len(chans)
        xg[g * GROWS:g * GROWS + nch * L] = xt[cb:cb + nch * L]
        xg[g * GROWS + nch * L] = 1.0
        cb += nch * L
    xg = xg.astype(bf)

    if "gate" not in _CACHE:
        _CACHE["gate"] = build_gate_program()
    nc1 = _CACHE["gate"]
    maps1 = [{"xg": np.ascontiguousarray(xg[:, i * BC:(i + 1) * BC]),
              "wcat": wcat, "wav": wav, "w1a": w1a, "b1c": b1c,
              "w2a": w2a, "b2c": b2c, "identb": identb} for i in cores]
    r1 = run_bass_kernel_spmd(nc1, maps1, cores).results
    gate16 = np.concatenate([np.asarray(r["gate"]) for r in r1], 0)  # (B,45)

    # ---- routing (host-mediated all-reduce) -----------------------------
    mean_gate = gate16.astype(np.float64).mean(0)
    sel = np.sort(np.argsort(-mean_gate, kind="stable")[:E])

    # ---- launch 2: attention -------------------------------------------
    wq, bq = inputs["wq"], inputs["bq"]
    wk, bk = inputs["wk"], inputs["bk"]
    wv, bv = inputs["wv"], inputs["bv"]
    wo, bo = inputs["wo"], inputs["bo"]
    alpha = (wq * wk).sum(1).astype(np.float64)
    gamma = (bq * wk).sum(1).astype(np.float64)
    pv = (wo * wv).sum(1).astype(np.float64)
    qv = ((wo * bv).sum(1) + bo).astype(np.float64)

    xsel = x[:, sel, :]                              # (B, E, L)
    umax = float(np.abs(xsel).max())
    zm = (np.abs(alpha).max() * umax + np.abs(gamma).max()) * umax
    cheb = np.polynomial.chebyshev.Chebyshev.interpolate(
        np.exp, J, domain=[-zm, zm])
    dc = cheb.convert(kind=np.polynomial.Polynomial).coef
    dj = [float(dc[j] * (2.0 ** j)) for j in range(J + 1)]

    key = tuple(np.round(dj, 12))
    if _CACHE.get("attn_key") != key:
        _CACHE["attn"] = build_attn_program(dj)
        _CACHE["attn_key"] = key
    nc2 = _CACHE["attn"]

    xs16 = np.ascontiguousarray(
        (xsel.transpose(0, 2, 1) * np.float32(0.5)).astype(np.float16)
        .reshape(B, EL))                             # l-major, u' = x/2
    gs16 = np.ascontiguousarray(gate16[:, sel])      # (B, 22) fp16
    a2vv = np.tile((2 * alpha).astype(np.float16), L)
    gvvv = np.tile(gamma.astype(np.float16), L)
    p2vv = (4 * pv).astype(np.float16)
    qvvv = (2 * qv).astype(np.float16)
    djvv = np.repeat(np.asarray(dj), 2 * E).astype(np.float16)
    maps2 = [{"xs16": xs16[i * BC:(i + 1) * BC],
              "gs16": gs16[i * BC:(i + 1) * BC],
              "a2v": a2vv, "gvv": gvvv, "p2v": p2vv, "qvv": qvvv,
              "djv": djvv}
             for i in cores]
    r2 = run_bass_kernel_spmd(nc2, maps2, cores).results
    o16 = np.concatenate([np.asarray(r["o16"]) for r in r2], 0)  # (B, 924)

    # ---- host unshard / scatter (device emits 2*A and A*x) -------------
    at = (o16[:, :EL].astype(np.float32) * 0.5).reshape(
        B, L, E).transpose(0, 2, 1)
    gt = o16[:, EL:].astype(np.float32).reshape(B, L, E).transpose(0, 2, 1)
    cols = (np.repeat(sel * L, L) + np.tile(np.arange(L), E))
    A_full = np.zeros((B, CL), np.float32)
    G_full = np.zeros((B, CL), np.float32)
    A_full[:, cols] = at.reshape(B, EL)
    G_full[:, cols] = gt.reshape(B, EL)
    return G_full, A_full


# revision 13
# speedup vs baseline: 2.7378x; 1.0844x over previous
"""Trainium2 Bass kernel for grouped-attention MoE routing.

Math (derived from the nn.Module):
  gate  = softmax(mlp(maxpool(conv(x))) + mlp(avgpool(conv(x))))      (B,45)
  sel   = sorted(top22(mean_b gate))                                  (22,)
  Per expert e with u = x[:, sel[e], :]:
    energy[l,m] = (a_e*u_l + g_e) * u_m   (rank-1; scalars a,g from weights)
    attn = softmax_m(energy);  s_l = sum_m u_m attn[l,m]
    y_l  = P_e*s_l + Q_e;      A[:,sel[e],:] = y * gate[:,sel[e]]
  G = x * A (flat);  return (G, A_flat)

Key optimization: with k = a*u_l + g, the softmax row sums are
  den_l = sum_m e^{k u_m},  num_l = sum_m u_m e^{k u_m}.
Approximating e^z by a degree-J Chebyshev fit P(z) = sum_j d_j z^j on the
realized z-range turns both into polynomials in k with power-sum
coefficients: den = sum_j d_j k^j S_j, num = sum_j d_j k^j S_{j+1}, where
S_j = sum_m u_m^j.  This replaces the O(L^2) energy tensor with O(L*J)
work (J=6 gives ~1e-3 end-to-end error vs the 2e-2 gate).

Layout: fp16 everywhere on the elementwise path with e (expert) innermost
so every scalar_tensor_tensor op hits the DVE 4x perf mode; power sums via
a pairwise tree reduce; Horner evaluation of num|den jointly (num and den
share the multiply-by-k steps and the immediate d_j coefficients once u is
pre-scaled by 1/2 on the host).

Strategy: pure data parallel over batch on 8 cores; two launches with the
45-float routing reduction mediated on host (equivalent of the all-reduce).
"""

import numpy as np
import ml_dtypes
from contextlib import ExitStack

import bass_rust
import concourse.bass as bass
import concourse.mybir as mybir
import concourse.tile as tile
from concourse.bass_utils import run_bass_kernel_spmd

_MULTIWAIT_OK = ("InstNoOp", "InstAllEngineBarrier",
                 "InstEventSemaphore", "InstUnconditionalBranch")


def legalize_sync_waits(nc):
    """walrus codegen on this stack rejects >1 sync wait on most
    instructions; hoist extra waits onto same-engine NoOps."""
    for func in nc.m.functions:
        for block in func.blocks:
            il = block.instructions
            out = []
            for inst in il:
                tname = type(inst).__name__
                si = getattr(inst, "sync_info", None)
                waits = list(si.on_wait) if si is not None else []
                if tname not in _MULTIWAIT_OK and len(waits) > 1:
                    for k, w in enumerate(waits):
                        nop = mybir.InstNoOp(
                            name=f"{inst.name}-synop{k}", ins=[], outs=[])
                        nop.engine = inst.engine
                        nop.sync_info = bass_rust.SyncInfo(
                            on_wait=[w], on_update=[])
                        out.append(nop)
                    inst.sync_info = bass_rust.SyncInfo(
                        on_wait=[], on_update=list(inst.sync_info.on_update))
                out.append(inst)
            il.clear()
            il.extend(out)


B, C, L, E = 8192, 45, 21, 22
NCORES = 8
BC = B // NCORES          # rows per core
P = 128                   # SBUF partitions
NT = BC // P              # batch tiles per core
CL = C * L                # 945
EL = E * L                # 462
J = 5                     # exp-approx polynomial degree
F32 = mybir.dt.float32
F16 = mybir.dt.float16
BF16 = mybir.dt.bfloat16
AF = mybir.ActivationFunctionType
ALU = mybir.AluOpType
AX = mybir.AxisListType
BYP = ALU.bypass

# channel groups for the gating conv matmul: 8 groups of <=6 channels
GROUPS = [list(range(g, min(g + 6, C))) for g in range(0, C, 6)]
NG = len(GROUPS)          # 8
GROWS = 127               # rows per chunk in the host-packed transposed x


def _ap(base, extra_free):
    """Custom free-dim access pattern on a tile slice: keep the partition
    dim of `base`, replace the free dims."""
    return bass.AP(tensor=base.tensor, offset=base.offset,
                   ap=[base.ap[0]] + extra_free)


def _dram_ap(dram, offset, ap):
    base = dram[:, :] if len(dram.shape) > 1 else dram[:]
    return bass.AP(tensor=base.tensor, offset=base.offset + offset, ap=ap)


def build_gate_program():
    """Gating network. x arrives host-transposed as 8 row-chunks of 127
    (6 channels x 21 taps + a ones row for bias), bf16.  Conv + avg-pool
    ride the PE as block-diagonal matmuls; max-pool on DVE/Pool; the MLP
    runs transposed (bias via per-partition activation bias) so only two
    PE transposes per branch are needed; softmax skips the max-subtract
    (|z| <= 2 by construction). Output gate in fp16."""
    nc = bass.Bass()
    xg = nc.declare_dram_parameter("xg", [NG * GROWS, BC], BF16,
                                   isOutput=False)
    wcat = nc.declare_dram_parameter("wcat", [GROWS, CL], BF16,
                                     isOutput=False)
    wav = nc.declare_dram_parameter("wav", [GROWS, C], BF16, isOutput=False)
    w1a = nc.declare_dram_parameter("w1a", [C, 25], BF16, isOutput=False)
    b1c = nc.declare_dram_parameter("b1c", [25, 1], F32, isOutput=False)
    w2a = nc.declare_dram_parameter("w2a", [25, C], BF16, isOutput=False)
    b2c = nc.declare_dram_parameter("b2c", [C, 1], F32, isOutput=False)
    identb = nc.declare_dram_parameter("identb", [P, P], BF16, isOutput=False)
    gate_o = nc.declare_dram_parameter("gate", [BC, C], F16, isOutput=True)

    # per-group geometry: (chunk row base, data rows, out-col base, n chans)
    geo = []
    cb = 0
    for g, chans in enumerate(GROUPS):
        nch = len(chans)
        geo.append((g * GROWS, nch * L, cb, nch))
        cb += nch * L

    with tile.TileContext(nc) as tc, ExitStack() as ctx:
        singles = ctx.enter_context(tc.tile_pool(name="singles", bufs=1))
        xs = ctx.enter_context(tc.tile_pool(name="xs", bufs=2))
        hw = ctx.enter_context(tc.tile_pool(name="hw", bufs=2))
        sm = ctx.enter_context(tc.tile_pool(name="sm", bufs=3))
        ps = ctx.enter_context(tc.tile_pool(name="ps", bufs=1, space="PSUM"))
        psm = ctx.enter_context(tc.tile_pool(name="psm", bufs=1, space="PSUM"))

        # PE-read consts funnel through DVE (one-wait matmul constraint);
        # warm-up transpose advances PE's observed DVE clock past them.
        def dve_const(dram, p, n, dt):
            raw = singles.tile([p, n], dt, name="raw_" + dram.name)
            nc.sync.dma_start(out=raw, in_=dram[:, :])
            t = singles.tile([p, n], dt, name="sb_" + dram.name)
            nc.vector.tensor_copy(out=t, in_=raw)
            return t

        sb_id = dve_const(identb, P, P, BF16)
        sb_wcat = dve_const(wcat, GROWS, CL, BF16)
        sb_wav = dve_const(wav, GROWS, C, BF16)
        sb_w1a = dve_const(w1a, C, 25, BF16)
        sb_w2a = dve_const(w2a, 25, C, BF16)
        sb_b1c = dve_const(b1c, 25, 1, F32)
        sb_b2c = dve_const(b2c, C, 1, F32)
        ones_col = singles.tile([P, 1], BF16)
        nc.vector.memset(ones_col, 1.0)
        warm_ps = psm.tile([C, P], BF16, tag="hT0")
        nc.tensor.transpose(warm_ps[0:1, :], ones_col, sb_id)

        for t in range(NT):
            # one DMA: all 8 transposed chunks side by side (127, 8*128)
            xgt = xs.tile([GROWS, NG * P], BF16, tag="xgt")
            nc.sync.dma_start(
                out=xgt[:, :],
                in_=_dram_ap(xg, t * P,
                             [[BC, GROWS], [GROWS * BC, NG], [1, P]]))

            # conv: out (128, 21*nch) per group; halves 504 + 441 cols
            tp0 = ps.tile([P, 504], F32, tag="tp0")
            tp1 = ps.tile([P, 486], F32, tag="tp1")  # 441 conv + 45 avg cols
            av_ps = tp1[:, 441:486]
            for g, (rbase, rdata, cbase, nch) in enumerate(geo):
                half, dst = (0, tp0) if g < 4 else (1, tp1)
                c0 = cbase - (0 if g < 4 else 504)
                lhs = xgt[0:rdata + 1, g * P:(g + 1) * P]
                nc.tensor.matmul(dst[:, c0:c0 + nch * L], lhs,
                                 sb_wcat[0:rdata + 1, cbase:cbase + nch * L],
                                 start=True, stop=True)
                cav = sum(len(ch) for ch in GROUPS[:g])
                nc.tensor.matmul(tp1[:, 441 + cav:441 + cav + nch], lhs,
                                 sb_wav[0:rdata + 1, cav:cav + nch],
                                 start=True, stop=True)

            # max-pool over the 21 conv outputs per channel -> bf16
            mxh = sm.tile([P, 48], BF16, tag="mxh")
            nc.vector.tensor_reduce(
                out=mxh[:, 0:24], in_=_ap(tp0[:, 0:504], [[126, 4], [21, 6], [1, L]]),
                axis=AX.X, op=ALU.max)
            nc.vector.tensor_reduce(
                out=mxh[:, 24:42], in_=_ap(tp1[:, 0:378], [[126, 3], [21, 6], [1, L]]),
                axis=AX.X, op=ALU.max)
            nc.vector.tensor_reduce(
                out=mxh[:, 42:45], in_=_ap(tp1[:, 378:441], [[21, 3], [1, L]]),
                axis=AX.X, op=ALU.max)
            # avg-pool came out of the PE directly; copy+cast to bf16
            avh = sm.tile([P, C], BF16, tag="avh")
            nc.scalar.activation(out=avh, in_=tp1[:, 441:486], func=AF.Copy)

            # MLP, transposed: p1T = w1a^T . hT ; tanh(+b1); p2T; tanh(+b2)
            zTs = []
            for br, h in enumerate((mxh, avh)):
                hT_ps = psm.tile([C, P], BF16, tag=f"hT{br}")
                nc.tensor.transpose(hT_ps, h[:, 0:C], sb_id)
                hT = hw.tile([C, P], BF16, tag=f"hT{br}")
                if br == 0:
                    nc.scalar.activation(out=hT, in_=hT_ps, func=AF.Copy)
                else:
                    nc.vector.tensor_copy(out=hT, in_=hT_ps)
                p1T_ps = psm.tile([25, P], F32, tag=f"p1T{br}")
                nc.tensor.matmul(p1T_ps, sb_w1a, hT, start=True, stop=True)
                t1T = hw.tile([25, P], BF16, tag=f"t1T{br}")
                nc.scalar.activation(out=t1T, in_=p1T_ps, func=AF.Tanh,
                                     bias=sb_b1c)
                p2T_ps = psm.tile([C, P], F32, tag="p2T")
                nc.tensor.matmul(p2T_ps, sb_w2a, t1T, start=True, stop=True)
                zT = hw.tile([C, P], BF16, tag=f"zT{br}")
                nc.scalar.activation(out=zT, in_=p2T_ps, func=AF.Tanh,
                                     bias=sb_b2c)
                zTs.append(zT)

            # sum the branches in SBUF, then one transpose back (the PE
            # transpose path does not accumulate in PSUM)
            zTsum = hw.tile([C, P], BF16, tag="zTsum")
            nc.vector.tensor_add(out=zTsum, in0=zTs[0], in1=zTs[1])
            zsum_ps = psm.tile([P, C], BF16, tag="zsum")
            nc.tensor.transpose(zsum_ps, zTsum, sb_id[0:C, 0:C])

            # softmax over 45 channels; |z|<=2 so no max-subtract needed
            eg = sm.tile([P, C], F16, tag="eg")
            ssum = sm.tile([P, 1], F32, tag="ssum")
            nc.scalar.activation(out=eg, in_=zsum_ps, func=AF.Exp,
                                 accum_out=ssum)
            rs = sm.tile([P, 1], F32, tag="rs")
            nc.vector.reciprocal(out=rs, in_=ssum)
            gt = sm.tile([P, C], F16, tag="gt")
            nc.vector.tensor_scalar_mul(out=gt, in0=eg, scalar1=rs)
            nc.scalar.dma_start(out=gate_o[t * P:(t + 1) * P, :], in_=gt)
    legalize_sync_waits(nc)
    return nc


def build_attn_program(dj):
    """Rank-1 attention via the polynomial trick.  Inputs are fp16,
    l-major (element (l,e) at l*22+e), with u pre-scaled by 1/2 on the
    host so fp16 power sums cannot overflow; dj[j] = cheb_j * 2^j are the
    shared step immediates (the num half then computes num/2, folded
    into the 4*P_e gate coefficient together with the output-side 2x
    that the host strips off again).

    Engine notes: only plain TensorScalarPtr/TensorCopy get the DVE
    2x/4x modes (scalar_tensor_tensor gets none), and walrus limits TSP
    to 2 free dims, so the hot loop uses fp16 InstTensorTensor (2x_1p)
    with 3-free-dim access patterns:
    - power stack interleaved (l, j, e): the pairwise tree reduce over l
      runs on (l, row-contiguous) patterns and its last step lands
      S'_1..S'_7 directly in (j, e) order next to the memset S'_0;
    - per-step coefficient pairs [S'_j | S'_{j+1}]*dj are prebuilt by a
      single windowed TT against a repeated-dj constant; the num-half
      coefficients additionally absorb gate*4P_e so the Horner num
      output is already the gated numerator.
    J=5 keeps end-to-end error ~2e-3 against the 2e-2 gate."""
    nc = bass.Bass()
    xsg = nc.declare_dram_parameter("xsg", [BC, EL + E], F16, isOutput=False)
    a2v = nc.declare_dram_parameter("a2v", [EL], F16, isOutput=False)
    gvv = nc.declare_dram_parameter("gvv", [EL], F16, isOutput=False)
    p2v = nc.declare_dram_parameter("p2v", [E], F16, isOutput=False)
    qvv = nc.declare_dram_parameter("qvv", [E], F16, isOutput=False)
    djv = nc.declare_dram_parameter("djv", [(J + 1) * 2 * E], F16,
                                    isOutput=False)
    o16 = nc.declare_dram_parameter("o16", [BC, 2 * EL], F16, isOutput=True)

    NJ = J + 1        # powers u'^1..u'^{J+1}
    ROW = NJ * E      # one l-row of the interleaved power stack

    with tile.TileContext(nc) as tc, ExitStack() as ctx:
        singles = ctx.enter_context(tc.tile_pool(name="singles", bufs=1))
        pstk = ctx.enter_context(tc.tile_pool(name="pstk", bufs=3))
        trp = ctx.enter_context(tc.tile_pool(name="trp", bufs=3))
        kp = ctx.enter_context(tc.tile_pool(name="kp", bufs=3))
        hp = ctx.enter_context(tc.tile_pool(name="hp", bufs=3))
        op = ctx.enter_context(tc.tile_pool(name="op", bufs=3))

        def bconst(dram, n):
            base = dram[:]
            t = singles.tile([P, n], F16, name="bc_" + dram.name)
            nc.gpsimd.dma_start(
                out=t, in_=bass.AP(tensor=base.tensor, offset=base.offset,
                                   ap=[[0, P], [1, n]]))
            return t

        a2B = bconst(a2v, EL)
        gB = bconst(gvv, EL)
        p2B = bconst(p2v, E)
        qB = bconst(qvv, E)
        djB = bconst(djv, (J + 1) * 2 * E)

        for t in range(NT):
            ug = kp.tile([P, EL + E], F16, tag="ug")  # [u' | gate_sel]
            nc.sync.dma_start(out=ug, in_=xsg[t * P:(t + 1) * P, :])
            u0 = ug[:, 0:EL]
            gst = ug[:, EL:EL + E]

            # interleaved power stack (l, j, e); ACT copies u' to slot 1
            pst = pstk.tile([P, L * ROW], F16, tag="pst")

            def slot(j):          # (l, e) view of power j
                return _ap(pst[:, (j - 1) * E:], [[ROW, L], [1, E]])

            nc.scalar.activation(out=slot(1), in_=u0, func=AF.Copy)

            # kap = (2a)u' + g   (Pool)
            kt = kp.tile([P, EL], F16, tag="kt")
            nc.gpsimd.tensor_mul(out=kt, in0=u0, in1=a2B)
            kap = kp.tile([P, EL], F16, tag="kap")
            nc.gpsimd.tensor_add(out=kap, in0=kt, in1=gB)

            # powers u'^2..: squares on ACT, odd muls on DVE/Pool
            nc.scalar.activation(out=slot(2), in_=slot(1), func=AF.Square)
            nc.gpsimd.tensor_mul(out=slot(3), in0=slot(2), in1=slot(1))
            nc.scalar.activation(out=slot(4), in_=slot(2), func=AF.Square)
            if NJ >= 5:
                nc.gpsimd.tensor_mul(out=slot(5), in0=slot(3), in1=slot(2))
            if NJ >= 6:
                nc.scalar.activation(out=slot(6), in_=slot(3), func=AF.Square)
            if NJ >= 7:
                nc.gpsimd.tensor_mul(out=slot(7), in0=slot(6), in1=slot(1))

            # pairwise tree over l; (j,e) stays contiguous throughout, so
            # the final step writes S'_1.. straight into the S-stack
            Sp = trp.tile([P, (NJ + 1) * E], F16, tag="Sp")
            nc.gpsimd.memset(Sp[:, 0:E], float(L))
            t1 = trp.tile([P, 10 * ROW], F16, tag="t1")
            t2 = trp.tile([P, 5 * ROW], F16, tag="t2")
            t3 = trp.tile([P, 2 * ROW], F16, tag="t3")
            t4 = trp.tile([P, ROW], F16, tag="t4")
            t5 = trp.tile([P, ROW], F16, tag="t5")

            pR = lambda l0, n: _ap(pst[:, l0 * ROW:], [[ROW, n], [1, ROW]])
            tR = lambda tl, l0, n: _ap(tl[:, l0 * ROW:], [[ROW, n], [1, ROW]])
            nc.vector.tensor_add(out=tR(t1, 0, 10), in0=pR(0, 10),
                                 in1=pR(10, 10))
            nc.vector.tensor_add(out=tR(t2, 0, 5), in0=tR(t1, 0, 5),
                                 in1=tR(t1, 5, 5))
            nc.vector.tensor_add(out=tR(t3, 0, 2), in0=tR(t2, 0, 2),
                                 in1=tR(t2, 2, 2))
            nc.vector.tensor_add(out=tR(t4, 0, 1), in0=tR(t3, 0, 1),
                                 in1=tR(t3, 1, 1))
            nc.vector.tensor_add(out=tR(t5, 0, 1), in0=tR(t4, 0, 1),
                                 in1=tR(t2, 4, 1))
            nc.vector.tensor_add(out=_ap(Sp[:, E:], [[1, ROW]]),
                                 in0=tR(t5, 0, 1), in1=pR(20, 1))

            # coefficient pairs SS_j = dj * [S'_j | S'_{j+1}] in one
            # windowed TT; then scale the num half by gate*4P_e
            SS = trp.tile([P, (J + 1) * 2 * E], F16, tag="SS")
            nc.vector.tensor_mul(
                out=_ap(SS[:, 0:], [[2 * E, J + 1], [1, 2 * E]]),
                in0=_ap(Sp[:, 0:], [[E, J + 1], [1, 2 * E]]),
                in1=_ap(djB[:, 0:], [[2 * E, J + 1], [1, 2 * E]]))
            gp = kp.tile([P, E], F16, tag="gp")
            nc.gpsimd.tensor_mul(out=gp, in0=gst, in1=p2B)
            nc.vector.tensor_mul(
                out=_ap(SS[:, E:], [[2 * E, J + 1], [1, E]]),
                in0=_ap(SS[:, E:], [[2 * E, J + 1], [1, E]]),
                in1=_ap(gp[:, 0:E], [[0, J + 1], [1, E]]))

            # joint Horner on [den | gated-num] with 3-free-dim TT ops
            ra = hp.tile([P, 2 * EL], F16, tag="ra")
            rb = hp.tile([P, 2 * EL], F16, tag="rb")
            f44 = lambda tl: _ap(tl[:, 0:], [[EL, 2], [E, L], [1, E]])
            cj = lambda j: _ap(SS[:, j * 2 * E:], [[E, 2], [0, L], [1, E]])
            kB = _ap(kap[:, 0:EL], [[0, 2], [E, L], [1, E]])
            cur, other = ra, rb
            nc.vector.tensor_mul(out=f44(cur), in0=cj(J), in1=kB)
            for j in range(J - 1, -1, -1):
                nc.vector.tensor_add(out=f44(other), in0=cj(j), in1=f44(cur))
                cur, other = other, cur
                if j > 0:
                    nc.vector.tensor_mul(out=f44(other), in0=f44(cur), in1=kB)
                    cur, other = other, cur

            # at2 = 2*A = gated-num/den + 2*gate*Q (host halves A on the
            # way out); gt = at2 * u' = A * x
            rd32 = op.tile([P, EL], F32, tag="rd32")
            nc.vector.reciprocal(out=rd32, in_=cur[:, 0:EL])
            atm = op.tile([P, EL], F16, tag="atm")
            nc.gpsimd.tensor_mul(out=atm, in0=cur[:, EL:2 * EL], in1=rd32)
            gq = kp.tile([P, E], F16, tag="gq")
            nc.gpsimd.tensor_mul(out=gq, in0=gst, in1=qB)
            ot = op.tile([P, 2 * EL], F16, tag="ot")
            gqB = _ap(gq[:, 0:E], [[0, L], [1, E]])
            nc.vector.tensor_add(out=ot[:, 0:EL], in0=atm, in1=gqB)
            nc.vector.tensor_mul(out=ot[:, EL:2 * EL], in0=ot[:, 0:EL],
                                 in1=u0)
            nc.scalar.dma_start(out=o16[t * P:(t + 1) * P, :], in_=ot)
    legalize_sync_waits(nc)
    return nc


def _gate_params(inputs):
    gc_w, gc_b = inputs["gc_w"], inputs["gc_b"]
    wbar = gc_w.mean(0)
    bbar = gc_b.mean()
    wcat = np.zeros((GROWS, CL), np.float32)
    wav = np.zeros((GROWS, C), np.float32)
    cb = 0
    for g, chans in enumerate(GROUPS):
        nch = len(chans)
        for k, c in enumerate(chans):
            wcat[k * L:(k + 1) * L, cb + k * L:cb + (k + 1) * L] = gc_w.T
            wav[k * L:(k + 1) * L, c] = wbar
            wav[nch * L, c] = bbar
        wcat[nch * L, cb:cb + nch * L] = np.tile(gc_b, nch)
        cb += nch * L
    bf = ml_dtypes.bfloat16
    return (wcat.astype(bf), wav.astype(bf),
            inputs["w1"].T.astype(bf).copy(),
            inputs["b1"][:, None].astype(np.float32).copy(),
            inputs["w2"].T.astype(bf).copy(),
            inputs["b2"][:, None].astype(np.float32).copy(),
            np.eye(P, dtype=np.float32).astype(bf))


_CACHE = {}


def kernel(**inputs):
    inputs = {k: np.ascontiguousarray(np.asarray(v)) for k, v in inputs.items()}
    x = inputs["x"].astype(np.float32)              # (B, C, L)
    bf = ml_dtypes.bfloat16
    cores = list(range(NCORES))

    # ---- launch 1: gate -------------------------------------------------
    wcat, wav, w1a, b1c, w2a, b2c, identb = _gate_params(inputs)
    # host-marshaled transposed x: 8 chunks of (6ch x 21 + ones row) x B
    xt = x.reshape(B, CL).T                          # (945, B)
    xg = np.zeros((NG * GROWS, B), np.float32)
    cb = 0
    for g, chans in enumerate(GROUPS):
        nch = len(chans)
        xg[g * GROWS:g * GROWS + nch * L] = xt[cb:cb + nch * L]
        xg[g * GROWS + nch * L] = 1.0
        cb += nch * L
    xg = xg.astype(bf)

    if "gate" not in _CACHE:
        _CACHE["gate"] = build_gate_program()
    nc1 = _CACHE["gate"]
    maps1 = [{"xg": np.ascontiguousarray(xg[:, i * BC:(i + 1) * BC]),
              "wcat": wcat, "wav": wav, "w1a": w1a, "b1c": b1c,
              "w2a": w2a, "b2c": b2c, "identb": identb} for i in cores]
    r1 = run_bass_kernel_spmd(nc1, maps1, cores).results
    gate16 = np.concatenate([np.asarray(r["gate"]) for r in r1], 0)  # (B,45)

    # ---- routing (host-mediated all-reduce) -----------------------------
    mean_gate = gate16.astype(np.float64).mean(0)
    sel = np.sort(np.argsort(-mean_gate, kind="stable")[:E])

    # ---- launch 2: attention -------------------------------------------
    wq, bq = inputs["wq"], inputs["bq"]
    wk, bk = inputs["wk"], inputs["bk"]
    wv, bv = inputs["wv"], inputs["bv"]
    wo, bo = inputs["wo"], inputs["bo"]
    alpha = (wq * wk).sum(1).astype(np.float64)
    gamma = (bq * wk).sum(1).astype(np.float64)
    pv = (wo * wv).sum(1).astype(np.float64)
    qv = ((wo * bv).sum(1) + bo).astype(np.float64)

    xsel = x[:, sel, :]                              # (B, E, L)
    umax = float(np.abs(xsel).max())
    zm = (np.abs(alpha).max() * umax + np.abs(gamma).max()) * umax
    cheb = np.polynomial.chebyshev.Chebyshev.interpolate(
        np.exp, J, domain=[-zm, zm])
    dc = cheb.convert(kind=np.polynomial.Polynomial).coef
    dj = [float(dc[j] * (2.0 ** j)) for j in range(J + 1)]

    key = tuple(np.round(dj, 12))
    if _CACHE.get("attn_key") != key:
        _CACHE["attn"] = build_attn_program(dj)
        _CACHE["attn_key"] = key
    nc2 = _CACHE["attn"]

    xsg = np.empty((B, EL + E), np.float16)          # [u' l-major | gate_sel]
    xsg[:, :EL] = (xsel.transpose(0, 2, 1) * np.float32(0.5)
                   ).astype(np.float16).reshape(B, EL)
    xsg[:, EL:] = gate16[:, sel]
    a2vv = np.tile((2 * alpha).astype(np.float16), L)
    gvvv = np.tile(gamma.astype(np.float16), L)
    p2vv = (4 * pv).astype(np.float16)
    qvvv = (2 * qv).astype(np.float16)
    djvv = np.repeat(np.asarray(dj), 2 * E).astype(np.float16)
    maps2 = [{"xsg": xsg[i * BC:(i + 1) * BC],
              "a2v": a2vv, "gvv": gvvv, "p2v": p2vv, "qvv": qvvv,
              "djv": djvv}
             for i in cores]
    r2 = run_bass_kernel_spmd(nc2, maps2, cores).results
    o16 = np.concatenate([np.asarray(r["o16"]) for r in r2], 0)  # (B, 924)

    # ---- host unshard / scatter (device emits 2*A and A*x) -------------
    at = (o16[:, :EL].astype(np.float32) * 0.5).reshape(
        B, L, E).transpose(0, 2, 1)
    gt = o16[:, EL:].astype(np.float32).reshape(B, L, E).transpose(0, 2, 1)
    cols = (np.repeat(sel * L, L) + np.tile(np.arange(L), E))
    A_full = np.zeros((B, CL), np.float32)
    G_full = np.zeros((B, CL), np.float32)
    A_full[:, cols] = at.reshape(B, EL)
    G_full[:, cols] = gt.reshape(B, EL)
    return G_full, A_full


# revision 15
# speedup vs baseline: 2.7461x; 1.0030x over previous
"""Trainium2 Bass kernel for grouped-attention MoE routing.

Math (derived from the nn.Module):
  gate  = softmax(mlp(maxpool(conv(x))) + mlp(avgpool(conv(x))))      (B,45)
  sel   = sorted(top22(mean_b gate))                                  (22,)
  Per expert e with u = x[:, sel[e], :]:
    energy[l,m] = (a_e*u_l + g_e) * u_m   (rank-1; scalars a,g from weights)
    attn = softmax_m(energy);  s_l = sum_m u_m attn[l,m]
    y_l  = P_e*s_l + Q_e;      A[:,sel[e],:] = y * gate[:,sel[e]]
  G = x * A (flat);  return (G, A_flat)

Key optimization: with k = a*u_l + g, the softmax row sums are
  den_l = sum_m e^{k u_m},  num_l = sum_m u_m e^{k u_m}.
Approximating e^z by a degree-J Chebyshev fit P(z) = sum_j d_j z^j on the
realized z-range turns both into polynomials in k with power-sum
coefficients: den = sum_j d_j k^j S_j, num = sum_j d_j k^j S_{j+1}, where
S_j = sum_m u_m^j.  This replaces the O(L^2) energy tensor with O(L*J)
work (J=6 gives ~1e-3 end-to-end error vs the 2e-2 gate).

Layout: fp16 everywhere on the elementwise path with e (expert) innermost
so every scalar_tensor_tensor op hits the DVE 4x perf mode; power sums via
a pairwise tree reduce; Horner evaluation of num|den jointly (num and den
share the multiply-by-k steps and the immediate d_j coefficients once u is
pre-scaled by 1/2 on the host).

Strategy: pure data parallel over batch on 8 cores; two launches with the
45-float routing reduction mediated on host (equivalent of the all-reduce).
"""

import numpy as np
import ml_dtypes
from contextlib import ExitStack

import bass_rust
import concourse.bass as bass
import concourse.mybir as mybir
import concourse.tile as tile
from concourse.bass_utils import run_bass_kernel_spmd

_MULTIWAIT_OK = ("InstNoOp", "InstAllEngineBarrier",
                 "InstEventSemaphore", "InstUnconditionalBranch")


def legalize_sync_waits(nc):
    """walrus codegen on this stack rejects >1 sync wait on most
    instructions; hoist extra waits onto same-engine NoOps."""
    for func in nc.m.functions:
        for block in func.blocks:
            il = block.instructions
            out = []
            for inst in il:
                tname = type(inst).__name__
                si = getattr(inst, "sync_info", None)
                waits = list(si.on_wait) if si is not None else []
                if tname not in _MULTIWAIT_OK and len(waits) > 1:
                    for k, w in enumerate(waits):
                        nop = mybir.InstNoOp(
                            name=f"{inst.name}-synop{k}", ins=[], outs=[])
                        nop.engine = inst.engine
                        nop.sync_info = bass_rust.SyncInfo(
                            on_wait=[w], on_update=[])
                        out.append(nop)
                    inst.sync_info = bass_rust.SyncInfo(
                        on_wait=[], on_update=list(inst.sync_info.on_update))
                out.append(inst)
            il.clear()
            il.extend(out)


B, C, L, E = 8192, 45, 21, 22
NCORES = 8
BC = B // NCORES          # rows per core
P = 128                   # SBUF partitions
NT = BC // P              # batch tiles per core
CL = C * L                # 945
EL = E * L                # 462
J = 5                     # exp-approx polynomial degree
F32 = mybir.dt.float32
F16 = mybir.dt.float16
BF16 = mybir.dt.bfloat16
AF = mybir.ActivationFunctionType
ALU = mybir.AluOpType
AX = mybir.AxisListType
BYP = ALU.bypass

# channel groups for the gating conv matmul: 8 groups of <=6 channels
GROUPS = [list(range(g, min(g + 6, C))) for g in range(0, C, 6)]
NG = len(GROUPS)          # 8
GROWS = 127               # rows per chunk in the host-packed transposed x


def _ap(base, extra_free):
    """Custom free-dim access pattern on a tile slice: keep the partition
    dim of `base`, replace the free dims."""
    return bass.AP(tensor=base.tensor, offset=base.offset,
                   ap=[base.ap[0]] + extra_free)


def _dram_ap(dram, offset, ap):
    base = dram[:, :] if len(dram.shape) > 1 else dram[:]
    return bass.AP(tensor=base.tensor, offset=base.offset + offset, ap=ap)


def build_gate_program():
    """Gating network. x arrives host-transposed as 8 row-chunks of 127
    (6 channels x 21 taps + a ones row for bias), bf16.  Conv + avg-pool
    ride the PE as block-diagonal matmuls into a single bf16 PSUM bank
    (double-buffered); max-pool on DVE; the MLP runs transposed (bias via
    per-partition activation bias) with per-branch PSUM banks so the two
    branches and adjacent tiles overlap; softmax skips the max-subtract
    (|z| <= 2 by construction). Output gate in fp16."""
    nc = bass.Bass()
    xg = nc.declare_dram_parameter("xg", [NG * GROWS, BC], BF16,
                                   isOutput=False)
    wcat = nc.declare_dram_parameter("wcat", [GROWS, CL], BF16,
                                     isOutput=False)
    wav = nc.declare_dram_parameter("wav", [GROWS, C], BF16, isOutput=False)
    w1a = nc.declare_dram_parameter("w1a", [C, 25], BF16, isOutput=False)
    b1c = nc.declare_dram_parameter("b1c", [25, 1], F32, isOutput=False)
    w2a = nc.declare_dram_parameter("w2a", [25, C], BF16, isOutput=False)
    b2c = nc.declare_dram_parameter("b2c", [C, 1], F32, isOutput=False)
    identb = nc.declare_dram_parameter("identb", [P, P], BF16, isOutput=False)
    gate_o = nc.declare_dram_parameter("gate", [BC, C], F16, isOutput=True)

    # per-group geometry: (chunk row base, data rows, out-col base, n chans)
    geo = []
    cb = 0
    for g, chans in enumerate(GROUPS):
        nch = len(chans)
        geo.append((g * GROWS, nch * L, cb, nch))
        cb += nch * L

    with tile.TileContext(nc) as tc, ExitStack() as ctx:
        singles = ctx.enter_context(tc.tile_pool(name="singles", bufs=1))
        xs = ctx.enter_context(tc.tile_pool(name="xs", bufs=3))
        hw = ctx.enter_context(tc.tile_pool(name="hw", bufs=3))
        sm = ctx.enter_context(tc.tile_pool(name="sm", bufs=3))
        ps = ctx.enter_context(tc.tile_pool(name="ps", bufs=2, space="PSUM"))
        psm = ctx.enter_context(tc.tile_pool(name="psm", bufs=1, space="PSUM"))

        # PE-read consts funnel through DVE (one-wait matmul constraint);
        # warm-up transpose advances PE's observed DVE clock past them.
        def dve_const(dram, p, n, dt):
            raw = singles.tile([p, n], dt, name="raw_" + dram.name)
            nc.sync.dma_start(out=raw, in_=dram[:, :])
            t = singles.tile([p, n], dt, name="sb_" + dram.name)
            nc.vector.tensor_copy(out=t, in_=raw)
            return t

        sb_id = dve_const(identb, P, P, BF16)
        sb_wcat = dve_const(wcat, GROWS, CL, BF16)
        sb_wav = dve_const(wav, GROWS, C, BF16)
        sb_w1a = dve_const(w1a, C, 25, BF16)
        sb_w2a = dve_const(w2a, 25, C, BF16)
        sb_b1c = dve_const(b1c, 25, 1, F32)
        sb_b2c = dve_const(b2c, C, 1, F32)
        ones_col = singles.tile([P, 1], BF16)
        nc.vector.memset(ones_col, 1.0)
        warm_ps = psm.tile([C, P], BF16, tag="hT")
        nc.tensor.transpose(warm_ps[0:1, :], ones_col, sb_id)

        for t in range(NT):
            # one DMA: all 8 transposed chunks side by side (127, 8*128)
            xgt = xs.tile([GROWS, NG * P], BF16, tag="xgt")
            nc.sync.dma_start(
                out=xgt[:, :],
                in_=_dram_ap(xg, t * P,
                             [[BC, GROWS], [GROWS * BC, NG], [1, P]]))

            # conv + avg into one PSUM tile: cols [0:945) conv,
            # [945:990) avg
            tp = ps.tile([P, CL + C], F32, tag="tp")
            for g, (rbase, rdata, cbase, nch) in enumerate(geo):
                lhs = xgt[0:rdata + 1, g * P:(g + 1) * P]
                nc.tensor.matmul(tp[:, cbase:cbase + nch * L], lhs,
                                 sb_wcat[0:rdata + 1, cbase:cbase + nch * L],
                                 start=True, stop=True)
                cav = sum(len(ch) for ch in GROUPS[:g])
                nc.tensor.matmul(tp[:, CL + cav:CL + cav + nch], lhs,
                                 sb_wav[0:rdata + 1, cav:cav + nch],
                                 start=True, stop=True)

            # max-pool over the 21 conv outputs per channel -> bf16
            mxh = sm.tile([P, 48], BF16, tag="mxh")
            nc.vector.tensor_reduce(
                out=mxh[:, 0:24],
                in_=_ap(tp[:, 0:504], [[126, 4], [21, 6], [1, L]]),
                axis=AX.X, op=ALU.max)
            nc.vector.tensor_reduce(
                out=mxh[:, 24:42],
                in_=_ap(tp[:, 504:882], [[126, 3], [21, 6], [1, L]]),
                axis=AX.X, op=ALU.max)
            nc.vector.tensor_reduce(
                out=mxh[:, 42:45],
                in_=_ap(tp[:, 882:945], [[21, 3], [1, L]]),
                axis=AX.X, op=ALU.max)
            # avg-pool came out of the PE directly; copy to bf16 SBUF
            avh = sm.tile([P, C], BF16, tag="avh")
            nc.scalar.activation(out=avh, in_=tp[:, CL:CL + C], func=AF.Copy)

            # MLP, transposed: p1T = w1a^T . hT ; tanh(+b1); p2T; tanh(+b2)
            zTs = []
            for br, h in enumerate((mxh, avh)):
                hT_ps = psm.tile([C, P], BF16, tag="hT")
                nc.tensor.transpose(hT_ps, h[:, 0:C], sb_id)
                hT = hw.tile([C, P], BF16, tag=f"hT{br}")
                if br == 0:
                    nc.scalar.activation(out=hT, in_=hT_ps, func=AF.Copy)
                else:
                    nc.vector.tensor_copy(out=hT, in_=hT_ps)
                pmlp = psm.tile([C, P], F32, tag=f"pmlp{br}")
                nc.tensor.matmul(pmlp[0:25, :], sb_w1a, hT,
                                 start=True, stop=True)
                t1T = hw.tile([25, P], BF16, tag=f"t1T{br}")
                nc.scalar.activation(out=t1T, in_=pmlp[0:25, :], func=AF.Tanh,
                                     bias=sb_b1c)
                p2T_ps = pmlp
                nc.tensor.matmul(p2T_ps, sb_w2a, t1T, start=True, stop=True)
                zT = hw.tile([C, P], BF16, tag=f"zT{br}")
                nc.scalar.activation(out=zT, in_=p2T_ps, func=AF.Tanh,
                                     bias=sb_b2c)
                zTs.append(zT)

            # sum the branches in SBUF, then one transpose back (the PE
            # transpose path does not accumulate in PSUM)
            zTsum = hw.tile([C, P], BF16, tag="zTsum")
            nc.vector.tensor_add(out=zTsum, in0=zTs[0], in1=zTs[1])
            zsum_ps = psm.tile([P, C], BF16, tag="zsum")
            nc.tensor.transpose(zsum_ps, zTsum, sb_id[0:C, 0:C])

            # softmax over 45 channels; |z|<=2 so no max-subtract needed
            eg = sm.tile([P, C], F16, tag="eg")
            ssum = sm.tile([P, 1], F32, tag="ssum")
            nc.scalar.activation(out=eg, in_=zsum_ps, func=AF.Exp,
                                 accum_out=ssum)
            rs = sm.tile([P, 1], F32, tag="rs")
            nc.vector.reciprocal(out=rs, in_=ssum)
            gt = sm.tile([P, C], F16, tag="gt")
            nc.vector.tensor_scalar_mul(out=gt, in0=eg, scalar1=rs)
            nc.scalar.dma_start(out=gate_o[t * P:(t + 1) * P, :], in_=gt)
    legalize_sync_waits(nc)
    return nc


def build_attn_program(dj):
    """Rank-1 attention via the polynomial trick.  Inputs are fp16,
    l-major (element (l,e) at l*22+e), with u pre-scaled by 1/2 on the
    host so fp16 power sums cannot overflow; dj[j] = cheb_j * 2^j are the
    shared step immediates (the num half then computes num/2, folded
    into the 4*P_e gate coefficient together with the output-side 2x
    that the host strips off again).

    Engine notes: only plain TensorScalarPtr/TensorCopy get the DVE
    2x/4x modes (scalar_tensor_tensor gets none), and walrus limits TSP
    to 2 free dims, so the hot loop uses fp16 InstTensorTensor (2x_1p)
    with 3-free-dim access patterns:
    - power stack interleaved (l, j, e): the pairwise tree reduce over l
      runs on (l, row-contiguous) patterns and its last step lands
      S'_1..S'_7 directly in (j, e) order next to the memset S'_0;
    - per-step coefficient pairs [S'_j | S'_{j+1}]*dj are prebuilt by a
      single windowed TT against a repeated-dj constant; the num-half
      coefficients additionally absorb gate*4P_e so the Horner num
      output is already the gated numerator.
    J=5 keeps end-to-end error ~2e-3 against the 2e-2 gate."""
    nc = bass.Bass()
    xsg = nc.declare_dram_parameter("xsg", [BC, EL + E], F16, isOutput=False)
    a2v = nc.declare_dram_parameter("a2v", [EL], F16, isOutput=False)
    gvv = nc.declare_dram_parameter("gvv", [EL], F16, isOutput=False)
    p2v = nc.declare_dram_parameter("p2v", [E], F16, isOutput=False)
    qvv = nc.declare_dram_parameter("qvv", [E], F16, isOutput=False)
    djv = nc.declare_dram_parameter("djv", [(J + 1) * 2 * E], F16,
                                    isOutput=False)
    o16 = nc.declare_dram_parameter("o16", [BC, 2 * EL], F16, isOutput=True)

    NJ = J + 1        # powers u'^1..u'^{J+1}
    ROW = NJ * E      # one l-row of the interleaved power stack

    with tile.TileContext(nc) as tc, ExitStack() as ctx:
        singles = ctx.enter_context(tc.tile_pool(name="singles", bufs=1))
        pstk = ctx.enter_context(tc.tile_pool(name="pstk", bufs=3))
        trp = ctx.enter_context(tc.tile_pool(name="trp", bufs=3))
        kp = ctx.enter_context(tc.tile_pool(name="kp", bufs=3))
        hp = ctx.enter_context(tc.tile_pool(name="hp", bufs=3))
        op = ctx.enter_context(tc.tile_pool(name="op", bufs=3))

        def bconst(dram, n):
            base = dram[:]
            t = singles.tile([P, n], F16, name="bc_" + dram.name)
            nc.gpsimd.dma_start(
                out=t, in_=bass.AP(tensor=base.tensor, offset=base.offset,
                                   ap=[[0, P], [1, n]]))
            return t

        a2B = bconst(a2v, EL)
        gB = bconst(gvv, EL)
        p2B = bconst(p2v, E)
        qB = bconst(qvv, E)
        djB = bconst(djv, (J + 1) * 2 * E)

        for t in range(NT):
            ug = kp.tile([P, EL + E], F16, tag="ug")  # [u' | gate_sel]
            nc.sync.dma_start(out=ug, in_=xsg[t * P:(t + 1) * P, :])
            u0 = ug[:, 0:EL]
            gst = ug[:, EL:EL + E]

            # interleaved power stack (l, j, e); ACT copies u' to slot 1
            pst = pstk.tile([P, L * ROW], F16, tag="pst")

            def slot(j):          # (l, e) view of power j
                return _ap(pst[:, (j - 1) * E:], [[ROW, L], [1, E]])

            nc.scalar.activation(out=slot(1), in_=u0, func=AF.Copy)

            # kap = (2a)u' + g   (Pool)
            kt = kp.tile([P, EL], F16, tag="kt")
            nc.gpsimd.tensor_mul(out=kt, in0=u0, in1=a2B)
            kap = kp.tile([P, EL], F16, tag="kap")
            nc.gpsimd.tensor_add(out=kap, in0=kt, in1=gB)

            # powers u'^2..: squares on ACT, odd muls on DVE/Pool
            nc.scalar.activation(out=slot(2), in_=slot(1), func=AF.Square)
            nc.gpsimd.tensor_mul(out=slot(3), in0=slot(2), in1=slot(1))
            nc.scalar.activation(out=slot(4), in_=slot(2), func=AF.Square)
            if NJ >= 5:
                nc.gpsimd.tensor_mul(out=slot(5), in0=slot(3), in1=slot(2))
            if NJ >= 6:
                nc.scalar.activation(out=slot(6), in_=slot(3), func=AF.Square)
            if NJ >= 7:
                nc.gpsimd.tensor_mul(out=slot(7), in0=slot(6), in1=slot(1))

            # pairwise tree over l; (j,e) stays contiguous throughout, so
            # the final step writes S'_1.. straight into the S-stack
            Sp = trp.tile([P, (NJ + 1) * E], F16, tag="Sp")
            nc.gpsimd.memset(Sp[:, 0:E], float(L))
            t1 = trp.tile([P, 10 * ROW], F16, tag="t1")
            t2 = trp.tile([P, 5 * ROW], F16, tag="t2")
            t3 = trp.tile([P, 2 * ROW], F16, tag="t3")
            t4 = trp.tile([P, ROW], F16, tag="t4")
            t5 = trp.tile([P, ROW], F16, tag="t5")

            pR = lambda l0, n: _ap(pst[:, l0 * ROW:], [[ROW, n], [1, ROW]])
            tR = lambda tl, l0, n: _ap(tl[:, l0 * ROW:], [[ROW, n], [1, ROW]])
            nc.vector.tensor_add(out=tR(t1, 0, 10), in0=pR(0, 10),
                                 in1=pR(10, 10))
            nc.vector.tensor_add(out=tR(t2, 0, 5), in0=tR(t1, 0, 5),
                                 in1=tR(t1, 5, 5))
            nc.vector.tensor_add(out=tR(t3, 0, 2), in0=tR(t2, 0, 2),
                                 in1=tR(t2, 2, 2))
            nc.vector.tensor_add(out=tR(t4, 0, 1), in0=tR(t3, 0, 1),
                                 in1=tR(t3, 1, 1))
            nc.vector.tensor_add(out=tR(t5, 0, 1), in0=tR(t4, 0, 1),
                                 in1=tR(t2, 4, 1))
            nc.vector.tensor_add(out=_ap(Sp[:, E:], [[1, ROW]]),
                                 in0=tR(t5, 0, 1), in1=pR(20, 1))

            # coefficient pairs SS_j = dj * [S'_j | S'_{j+1}] in one
            # windowed TT; then scale the num half by gate*4P_e
            SS = trp.tile([P, (J + 1) * 2 * E], F16, tag="SS")
            nc.vector.tensor_mul(
                out=_ap(SS[:, 0:], [[2 * E, J + 1], [1, 2 * E]]),
                in0=_ap(Sp[:, 0:], [[E, J + 1], [1, 2 * E]]),
                in1=_ap(djB[:, 0:], [[2 * E, J + 1], [1, 2 * E]]))
            gp = kp.tile([P, E], F16, tag="gp")
            nc.gpsimd.tensor_mul(out=gp, in0=gst, in1=p2B)
            nc.vector.tensor_mul(
                out=_ap(SS[:, E:], [[2 * E, J + 1], [1, E]]),
                in0=_ap(SS[:, E:], [[2 * E, J + 1], [1, E]]),
                in1=_ap(gp[:, 0:E], [[0, J + 1], [1, E]]))

            # joint Horner on [den | gated-num] with 3-free-dim TT ops
            ra = hp.tile([P, 2 * EL], F16, tag="ra")
            rb = hp.tile([P, 2 * EL], F16, tag="rb")
            f44 = lambda tl: _ap(tl[:, 0:], [[EL, 2], [E, L], [1, E]])
            cj = lambda j: _ap(SS[:, j * 2 * E:], [[E, 2], [0, L], [1, E]])
            kB = _ap(kap[:, 0:EL], [[0, 2], [E, L], [1, E]])
            cur, other = ra, rb
            nc.vector.tensor_mul(out=f44(cur), in0=cj(J), in1=kB)
            for j in range(J - 1, -1, -1):
                nc.vector.tensor_add(out=f44(other), in0=cj(j), in1=f44(cur))
                cur, other = other, cur
                if j > 0:
                    nc.vector.tensor_mul(out=f44(other), in0=f44(cur), in1=kB)
                    cur, other = other, cur

            # at2 = 2*A = gated-num/den + 2*gate*Q (host halves A on the
            # way out); gt = at2 * u' = A * x
            rd32 = op.tile([P, EL], F32, tag="rd32")
            nc.vector.reciprocal(out=rd32, in_=cur[:, 0:EL])
            atm = op.tile([P, EL], F16, tag="atm")
            nc.gpsimd.tensor_mul(out=atm, in0=cur[:, EL:2 * EL], in1=rd32)
            gq = kp.tile([P, E], F16, tag="gq")
            nc.gpsimd.tensor_mul(out=gq, in0=gst, in1=qB)
            ot = op.tile([P, 2 * EL], F16, tag="ot")
            gqB = _ap(gq[:, 0:E], [[0, L], [1, E]])
            nc.vector.tensor_add(out=ot[:, 0:EL], in0=atm, in1=gqB)
            nc.vector.tensor_mul(out=ot[:, EL:2 * EL], in0=ot[:, 0:EL],
                                 in1=u0)
            nc.scalar.dma_start(out=o16[t * P:(t + 1) * P, :], in_=ot)
    legalize_sync_waits(nc)
    return nc


def _gate_params(inputs):
    gc_w, gc_b = inputs["gc_w"], inputs["gc_b"]
    wbar = gc_w.mean(0)
    bbar = gc_b.mean()
    wcat = np.zeros((GROWS, CL), np.float32)
    wav = np.zeros((GROWS, C), np.float32)
    cb = 0
    for g, chans in enumerate(GROUPS):
        nch = len(chans)
        for k, c in enumerate(chans):
            wcat[k * L:(k + 1) * L, cb + k * L:cb + (k + 1) * L] = gc_w.T
            wav[k * L:(k + 1) * L, c] = wbar
            wav[nch * L, c] = bbar
        wcat[nch * L, cb:cb + nch * L] = np.tile(gc_b, nch)
        cb += nch * L
    bf = ml_dtypes.bfloat16
    return (wcat.astype(bf), wav.astype(bf),
            inputs["w1"].T.astype(bf).copy(),
            inputs["b1"][:, None].astype(np.float32).copy(),
            inputs["w2"].T.astype(bf).copy(),
            inputs["b2"][:, None].astype(np.float32).copy(),
            np.eye(P, dtype=np.float32).astype(bf))


_CACHE = {}


def kernel(**inputs):
    inputs = {k: np.ascontiguousarray(np.asarray(v)) for k, v in inputs.items()}
    x = inputs["x"].astype(np.float32)              # (B, C, L)
    bf = ml_dtypes.bfloat16
    cores = list(range(NCORES))

    # ---- launch 1: gate -------------------------------------------------
    wcat, wav, w1a, b1c, w2a, b2c, identb = _gate_params(inputs)
    # host-marshaled transposed x: 8 chunks of (6ch x 21 + ones row) x B
    xt = x.reshape(B, CL).T                          # (945, B)
    xg = np.zeros((NG * GROWS, B), np.float32)
    cb = 0
    for g, chans in enumerate(GROUPS):
        nch = len(chans)
        xg[g * GROWS:g * GROWS + nch * L] = xt[cb:cb + nch * L]
        xg[g * GROWS + nch * L] = 1.0
        cb += nch * L
    xg = xg.astype(bf)

    if "gate" not in _CACHE:
        _CACHE["gate"] = build_gate_program()
    nc1 = _CACHE["gate"]
    maps1 = [{"xg": np.ascontiguousarray(xg[:, i * BC:(i + 1) * BC]),
              "wcat": wcat, "wav": wav, "w1a": w1a, "b1c": b1c,
              "w2a": w2a, "b2c": b2c, "identb": identb} for i in cores]
    r1 = run_bass_kernel_spmd(nc1, maps1, cores).results
    gate16 = np.concatenate([np.asarray(r["gate"]) for r in r1], 0)  # (B,45)

    # ---- routing (host-mediated all-reduce) -----------------------------
    mean_gate = gate16.astype(np.float64).mean(0)
    sel = np.sort(np.argsort(-mean_gate, kind="stable")[:E])

    # ---- launch 2: attention -------------------------------------------
    wq, bq = inputs["wq"], inputs["bq"]
    wk, bk = inputs["wk"], inputs["bk"]
    wv, bv = inputs["wv"], inputs["bv"]
    wo, bo = inputs["wo"], inputs["bo"]
    alpha = (wq * wk).sum(1).astype(np.float64)
    gamma = (bq * wk).sum(1).astype(np.float64)
    pv = (wo * wv).sum(1).astype(np.float64)
    qv = ((wo * bv).sum(1) + bo).astype(np.float64)

    xsel = x[:, sel, :]                              # (B, E, L)
    umax = float(np.abs(xsel).max())
    zm = (np.abs(alpha).max() * umax + np.abs(gamma).max()) * umax
    cheb = np.polynomial.chebyshev.Chebyshev.interpolate(
        np.exp, J, domain=[-zm, zm])
    dc = cheb.convert(kind=np.polynomial.Polynomial).coef
    dj = [float(dc[j] * (2.0 ** j)) for j in range(J + 1)]

    key = tuple(np.round(dj, 12))
    if _CACHE.get("attn_key") != key:
        _CACHE["attn"] = build_attn_program(dj)
        _CACHE["attn_key"] = key
    nc2 = _CACHE["attn"]

    xsg = np.empty((B, EL + E), np.float16)          # [u' l-major | gate_sel]
    xsg[:, :EL] = (xsel.transpose(0, 2, 1) * np.float32(0.5)
                   ).astype(np.float16).reshape(B, EL)
    xsg[:, EL:] = gate16[:, sel]
    a2vv = np.tile((2 * alpha).astype(np.float16), L)
    gvvv = np.tile(gamma.astype(np.float16), L)
    p2vv = (4 * pv).astype(np.float16)
    qvvv = (2 * qv).astype(np.float16)
    djvv = np.repeat(np.asarray(dj), 2 * E).astype(np.float16)
    maps2 = [{"xsg": xsg[i * BC:(i + 1) * BC],
              "a2v": a2vv, "gvv": gvvv, "p2v": p2vv, "qvv": qvvv,
              "djv": djvv}
             for i in cores]
    r2 = run_bass_kernel_spmd(nc2, maps2, cores).results
    o16 = np.concatenate([np.asarray(r["o16"]) for r in r2], 0)  # (B, 924)

    # ---- host unshard / scatter (device emits 2*A and A*x) -------------
    at = (o16[:, :EL].astype(np.float32) * 0.5).reshape(
        B, L, E).transpose(0, 2, 1)
    gt = o16[:, EL:].astype(np.float32).reshape(B, L, E).transpose(0, 2, 1)
    cols = (np.repeat(sel * L, L) + np.tile(np.arange(L), E))
    A_full = np.zeros((B, CL), np.float32)
    G_full = np.zeros((B, CL), np.float32)
    A_full[:, cols] = at.reshape(B, EL)
    G_full[:, cols] = gt.reshape(B, EL)
    return G_full, A_full


# revision 18
# speedup vs baseline: 2.7801x; 1.0124x over previous
"""Trainium2 Bass kernel for grouped-attention MoE routing.

Math (derived from the nn.Module):
  gate  = softmax(mlp(maxpool(conv(x))) + mlp(avgpool(conv(x))))      (B,45)
  sel   = sorted(top22(mean_b gate))                                  (22,)
  Per expert e with u = x[:, sel[e], :]:
    energy[l,m] = (a_e*u_l + g_e) * u_m   (rank-1; scalars a,g from weights)
    attn = softmax_m(energy);  s_l = sum_m u_m attn[l,m]
    y_l  = P_e*s_l + Q_e;      A[:,sel[e],:] = y * gate[:,sel[e]]
  G = x * A (flat);  return (G, A_flat)

Key optimization: with k = a*u_l + g, the softmax row sums are
  den_l = sum_m e^{k u_m},  num_l = sum_m u_m e^{k u_m}.
Approximating e^z by a degree-J Chebyshev fit P(z) = sum_j d_j z^j on the
realized z-range turns both into polynomials in k with power-sum
coefficients: den = sum_j d_j k^j S_j, num = sum_j d_j k^j S_{j+1}, where
S_j = sum_m u_m^j.  This replaces the O(L^2) energy tensor with O(L*J)
work (J=6 gives ~1e-3 end-to-end error vs the 2e-2 gate).

Layout: fp16 everywhere on the elementwise path with e (expert) innermost
so every scalar_tensor_tensor op hits the DVE 4x perf mode; power sums via
a pairwise tree reduce; Horner evaluation of num|den jointly (num and den
share the multiply-by-k steps and the immediate d_j coefficients once u is
pre-scaled by 1/2 on the host).

Strategy: pure data parallel over batch on 8 cores; two launches with the
45-float routing reduction mediated on host (equivalent of the all-reduce).
"""

import numpy as np
import ml_dtypes
from contextlib import ExitStack

import bass_rust
import concourse.bass as bass
import concourse.mybir as mybir
import concourse.tile as tile
from concourse.bass_utils import run_bass_kernel_spmd

_MULTIWAIT_OK = ("InstNoOp", "InstAllEngineBarrier",
                 "InstEventSemaphore", "InstUnconditionalBranch")


def legalize_sync_waits(nc):
    """walrus codegen on this stack rejects >1 sync wait on most
    instructions; hoist extra waits onto same-engine NoOps."""
    for func in nc.m.functions:
        for block in func.blocks:
            il = block.instructions
            out = []
            for inst in il:
                tname = type(inst).__name__
                si = getattr(inst, "sync_info", None)
                waits = list(si.on_wait) if si is not None else []
                if tname not in _MULTIWAIT_OK and len(waits) > 1:
                    for k, w in enumerate(waits):
                        nop = mybir.InstNoOp(
                            name=f"{inst.name}-synop{k}", ins=[], outs=[])
                        nop.engine = inst.engine
                        nop.sync_info = bass_rust.SyncInfo(
                            on_wait=[w], on_update=[])
                        out.append(nop)
                    inst.sync_info = bass_rust.SyncInfo(
                        on_wait=[], on_update=list(inst.sync_info.on_update))
                out.append(inst)
            il.clear()
            il.extend(out)


B, C, L, E = 8192, 45, 21, 22
NCORES = 8
BC = B // NCORES          # rows per core
P = 128                   # SBUF partitions
NT = BC // P              # batch tiles per core
CL = C * L                # 945
EL = E * L                # 462
J = 5                     # exp-approx polynomial degree
F32 = mybir.dt.float32
F16 = mybir.dt.float16
BF16 = mybir.dt.bfloat16
AF = mybir.ActivationFunctionType
ALU = mybir.AluOpType
AX = mybir.AxisListType
BYP = ALU.bypass

# channel groups for the gating conv matmul: 8 groups of <=6 channels
GROUPS = [list(range(g, min(g + 6, C))) for g in range(0, C, 6)]
NG = len(GROUPS)          # 8
GROWS = 127               # rows per chunk in the host-packed transposed x


def _ap(base, extra_free):
    """Custom free-dim access pattern on a tile slice: keep the partition
    dim of `base`, replace the free dims."""
    return bass.AP(tensor=base.tensor, offset=base.offset,
                   ap=[base.ap[0]] + extra_free)


def _dram_ap(dram, offset, ap):
    base = dram[:, :] if len(dram.shape) > 1 else dram[:]
    return bass.AP(tensor=base.tensor, offset=base.offset + offset, ap=ap)


def build_gate_program():
    """Gating network. x arrives host-transposed as 8 row-chunks of 127
    (6 channels x 21 taps + a ones row for bias), bf16.  Conv + avg-pool
    ride the PE as block-diagonal matmuls into a single bf16 PSUM bank
    (double-buffered); max-pool on DVE; the MLP runs transposed (bias via
    per-partition activation bias) with per-branch PSUM banks so the two
    branches and adjacent tiles overlap; softmax skips the max-subtract
    (|z| <= 2 by construction). Output gate in fp16."""
    nc = bass.Bass()
    xg = nc.declare_dram_parameter("xg", [NG * GROWS, BC], BF16,
                                   isOutput=False)
    wcat = nc.declare_dram_parameter("wcat", [GROWS, CL], BF16,
                                     isOutput=False)
    wav = nc.declare_dram_parameter("wav", [GROWS, C], BF16, isOutput=False)
    w1a = nc.declare_dram_parameter("w1a", [C, 25], BF16, isOutput=False)
    b1c = nc.declare_dram_parameter("b1c", [25, 1], F32, isOutput=False)
    w2a = nc.declare_dram_parameter("w2a", [25, C], BF16, isOutput=False)
    b2c = nc.declare_dram_parameter("b2c", [C, 1], F32, isOutput=False)
    identb = nc.declare_dram_parameter("identb", [P, P], BF16, isOutput=False)
    gate_o = nc.declare_dram_parameter("gate", [BC, C], F16, isOutput=True)

    # per-group geometry: (chunk row base, data rows, out-col base, n chans)
    geo = []
    cb = 0
    for g, chans in enumerate(GROUPS):
        nch = len(chans)
        geo.append((g * GROWS, nch * L, cb, nch))
        cb += nch * L

    with tile.TileContext(nc) as tc, ExitStack() as ctx:
        singles = ctx.enter_context(tc.tile_pool(name="singles", bufs=1))
        xs = ctx.enter_context(tc.tile_pool(name="xs", bufs=3))
        cp = ctx.enter_context(tc.tile_pool(name="cp", bufs=2))
        hw = ctx.enter_context(tc.tile_pool(name="hw", bufs=2))
        sm = ctx.enter_context(tc.tile_pool(name="sm", bufs=3))
        ps = ctx.enter_context(tc.tile_pool(name="ps", bufs=2, space="PSUM"))
        psm = ctx.enter_context(tc.tile_pool(name="psm", bufs=1, space="PSUM"))

        # PE-read consts funnel through DVE (one-wait matmul constraint);
        # warm-up transpose advances PE's observed DVE clock past them.
        def dve_const(dram, p, n, dt):
            raw = singles.tile([p, n], dt, name="raw_" + dram.name)
            nc.sync.dma_start(out=raw, in_=dram[:, :])
            t = singles.tile([p, n], dt, name="sb_" + dram.name)
            nc.vector.tensor_copy(out=t, in_=raw)
            return t

        sb_id = dve_const(identb, P, P, BF16)
        sb_wcat = dve_const(wcat, GROWS, CL, BF16)
        sb_wav = dve_const(wav, GROWS, C, BF16)
        sb_w1a = dve_const(w1a, C, 25, BF16)
        sb_w2a = dve_const(w2a, 25, C, BF16)
        sb_b1c = dve_const(b1c, 25, 1, F32)
        sb_b2c = dve_const(b2c, C, 1, F32)
        ones_col = singles.tile([P, 1], BF16)
        nc.vector.memset(ones_col, 1.0)
        warm_ps = psm.tile([C, 4 * P], BF16, tag="hTq")
        nc.tensor.transpose(warm_ps[0:1, 0:P], ones_col, sb_id)

        QT = 4                      # tiles batched through one MLP pass
        for q in range(NT // QT):
            mxq = sm.tile([P, QT * 48], BF16, tag="mxq")
            avq = sm.tile([P, QT * C], BF16, tag="avq")
            for ti in range(QT):
                t = q * QT + ti
                # one DMA: 8 transposed chunks side by side (127, 8*128)
                xgt = xs.tile([GROWS, NG * P], BF16, tag="xgt")
                nc.sync.dma_start(
                    out=xgt[:, :],
                    in_=_dram_ap(xg, t * P,
                                 [[BC, GROWS], [GROWS * BC, NG], [1, P]]))

                # conv + avg into one PSUM tile: [0:945) conv, [945:990) avg
                tp = ps.tile([P, CL + C], F32, tag="tp")
                for g, (rbase, rdata, cbase, nch) in enumerate(geo):
                    lhs = xgt[0:rdata + 1, g * P:(g + 1) * P]
                    nc.tensor.matmul(
                        tp[:, cbase:cbase + nch * L], lhs,
                        sb_wcat[0:rdata + 1, cbase:cbase + nch * L],
                        start=True, stop=True)
                    cav = sum(len(ch) for ch in GROUPS[:g])
                    nc.tensor.matmul(tp[:, CL + cav:CL + cav + nch], lhs,
                                     sb_wav[0:rdata + 1, cav:cav + nch],
                                     start=True, stop=True)

                # avg copy (ACT); max-pool via three PSUM reduces (DVE)
                nc.scalar.activation(out=avq[:, ti * C:(ti + 1) * C],
                                     in_=tp[:, CL:CL + C], func=AF.Copy)
                nc.vector.tensor_reduce(
                    out=mxq[:, ti * 48:ti * 48 + 24],
                    in_=_ap(tp[:, 0:504], [[126, 4], [21, 6], [1, L]]),
                    axis=AX.X, op=ALU.max)
                nc.vector.tensor_reduce(
                    out=mxq[:, ti * 48 + 24:ti * 48 + 42],
                    in_=_ap(tp[:, 504:882], [[126, 3], [21, 6], [1, L]]),
                    axis=AX.X, op=ALU.max)
                nc.vector.tensor_reduce(
                    out=mxq[:, ti * 48 + 42:ti * 48 + 45],
                    in_=_ap(tp[:, 882:945], [[21, 3], [1, L]]),
                    axis=AX.X, op=ALU.max)

            # quad MLP: 4 tiles share each matmul/tanh as (.., 512) passes
            zTs = []
            for br, hq in enumerate((mxq, avq)):
                step = 48 if br == 0 else C
                hTq_ps = psm.tile([C, QT * P], BF16, tag="hTq")
                for ti in range(QT):
                    nc.tensor.transpose(
                        hTq_ps[:, ti * P:(ti + 1) * P],
                        hq[:, ti * step:ti * step + C], sb_id)
                hTq = hw.tile([C, QT * P], BF16, tag=f"hTq{br}")
                if br == 0:
                    nc.scalar.activation(out=hTq, in_=hTq_ps, func=AF.Copy)
                else:
                    nc.vector.tensor_copy(out=hTq, in_=hTq_ps)
                pmlp = psm.tile([C, QT * P], F32, tag=f"pmlp{br}")
                nc.tensor.matmul(pmlp[0:25, :], sb_w1a, hTq,
                                 start=True, stop=True)
                t1Tq = hw.tile([25, QT * P], BF16, tag=f"t1Tq{br}")
                nc.scalar.activation(out=t1Tq, in_=pmlp[0:25, :],
                                     func=AF.Tanh, bias=sb_b1c)
                nc.tensor.matmul(pmlp, sb_w2a, t1Tq, start=True, stop=True)
                zT = hw.tile([C, QT * P], BF16, tag=f"zTq{br}")
                nc.scalar.activation(out=zT, in_=pmlp, func=AF.Tanh,
                                     bias=sb_b2c)
                zTs.append(zT)

            # sum branches in SBUF; transpose back per tile (PE transpose
            # does not accumulate in PSUM); softmax per tile
            zTsum = hw.tile([C, QT * P], BF16, tag="zTsum")
            nc.vector.tensor_add(out=zTsum, in0=zTs[0], in1=zTs[1])
            zsq = psm.tile([P, QT * 48], BF16, tag="zsq")
            for ti in range(QT):
                t = q * QT + ti
                nc.tensor.transpose(zsq[:, ti * 48:ti * 48 + C],
                                    zTsum[:, ti * P:(ti + 1) * P],
                                    sb_id[0:C, 0:C])
                eg = sm.tile([P, C], F16, tag=f"eg{ti}")
                ssum = sm.tile([P, 1], F32, tag=f"ssum{ti}")
                nc.scalar.activation(out=eg, in_=zsq[:, ti * 48:ti * 48 + C],
                                     func=AF.Exp, accum_out=ssum)
                rs = sm.tile([P, 1], F32, tag=f"rs{ti}")
                nc.vector.reciprocal(out=rs, in_=ssum)
                gt = sm.tile([P, C], F16, tag=f"gt{ti}")
                nc.vector.tensor_scalar_mul(out=gt, in0=eg, scalar1=rs)
                nc.scalar.dma_start(out=gate_o[t * P:(t + 1) * P, :], in_=gt)
    legalize_sync_waits(nc)
    return nc


def build_attn_program(dj):
    """Rank-1 attention via the polynomial trick.  Inputs are fp16,
    l-major (element (l,e) at l*22+e), with u pre-scaled by 1/2 on the
    host so fp16 power sums cannot overflow; dj[j] = cheb_j * 2^j are the
    shared step immediates (the num half then computes num/2, folded
    into the 4*P_e gate coefficient together with the output-side 2x
    that the host strips off again).

    Engine notes: only plain TensorScalarPtr/TensorCopy get the DVE
    2x/4x modes (scalar_tensor_tensor gets none), and walrus limits TSP
    to 2 free dims, so the hot loop uses fp16 InstTensorTensor (2x_1p)
    with 3-free-dim access patterns:
    - power stack interleaved (l, j, e): the pairwise tree reduce over l
      runs on (l, row-contiguous) patterns and its last step lands
      S'_1..S'_7 directly in (j, e) order next to the memset S'_0;
    - per-step coefficient pairs [S'_j | S'_{j+1}]*dj are prebuilt by a
      single windowed TT against a repeated-dj constant; the num-half
      coefficients additionally absorb gate*4P_e so the Horner num
      output is already the gated numerator.
    J=5 keeps end-to-end error ~2e-3 against the 2e-2 gate."""
    nc = bass.Bass()
    xsg = nc.declare_dram_parameter("xsg", [BC, EL + E], F16, isOutput=False)
    a2v = nc.declare_dram_parameter("a2v", [EL], F16, isOutput=False)
    gvv = nc.declare_dram_parameter("gvv", [EL], F16, isOutput=False)
    p2v = nc.declare_dram_parameter("p2v", [E], F16, isOutput=False)
    qvv = nc.declare_dram_parameter("qvv", [E], F16, isOutput=False)
    djv = nc.declare_dram_parameter("djv", [(J + 1) * 2 * E], F16,
                                    isOutput=False)
    o16 = nc.declare_dram_parameter("o16", [BC, 2 * EL], F16, isOutput=True)

    NJ = J + 1        # powers u'^1..u'^{J+1}
    ROW = NJ * E      # one l-row of the interleaved power stack

    with tile.TileContext(nc) as tc, ExitStack() as ctx:
        singles = ctx.enter_context(tc.tile_pool(name="singles", bufs=1))
        pstk = ctx.enter_context(tc.tile_pool(name="pstk", bufs=3))
        trp = ctx.enter_context(tc.tile_pool(name="trp", bufs=3))
        kp = ctx.enter_context(tc.tile_pool(name="kp", bufs=3))
        hp = ctx.enter_context(tc.tile_pool(name="hp", bufs=3))
        op = ctx.enter_context(tc.tile_pool(name="op", bufs=3))

        def bconst(dram, n):
            base = dram[:]
            t = singles.tile([P, n], F16, name="bc_" + dram.name)
            nc.gpsimd.dma_start(
                out=t, in_=bass.AP(tensor=base.tensor, offset=base.offset,
                                   ap=[[0, P], [1, n]]))
            return t

        a2B = bconst(a2v, EL)
        gB = bconst(gvv, EL)
        p2B = bconst(p2v, E)
        qB = bconst(qvv, E)
        djB = bconst(djv, (J + 1) * 2 * E)

        for t in range(NT):
            ug = kp.tile([P, EL + E], F16, tag="ug")  # [u' | gate_sel]
            nc.sync.dma_start(out=ug, in_=xsg[t * P:(t + 1) * P, :])
            u0 = ug[:, 0:EL]
            gst = ug[:, EL:EL + E]

            # interleaved power stack (l, j, e); ACT copies u' to slot 1
            pst = pstk.tile([P, L * ROW], F16, tag="pst")

            def slot(j):          # (l, e) view of power j
                return _ap(pst[:, (j - 1) * E:], [[ROW, L], [1, E]])

            nc.scalar.activation(out=slot(1), in_=u0, func=AF.Copy)

            # kap = (2a)u' + g   (Pool)
            kt = kp.tile([P, EL], F16, tag="kt")
            nc.gpsimd.tensor_mul(out=kt, in0=u0, in1=a2B)
            kap = kp.tile([P, EL], F16, tag="kap")
            nc.gpsimd.tensor_add(out=kap, in0=kt, in1=gB)

            # powers u'^2..: squares on ACT, odd muls on DVE/Pool
            nc.scalar.activation(out=slot(2), in_=slot(1), func=AF.Square)
            nc.gpsimd.tensor_mul(out=slot(3), in0=slot(2), in1=slot(1))
            nc.scalar.activation(out=slot(4), in_=slot(2), func=AF.Square)
            if NJ >= 5:
                nc.gpsimd.tensor_mul(out=slot(5), in0=slot(3), in1=slot(2))
            if NJ >= 6:
                nc.scalar.activation(out=slot(6), in_=slot(3), func=AF.Square)
            if NJ >= 7:
                nc.gpsimd.tensor_mul(out=slot(7), in0=slot(6), in1=slot(1))

            # pairwise tree over l; (j,e) stays contiguous throughout, so
            # the final step writes S'_1.. straight into the S-stack
            Sp = trp.tile([P, (NJ + 1) * E], F16, tag="Sp")
            nc.gpsimd.memset(Sp[:, 0:E], float(L))
            t1 = trp.tile([P, 10 * ROW], F16, tag="t1")
            t2 = trp.tile([P, 5 * ROW], F16, tag="t2")
            t3 = trp.tile([P, 2 * ROW], F16, tag="t3")
            t4 = trp.tile([P, ROW], F16, tag="t4")
            t5 = trp.tile([P, ROW], F16, tag="t5")

            pR = lambda l0, n: _ap(pst[:, l0 * ROW:], [[ROW, n], [1, ROW]])
            tR = lambda tl, l0, n: _ap(tl[:, l0 * ROW:], [[ROW, n], [1, ROW]])
            nc.vector.tensor_add(out=tR(t1, 0, 10), in0=pR(0, 10),
                                 in1=pR(10, 10))
            nc.vector.tensor_add(out=tR(t2, 0, 5), in0=tR(t1, 0, 5),
                                 in1=tR(t1, 5, 5))
            nc.vector.tensor_add(out=tR(t3, 0, 2), in0=tR(t2, 0, 2),
                                 in1=tR(t2, 2, 2))
            nc.vector.tensor_add(out=tR(t4, 0, 1), in0=tR(t3, 0, 1),
                                 in1=tR(t3, 1, 1))
            nc.vector.tensor_add(out=tR(t5, 0, 1), in0=tR(t4, 0, 1),
                                 in1=tR(t2, 4, 1))
            nc.vector.tensor_add(out=_ap(Sp[:, E:], [[1, ROW]]),
                                 in0=tR(t5, 0, 1), in1=pR(20, 1))

            # coefficient pairs SS_j = dj * [S'_j | S'_{j+1}] in one
            # windowed TT; then scale the num half by gate*4P_e
            SS = trp.tile([P, (J + 1) * 2 * E], F16, tag="SS")
            nc.vector.tensor_mul(
                out=_ap(SS[:, 0:], [[2 * E, J + 1], [1, 2 * E]]),
                in0=_ap(Sp[:, 0:], [[E, J + 1], [1, 2 * E]]),
                in1=_ap(djB[:, 0:], [[2 * E, J + 1], [1, 2 * E]]))
            gp = kp.tile([P, E], F16, tag="gp")
            nc.gpsimd.tensor_mul(out=gp, in0=gst, in1=p2B)
            nc.vector.tensor_mul(
                out=_ap(SS[:, E:], [[2 * E, J + 1], [1, E]]),
                in0=_ap(SS[:, E:], [[2 * E, J + 1], [1, E]]),
                in1=_ap(gp[:, 0:E], [[0, J + 1], [1, E]]))

            # joint Horner on [den | gated-num] with 3-free-dim TT ops
            ra = hp.tile([P, 2 * EL], F16, tag="ra")
            rb = hp.tile([P, 2 * EL], F16, tag="rb")
            f44 = lambda tl: _ap(tl[:, 0:], [[EL, 2], [E, L], [1, E]])
            cj = lambda j: _ap(SS[:, j * 2 * E:], [[E, 2], [0, L], [1, E]])
            kB = _ap(kap[:, 0:EL], [[0, 2], [E, L], [1, E]])
            cur, other = ra, rb
            nc.vector.tensor_mul(out=f44(cur), in0=cj(J), in1=kB)
            for j in range(J - 1, -1, -1):
                nc.vector.tensor_add(out=f44(other), in0=cj(j), in1=f44(cur))
                cur, other = other, cur
                if j > 0:
                    nc.vector.tensor_mul(out=f44(other), in0=f44(cur), in1=kB)
                    cur, other = other, cur

            # at2 = 2*A = gated-num/den + 2*gate*Q (host halves A on the
            # way out); gt = at2 * u' = A * x
            rd32 = op.tile([P, EL], F32, tag="rd32")
            nc.vector.reciprocal(out=rd32, in_=cur[:, 0:EL])
            atm = op.tile([P, EL], F16, tag="atm")
            nc.gpsimd.tensor_mul(out=atm, in0=cur[:, EL:2 * EL], in1=rd32)
            gq = kp.tile([P, E], F16, tag="gq")
            nc.gpsimd.tensor_mul(out=gq, in0=gst, in1=qB)
            ot = op.tile([P, 2 * EL], F16, tag="ot")
            gqB = _ap(gq[:, 0:E], [[0, L], [1, E]])
            nc.vector.tensor_add(out=ot[:, 0:EL], in0=atm, in1=gqB)
            nc.vector.tensor_mul(out=ot[:, EL:2 * EL], in0=ot[:, 0:EL],
                                 in1=u0)
            nc.scalar.dma_start(out=o16[t * P:(t + 1) * P, :], in_=ot)
    legalize_sync_waits(nc)
    return nc


def _gate_params(inputs):
    gc_w, gc_b = inputs["gc_w"], inputs["gc_b"]
    wbar = gc_w.mean(0)
    bbar = gc_b.mean()
    wcat = np.zeros((GROWS, CL), np.float32)
    wav = np.zeros((GROWS, C), np.float32)
    cb = 0
    for g, chans in enumerate(GROUPS):
        nch = len(chans)
        for k, c in enumerate(chans):
            wcat[k * L:(k + 1) * L, cb + k * L:cb + (k + 1) * L] = gc_w.T
            wav[k * L:(k + 1) * L, c] = wbar
            wav[nch * L, c] = bbar
        wcat[nch * L, cb:cb + nch * L] = np.tile(gc_b, nch)
        cb += nch * L
    bf = ml_dtypes.bfloat16
    return (wcat.astype(bf), wav.astype(bf),
            inputs["w1"].T.astype(bf).copy(),
            inputs["b1"][:, None].astype(np.float32).copy(),
            inputs["w2"].T.astype(bf).copy(),
            inputs["b2"][:, None].astype(np.float32).copy(),
            np.eye(P, dtype=np.float32).astype(bf))


_CACHE = {}


def kernel(**inputs):
    inputs = {k: np.ascontiguousarray(np.asarray(v)) for k, v in inputs.items()}
    x = inputs["x"].astype(np.float32)              # (B, C, L)
    bf = ml_dtypes.bfloat16
    cores = list(range(NCORES))

    # ---- launch 1: gate -------------------------------------------------
    wcat, wav, w1a, b1c, w2a, b2c, identb = _gate_params(inputs)
    # host-marshaled transposed x: 8 chunks of (6ch x 21 + ones row) x B
    xt = x.reshape(B, CL).T                          # (945, B)
    xg = np.zeros((NG * GROWS, B), np.float32)
    cb = 0
    for g, chans in enumerate(GROUPS):
        nch = len(chans)
        xg[g * GROWS:g * GROWS + nch * L] = xt[cb:cb + nch * L]
        xg[g * GROWS + nch * L] = 1.0
        cb += nch * L
    xg = xg.astype(bf)

    if "gate" not in _CACHE:
        _CACHE["gate"] = build_gate_program()
    nc1 = _CACHE["gate"]
    maps1 = [{"xg": np.ascontiguousarray(xg[:, i * BC:(i + 1) * BC]),
              "wcat": wcat, "wav": wav, "w1a": w1a, "b1c": b1c,
              "w2a": w2a, "b2c": b2c, "identb": identb} for i in cores]
    r1 = run_bass_kernel_spmd(nc1, maps1, cores).results
    gate16 = np.concatenate([np.asarray(r["gate"]) for r in r1], 0)  # (B,45)

    # ---- routing (host-mediated all-reduce) -----------------------------
    mean_gate = gate16.astype(np.float64).mean(0)
    sel = np.sort(np.argsort(-mean_gate, kind="stable")[:E])

    # ---- launch 2: attention -------------------------------------------
    wq, bq = inputs["wq"], inputs["bq"]
    wk, bk = inputs["wk"], inputs["bk"]
    wv, bv = inputs["wv"], inputs["bv"]
    wo, bo = inputs["wo"], inputs["bo"]
    alpha = (wq * wk).sum(1).astype(np.float64)
    gamma = (bq * wk).sum(1).astype(np.float64)
    pv = (wo * wv).sum(1).astype(np.float64)
    qv = ((wo * bv).sum(1) + bo).astype(np.float64)

    xsel = x[:, sel, :]                              # (B, E, L)
    umax = float(np.abs(xsel).max())
    zm = (np.abs(alpha).max() * umax + np.abs(gamma).max()) * umax
    cheb = np.polynomial.chebyshev.Chebyshev.interpolate(
        np.exp, J, domain=[-zm, zm])
    dc = cheb.convert(kind=np.polynomial.Polynomial).coef
    dj = [float(dc[j] * (2.0 ** j)) for j in range(J + 1)]

    key = tuple(np.round(dj, 12))
    if _CACHE.get("attn_key") != key:
        _CACHE["attn"] = build_attn_program(dj)
        _CACHE["attn_key"] = key
    nc2 = _CACHE["attn"]

    xsg = np.empty((B, EL + E), np.float16)          # [u' l-major | gate_sel]
    xsg[:, :EL] = (xsel.transpose(0, 2, 1) * np.float32(0.5)
                   ).astype(np.float16).reshape(B, EL)
    xsg[:, EL:] = gate16[:, sel]
    a2vv = np.tile((2 * alpha).astype(np.float16), L)
    gvvv = np.tile(gamma.astype(np.float16), L)
    p2vv = (4 * pv).astype(np.float16)
    qvvv = (2 * qv).astype(np.float16)
    djvv = np.repeat(np.asarray(dj), 2 * E).astype(np.float16)
    maps2 = [{"xsg": xsg[i * BC:(i + 1) * BC],
              "a2v": a2vv, "gvv": gvvv, "p2v": p2vv, "qvv": qvvv,
              "djv": djvv}
             for i in cores]
    r2 = run_bass_kernel_spmd(nc2, maps2, cores).results
    o16 = np.concatenate([np.asarray(r["o16"]) for r in r2], 0)  # (B, 924)

    # ---- host unshard / scatter (device emits 2*A and A*x) -------------
    at = (o16[:, :EL].astype(np.float32) * 0.5).reshape(
        B, L, E).transpose(0, 2, 1)
    gt = o16[:, EL:].astype(np.float32).reshape(B, L, E).transpose(0, 2, 1)
    cols = (np.repeat(sel * L, L) + np.tile(np.arange(L), E))
    A_full = np.zeros((B, CL), np.float32)
    G_full = np.zeros((B, CL), np.float32)
    A_full[:, cols] = at.reshape(B, EL)
    G_full[:, cols] = gt.reshape(B, EL)
    return G_full, A_full


# revision 19
# speedup vs baseline: 2.8703x; 1.0325x over previous
"""Trainium2 Bass kernel for grouped-attention MoE routing.

Math (derived from the nn.Module):
  gate  = softmax(mlp(maxpool(conv(x))) + mlp(avgpool(conv(x))))      (B,45)
  sel   = sorted(top22(mean_b gate))                                  (22,)
  Per expert e with u = x[:, sel[e], :]:
    energy[l,m] = (a_e*u_l + g_e) * u_m   (rank-1; scalars a,g from weights)
    attn = softmax_m(energy);  s_l = sum_m u_m attn[l,m]
    y_l  = P_e*s_l + Q_e;      A[:,sel[e],:] = y * gate[:,sel[e]]
  G = x * A (flat);  return (G, A_flat)

Key optimization: with k = a*u_l + g, the softmax row sums are
  den_l = sum_m e^{k u_m},  num_l = sum_m u_m e^{k u_m}.
Approximating e^z by a degree-J Chebyshev fit P(z) = sum_j d_j z^j on the
realized z-range turns both into polynomials in k with power-sum
coefficients: den = sum_j d_j k^j S_j, num = sum_j d_j k^j S_{j+1}, where
S_j = sum_m u_m^j.  This replaces the O(L^2) energy tensor with O(L*J)
work (J=6 gives ~1e-3 end-to-end error vs the 2e-2 gate).

Layout: fp16 everywhere on the elementwise path with e (expert) innermost
so every scalar_tensor_tensor op hits the DVE 4x perf mode; power sums via
a pairwise tree reduce; Horner evaluation of num|den jointly (num and den
share the multiply-by-k steps and the immediate d_j coefficients once u is
pre-scaled by 1/2 on the host).

Strategy: pure data parallel over batch on 8 cores; two launches with the
45-float routing reduction mediated on host (equivalent of the all-reduce).
"""

import numpy as np
import ml_dtypes
from contextlib import ExitStack

import bass_rust
import concourse.bass as bass
import concourse.mybir as mybir
import concourse.tile as tile
from concourse.bass_utils import run_bass_kernel_spmd

_MULTIWAIT_OK = ("InstNoOp", "InstAllEngineBarrier",
                 "InstEventSemaphore", "InstUnconditionalBranch")


def legalize_sync_waits(nc):
    """walrus codegen on this stack rejects >1 sync wait on most
    instructions; hoist extra waits onto same-engine NoOps."""
    for func in nc.m.functions:
        for block in func.blocks:
            il = block.instructions
            out = []
            for inst in il:
                tname = type(inst).__name__
                si = getattr(inst, "sync_info", None)
                waits = list(si.on_wait) if si is not None else []
                if tname not in _MULTIWAIT_OK and len(waits) > 1:
                    for k, w in enumerate(waits):
                        nop = mybir.InstNoOp(
                            name=f"{inst.name}-synop{k}", ins=[], outs=[])
                        nop.engine = inst.engine
                        nop.sync_info = bass_rust.SyncInfo(
                            on_wait=[w], on_update=[])
                        out.append(nop)
                    inst.sync_info = bass_rust.SyncInfo(
                        on_wait=[], on_update=list(inst.sync_info.on_update))
                out.append(inst)
            il.clear()
            il.extend(out)


B, C, L, E = 8192, 45, 21, 22
NCORES = 8
BC = B // NCORES          # rows per core
P = 128                   # SBUF partitions
NT = BC // P              # batch tiles per core
CL = C * L                # 945
EL = E * L                # 462
J = 5                     # exp-approx polynomial degree
F32 = mybir.dt.float32
F16 = mybir.dt.float16
BF16 = mybir.dt.bfloat16
AF = mybir.ActivationFunctionType
ALU = mybir.AluOpType
AX = mybir.AxisListType
BYP = ALU.bypass

# channel groups for the gating conv matmul: 8 groups of <=6 channels
GROUPS = [list(range(g, min(g + 6, C))) for g in range(0, C, 6)]
NG = len(GROUPS)          # 8
GROWS = 127               # rows per chunk in the host-packed transposed x


def _ap(base, extra_free):
    """Custom free-dim access pattern on a tile slice: keep the partition
    dim of `base`, replace the free dims."""
    return bass.AP(tensor=base.tensor, offset=base.offset,
                   ap=[base.ap[0]] + extra_free)


def _dram_ap(dram, offset, ap):
    base = dram[:, :] if len(dram.shape) > 1 else dram[:]
    return bass.AP(tensor=base.tensor, offset=base.offset + offset, ap=ap)


def build_gate_program():
    """Gating network. x arrives host-transposed as 8 row-chunks of 127
    (6 channels x 21 taps + a ones row for bias), bf16.  Conv + avg-pool
    ride the PE as block-diagonal matmuls into a single bf16 PSUM bank
    (double-buffered); max-pool on DVE; the MLP runs transposed (bias via
    per-partition activation bias) with per-branch PSUM banks so the two
    branches and adjacent tiles overlap; softmax skips the max-subtract
    (|z| <= 2 by construction). Output gate in fp16."""
    nc = bass.Bass()
    # packed constants: one bf16 block [wcat | wav | w1a | w2a | ident]
    # (column offsets 0/945/990/1015/1060) and one f32 block [b1c | b2c]
    NCB = CL + C + 25 + C + P
    xg = nc.declare_dram_parameter("xg", [NG * GROWS, BC], BF16,
                                   isOutput=False)
    cstb = nc.declare_dram_parameter("cstb", [P, NCB], BF16, isOutput=False)
    cstf = nc.declare_dram_parameter("cstf", [C, 2], F32, isOutput=False)
    gate_o = nc.declare_dram_parameter("gate", [BC, C], F16, isOutput=True)

    # per-group geometry: (chunk row base, data rows, out-col base, n chans)
    geo = []
    cb = 0
    for g, chans in enumerate(GROUPS):
        nch = len(chans)
        geo.append((g * GROWS, nch * L, cb, nch))
        cb += nch * L

    with tile.TileContext(nc) as tc, ExitStack() as ctx:
        singles = ctx.enter_context(tc.tile_pool(name="singles", bufs=1))
        xs = ctx.enter_context(tc.tile_pool(name="xs", bufs=3))
        cp = ctx.enter_context(tc.tile_pool(name="cp", bufs=2))
        hw = ctx.enter_context(tc.tile_pool(name="hw", bufs=2))
        sm = ctx.enter_context(tc.tile_pool(name="sm", bufs=3))
        ps = ctx.enter_context(tc.tile_pool(name="ps", bufs=2, space="PSUM"))
        psm = ctx.enter_context(tc.tile_pool(name="psm", bufs=1, space="PSUM"))

        # PE-read consts funnel through DVE (one-wait matmul constraint);
        # warm-up transpose advances PE's observed DVE clock past them.
        def dve_const(dram, p, n, dt):
            raw = singles.tile([p, n], dt, name="raw_" + dram.name)
            nc.sync.dma_start(out=raw, in_=dram[:, :])
            t = singles.tile([p, n], dt, name="sb_" + dram.name)
            nc.vector.tensor_copy(out=t, in_=raw)
            return t

        sb_cb = dve_const(cstb, P, NCB, BF16)
        sb_cf = dve_const(cstf, C, 2, F32)
        sb_wcat = sb_cb[0:GROWS, 0:CL]
        sb_wav = sb_cb[0:GROWS, CL:CL + C]
        sb_w1a = sb_cb[0:C, CL + C:CL + C + 25]
        sb_w2a = sb_cb[0:25, CL + C + 25:CL + 2 * C + 25]
        sb_id = sb_cb[0:P, CL + 2 * C + 25:NCB]
        sb_b1c = sb_cf[0:25, 0:1]
        sb_b2c = sb_cf[0:C, 1:2]
        ones_col = singles.tile([P, 1], BF16)
        nc.vector.memset(ones_col, 1.0)
        warm_ps = psm.tile([C, 4 * P], BF16, tag="hTq")
        nc.tensor.transpose(warm_ps[0:1, 0:P], ones_col, sb_id)

        QT = 2                      # tiles batched through one MLP pass
        for q in range(NT // QT):
            mxq = sm.tile([P, QT * 48], BF16, tag="mxq")
            avq = sm.tile([P, QT * C], BF16, tag="avq")
            for ti in range(QT):
                t = q * QT + ti
                # one DMA: 8 transposed chunks side by side (127, 8*128)
                xgt = xs.tile([GROWS, NG * P], BF16, tag="xgt")
                nc.sync.dma_start(
                    out=xgt[:, :],
                    in_=_dram_ap(xg, t * P,
                                 [[BC, GROWS], [GROWS * BC, NG], [1, P]]))

                # conv + avg into one PSUM tile: [0:945) conv, [945:990) avg
                tp = ps.tile([P, CL + C], F32, tag="tp")
                for g, (rbase, rdata, cbase, nch) in enumerate(geo):
                    lhs = xgt[0:rdata + 1, g * P:(g + 1) * P]
                    nc.tensor.matmul(
                        tp[:, cbase:cbase + nch * L], lhs,
                        sb_cb[0:rdata + 1, cbase:cbase + nch * L],
                        start=True, stop=True)
                    cav = sum(len(ch) for ch in GROUPS[:g])
                    nc.tensor.matmul(tp[:, CL + cav:CL + cav + nch], lhs,
                                     sb_cb[0:rdata + 1, CL + cav:CL + cav + nch],
                                     start=True, stop=True)

                # avg copy (ACT); max-pool via three PSUM reduces (DVE)
                nc.scalar.activation(out=avq[:, ti * C:(ti + 1) * C],
                                     in_=tp[:, CL:CL + C], func=AF.Copy)
                nc.vector.tensor_reduce(
                    out=mxq[:, ti * 48:ti * 48 + 24],
                    in_=_ap(tp[:, 0:504], [[126, 4], [21, 6], [1, L]]),
                    axis=AX.X, op=ALU.max)
                nc.vector.tensor_reduce(
                    out=mxq[:, ti * 48 + 24:ti * 48 + 42],
                    in_=_ap(tp[:, 504:882], [[126, 3], [21, 6], [1, L]]),
                    axis=AX.X, op=ALU.max)
                nc.vector.tensor_reduce(
                    out=mxq[:, ti * 48 + 42:ti * 48 + 45],
                    in_=_ap(tp[:, 882:945], [[21, 3], [1, L]]),
                    axis=AX.X, op=ALU.max)

            # quad MLP: 4 tiles share each matmul/tanh as (.., 512) passes
            zTs = []
            for br, hq in enumerate((mxq, avq)):
                step = 48 if br == 0 else C
                hTq_ps = psm.tile([C, QT * P], BF16, tag="hTq")
                for ti in range(QT):
                    nc.tensor.transpose(
                        hTq_ps[:, ti * P:(ti + 1) * P],
                        hq[:, ti * step:ti * step + C], sb_id)
                hTq = hw.tile([C, QT * P], BF16, tag=f"hTq{br}")
                if br == 0:
                    nc.scalar.activation(out=hTq, in_=hTq_ps, func=AF.Copy)
                else:
                    nc.vector.tensor_copy(out=hTq, in_=hTq_ps)
                pmlp = psm.tile([C, QT * P], F32, tag=f"pmlp{br}")
                nc.tensor.matmul(pmlp[0:25, :], sb_w1a, hTq,
                                 start=True, stop=True)
                t1Tq = hw.tile([25, QT * P], BF16, tag=f"t1Tq{br}")
                nc.scalar.activation(out=t1Tq, in_=pmlp[0:25, :],
                                     func=AF.Tanh, bias=sb_b1c)
                nc.tensor.matmul(pmlp, sb_w2a, t1Tq, start=True, stop=True)
                zT = hw.tile([C, QT * P], BF16, tag=f"zTq{br}")
                nc.scalar.activation(out=zT, in_=pmlp, func=AF.Tanh,
                                     bias=sb_b2c)
                zTs.append(zT)

            # sum branches in SBUF; transpose back per tile (PE transpose
            # does not accumulate in PSUM); softmax per tile
            zTsum = hw.tile([C, QT * P], BF16, tag="zTsum")
            nc.vector.tensor_add(out=zTsum, in0=zTs[0], in1=zTs[1])
            zsq = psm.tile([P, QT * 48], BF16, tag="zsq")
            for ti in range(QT):
                t = q * QT + ti
                nc.tensor.transpose(zsq[:, ti * 48:ti * 48 + C],
                                    zTsum[:, ti * P:(ti + 1) * P],
                                    sb_cb[0:C, CL + 2 * C + 25:CL + 2 * C + 25 + C])
                eg = sm.tile([P, C], F16, tag=f"eg{ti}")
                ssum = sm.tile([P, 1], F32, tag=f"ssum{ti}")
                nc.scalar.activation(out=eg, in_=zsq[:, ti * 48:ti * 48 + C],
                                     func=AF.Exp, accum_out=ssum)
                rs = sm.tile([P, 1], F32, tag=f"rs{ti}")
                nc.vector.reciprocal(out=rs, in_=ssum)
                gt = sm.tile([P, C], F16, tag=f"gt{ti}")
                nc.vector.tensor_scalar_mul(out=gt, in0=eg, scalar1=rs)
                nc.scalar.dma_start(out=gate_o[t * P:(t + 1) * P, :], in_=gt)
    legalize_sync_waits(nc)
    return nc


def build_attn_program(dj):
    """Rank-1 attention via the polynomial trick.  Inputs are fp16,
    l-major (element (l,e) at l*22+e), with u pre-scaled by 1/2 on the
    host so fp16 power sums cannot overflow; dj[j] = cheb_j * 2^j are the
    shared step immediates (the num half then computes num/2, folded
    into the 4*P_e gate coefficient together with the output-side 2x
    that the host strips off again).

    Engine notes: only plain TensorScalarPtr/TensorCopy get the DVE
    2x/4x modes (scalar_tensor_tensor gets none), and walrus limits TSP
    to 2 free dims, so the hot loop uses fp16 InstTensorTensor (2x_1p)
    with 3-free-dim access patterns:
    - power stack interleaved (l, j, e): the pairwise tree reduce over l
      runs on (l, row-contiguous) patterns and its last step lands
      S'_1..S'_7 directly in (j, e) order next to the memset S'_0;
    - per-step coefficient pairs [S'_j | S'_{j+1}]*dj are prebuilt by a
      single windowed TT against a repeated-dj constant; the num-half
      coefficients additionally absorb gate*4P_e so the Horner num
      output is already the gated numerator.
    J=5 keeps end-to-end error ~2e-3 against the 2e-2 gate."""
    nc = bass.Bass()
    xsg = nc.declare_dram_parameter("xsg", [BC, EL + E], F16, isOutput=False)
    a2v = nc.declare_dram_parameter("a2v", [EL], F16, isOutput=False)
    gvv = nc.declare_dram_parameter("gvv", [EL], F16, isOutput=False)
    p2v = nc.declare_dram_parameter("p2v", [E], F16, isOutput=False)
    qvv = nc.declare_dram_parameter("qvv", [E], F16, isOutput=False)
    djv = nc.declare_dram_parameter("djv", [(J + 1) * 2 * E], F16,
                                    isOutput=False)
    o16 = nc.declare_dram_parameter("o16", [BC, 2 * EL], F16, isOutput=True)

    NJ = J + 1        # powers u'^1..u'^{J+1}
    ROW = NJ * E      # one l-row of the interleaved power stack

    with tile.TileContext(nc) as tc, ExitStack() as ctx:
        singles = ctx.enter_context(tc.tile_pool(name="singles", bufs=1))
        pstk = ctx.enter_context(tc.tile_pool(name="pstk", bufs=3))
        trp = ctx.enter_context(tc.tile_pool(name="trp", bufs=3))
        kp = ctx.enter_context(tc.tile_pool(name="kp", bufs=3))
        hp = ctx.enter_context(tc.tile_pool(name="hp", bufs=3))
        op = ctx.enter_context(tc.tile_pool(name="op", bufs=3))

        def bconst(dram, n):
            base = dram[:]
            t = singles.tile([P, n], F16, name="bc_" + dram.name)
            nc.gpsimd.dma_start(
                out=t, in_=bass.AP(tensor=base.tensor, offset=base.offset,
                                   ap=[[0, P], [1, n]]))
            return t

        a2B = bconst(a2v, EL)
        gB = bconst(gvv, EL)
        p2B = bconst(p2v, E)
        qB = bconst(qvv, E)
        djB = bconst(djv, (J + 1) * 2 * E)

        for t in range(NT):
            ug = kp.tile([P, EL + E], F16, tag="ug")  # [u' | gate_sel]
            nc.sync.dma_start(out=ug, in_=xsg[t * P:(t + 1) * P, :])
            u0 = ug[:, 0:EL]
            gst = ug[:, EL:EL + E]

            # interleaved power stack (l, j, e); ACT copies u' to slot 1
            pst = pstk.tile([P, L * ROW], F16, tag="pst")

            def slot(j):          # (l, e) view of power j
                return _ap(pst[:, (j - 1) * E:], [[ROW, L], [1, E]])

            nc.scalar.activation(out=slot(1), in_=u0, func=AF.Copy)

            # kap = (2a)u' + g   (Pool)
            kt = kp.tile([P, EL], F16, tag="kt")
            nc.gpsimd.tensor_mul(out=kt, in0=u0, in1=a2B)
            kap = kp.tile([P, EL], F16, tag="kap")
            nc.gpsimd.tensor_add(out=kap, in0=kt, in1=gB)

            # powers u'^2..: squares on ACT, odd muls on DVE/Pool
            nc.scalar.activation(out=slot(2), in_=slot(1), func=AF.Square)
            nc.gpsimd.tensor_mul(out=slot(3), in0=slot(2), in1=slot(1))
            nc.scalar.activation(out=slot(4), in_=slot(2), func=AF.Square)
            if NJ >= 5:
                nc.gpsimd.tensor_mul(out=slot(5), in0=slot(3), in1=slot(2))
            if NJ >= 6:
                nc.scalar.activation(out=slot(6), in_=slot(3), func=AF.Square)
            if NJ >= 7:
                nc.gpsimd.tensor_mul(out=slot(7), in0=slot(6), in1=slot(1))

            # pairwise tree over l; (j,e) stays contiguous throughout, so
            # the final step writes S'_1.. straight into the S-stack
            Sp = trp.tile([P, (NJ + 1) * E], F16, tag="Sp")
            nc.gpsimd.memset(Sp[:, 0:E], float(L))
            t1 = trp.tile([P, 10 * ROW], F16, tag="t1")
            t2 = trp.tile([P, 5 * ROW], F16, tag="t2")
            t3 = trp.tile([P, 2 * ROW], F16, tag="t3")
            t4 = trp.tile([P, ROW], F16, tag="t4")
            t5 = trp.tile([P, ROW], F16, tag="t5")

            pR = lambda l0, n: _ap(pst[:, l0 * ROW:], [[ROW, n], [1, ROW]])
            tR = lambda tl, l0, n: _ap(tl[:, l0 * ROW:], [[ROW, n], [1, ROW]])
            nc.vector.tensor_add(out=tR(t1, 0, 10), in0=pR(0, 10),
                                 in1=pR(10, 10))
            nc.vector.tensor_add(out=tR(t2, 0, 5), in0=tR(t1, 0, 5),
                                 in1=tR(t1, 5, 5))
            nc.vector.tensor_add(out=tR(t3, 0, 2), in0=tR(t2, 0, 2),
                                 in1=tR(t2, 2, 2))
            nc.vector.tensor_add(out=tR(t4, 0, 1), in0=tR(t3, 0, 1),
                                 in1=tR(t3, 1, 1))
            nc.vector.tensor_add(out=tR(t5, 0, 1), in0=tR(t4, 0, 1),
                                 in1=tR(t2, 4, 1))
            nc.vector.tensor_add(out=_ap(Sp[:, E:], [[1, ROW]]),
                                 in0=tR(t5, 0, 1), in1=pR(20, 1))

            # coefficient pairs SS_j = dj * [S'_j | S'_{j+1}] in one
            # windowed TT; then scale the num half by gate*4P_e
            SS = trp.tile([P, (J + 1) * 2 * E], F16, tag="SS")
            nc.vector.tensor_mul(
                out=_ap(SS[:, 0:], [[2 * E, J + 1], [1, 2 * E]]),
                in0=_ap(Sp[:, 0:], [[E, J + 1], [1, 2 * E]]),
                in1=_ap(djB[:, 0:], [[2 * E, J + 1], [1, 2 * E]]))
            gp = kp.tile([P, E], F16, tag="gp")
            nc.gpsimd.tensor_mul(out=gp, in0=gst, in1=p2B)
            nc.vector.tensor_mul(
                out=_ap(SS[:, E:], [[2 * E, J + 1], [1, E]]),
                in0=_ap(SS[:, E:], [[2 * E, J + 1], [1, E]]),
                in1=_ap(gp[:, 0:E], [[0, J + 1], [1, E]]))

            # joint Horner on [den | gated-num] with 3-free-dim TT ops
            ra = hp.tile([P, 2 * EL], F16, tag="ra")
            rb = hp.tile([P, 2 * EL], F16, tag="rb")
            f44 = lambda tl: _ap(tl[:, 0:], [[EL, 2], [E, L], [1, E]])
            cj = lambda j: _ap(SS[:, j * 2 * E:], [[E, 2], [0, L], [1, E]])
            kB = _ap(kap[:, 0:EL], [[0, 2], [E, L], [1, E]])
            cur, other = ra, rb
            nc.vector.tensor_mul(out=f44(cur), in0=cj(J), in1=kB)
            for j in range(J - 1, -1, -1):
                nc.vector.tensor_add(out=f44(other), in0=cj(j), in1=f44(cur))
                cur, other = other, cur
                if j > 0:
                    nc.vector.tensor_mul(out=f44(other), in0=f44(cur), in1=kB)
                    cur, other = other, cur

            # at2 = 2*A = gated-num/den + 2*gate*Q (host halves A on the
            # way out); gt = at2 * u' = A * x
            rd32 = op.tile([P, EL], F32, tag="rd32")
            nc.vector.reciprocal(out=rd32, in_=cur[:, 0:EL])
            atm = op.tile([P, EL], F16, tag="atm")
            nc.gpsimd.tensor_mul(out=atm, in0=cur[:, EL:2 * EL], in1=rd32)
            gq = kp.tile([P, E], F16, tag="gq")
            nc.gpsimd.tensor_mul(out=gq, in0=gst, in1=qB)
            ot = op.tile([P, 2 * EL], F16, tag="ot")
            gqB = _ap(gq[:, 0:E], [[0, L], [1, E]])
            nc.vector.tensor_add(out=ot[:, 0:EL], in0=atm, in1=gqB)
            nc.vector.tensor_mul(out=ot[:, EL:2 * EL], in0=ot[:, 0:EL],
                                 in1=u0)
            nc.scalar.dma_start(out=o16[t * P:(t + 1) * P, :], in_=ot)
    legalize_sync_waits(nc)
    return nc


def _gate_params(inputs):
    gc_w, gc_b = inputs["gc_w"], inputs["gc_b"]
    wbar = gc_w.mean(0)
    bbar = gc_b.mean()
    NCB = CL + C + 25 + C + P
    cstb = np.zeros((P, NCB), np.float32)
    cb = 0
    for g, chans in enumerate(GROUPS):
        nch = len(chans)
        for k, c in enumerate(chans):
            cstb[k * L:(k + 1) * L, cb + k * L:cb + (k + 1) * L] = gc_w.T
            cstb[k * L:(k + 1) * L, CL + c] = wbar
            cstb[nch * L, CL + c] = bbar
        cstb[nch * L, cb:cb + nch * L] = np.tile(gc_b, nch)
        cb += nch * L
    cstb[0:C, CL + C:CL + C + 25] = inputs["w1"].T
    cstb[0:25, CL + C + 25:CL + 2 * C + 25] = inputs["w2"].T
    cstb[:, CL + 2 * C + 25:NCB] = np.eye(P)
    cstf = np.zeros((C, 2), np.float32)
    cstf[0:25, 0] = inputs["b1"]
    cstf[0:C, 1] = inputs["b2"]
    return cstb.astype(ml_dtypes.bfloat16), cstf


_CACHE = {}


def kernel(**inputs):
    inputs = {k: np.ascontiguousarray(np.asarray(v)) for k, v in inputs.items()}
    x = inputs["x"].astype(np.float32)              # (B, C, L)
    bf = ml_dtypes.bfloat16
    cores = list(range(NCORES))

    # ---- launch 1: gate -------------------------------------------------
    cstb, cstf = _gate_params(inputs)
    # host-marshaled transposed x: 8 chunks of (6ch x 21 + ones row) x B
    xt = x.reshape(B, CL).T                          # (945, B)
    xg = np.zeros((NG * GROWS, B), np.float32)
    cb = 0
    for g, chans in enumerate(GROUPS):
        nch = len(chans)
        xg[g * GROWS:g * GROWS + nch * L] = xt[cb:cb + nch * L]
        xg[g * GROWS + nch * L] = 1.0
        cb += nch * L
    xg = xg.astype(bf)

    if "gate" not in _CACHE:
        _CACHE["gate"] = build_gate_program()
    nc1 = _CACHE["gate"]
    maps1 = [{"xg": np.ascontiguousarray(xg[:, i * BC:(i + 1) * BC]),
              "cstb": cstb, "cstf": cstf} for i in cores]
    r1 = run_bass_kernel_spmd(nc1, maps1, cores).results
    gate16 = np.concatenate([np.asarray(r["gate"]) for r in r1], 0)  # (B,45)

    # ---- routing (host-mediated all-reduce) -----------------------------
    mean_gate = gate16.astype(np.float64).mean(0)
    sel = np.sort(np.argsort(-mean_gate, kind="stable")[:E])

    # ---- launch 2: attention -------------------------------------------
    wq, bq = inputs["wq"], inputs["bq"]
    wk, bk = inputs["wk"], inputs["bk"]
    wv, bv = inputs["wv"], inputs["bv"]
    wo, bo = inputs["wo"], inputs["bo"]
    alpha = (wq * wk).sum(1).astype(np.float64)
    gamma = (bq * wk).sum(1).astype(np.float64)
    pv = (wo * wv).sum(1).astype(np.float64)
    qv = ((wo * bv).sum(1) + bo).astype(np.float64)

    xsel = x[:, sel, :]                              # (B, E, L)
    umax = float(np.abs(xsel).max())
    zm = (np.abs(alpha).max() * umax + np.abs(gamma).max()) * umax
    cheb = np.polynomial.chebyshev.Chebyshev.interpolate(
        np.exp, J, domain=[-zm, zm])
    dc = cheb.convert(kind=np.polynomial.Polynomial).coef
    dj = [float(dc[j] * (2.0 ** j)) for j in range(J + 1)]

    key = tuple(np.round(dj, 12))
    if _CACHE.get("attn_key") != key:
        _CACHE["attn"] = build_attn_program(dj)
        _CACHE["attn_key"] = key
    nc2 = _CACHE["attn"]

    xsg = np.empty((B, EL + E), np.float16)          # [u' l-major | gate_sel]
    xsg[:, :EL] = (xsel.transpose(0, 2, 1) * np.float32(0.5)
                   ).astype(np.float16).reshape(B, EL)
    xsg[:, EL:] = gate16[:, sel]
    a2vv = np.tile((2 * alpha).astype(np.float16), L)
    gvvv = np.tile(gamma.astype(np.float16), L)
    p2vv = (4 * pv).astype(np.float16)
    qvvv = (2 * qv).astype(np.float16)
    djvv = np.repeat(np.asarray(dj), 2 * E).astype(np.float16)
    maps2 = [{"xsg": xsg[i * BC:(i + 1) * BC],
              "a2v": a2vv, "gvv": gvvv, "p2v": p2vv, "qvv": qvvv,
              "djv": djvv}
             for i in cores]
    r2 = run_bass_kernel_spmd(nc2, maps2, cores).results
    o16 = np.concatenate([np.asarray(r["o16"]) for r in r2], 0)  # (B, 924)

    # ---- host unshard / scatter (device emits 2*A and A*x) -------------
    at = (o16[:, :EL].astype(np.float32) * 0.5).reshape(
        B, L, E).transpose(0, 2, 1)
    gt = o16[:, EL:].astype(np.float32).reshape(B, L, E).transpose(0, 2, 1)
    cols = (np.repeat(sel * L, L) + np.tile(np.arange(L), E))
    A_full = np.zeros((B, CL), np.float32)
    G_full = np.zeros((B, CL), np.float32)
    A_full[:, cols] = at.reshape(B, EL)
    G_full[:, cols] = gt.reshape(B, EL)
    return G_full, A_full


# revision 20
# speedup vs baseline: 3.0873x; 1.0756x over previous
"""Trainium2 Bass kernel for grouped-attention MoE routing.

Math (derived from the nn.Module):
  gate  = softmax(mlp(maxpool(conv(x))) + mlp(avgpool(conv(x))))      (B,45)
  sel   = sorted(top22(mean_b gate))                                  (22,)
  Per expert e with u = x[:, sel[e], :]:
    energy[l,m] = (a_e*u_l + g_e) * u_m   (rank-1; scalars a,g from weights)
    attn = softmax_m(energy);  s_l = sum_m u_m attn[l,m]
    y_l  = P_e*s_l + Q_e;      A[:,sel[e],:] = y * gate[:,sel[e]]
  G = x * A (flat);  return (G, A_flat)

Key optimization: with k = a*u_l + g, the softmax row sums are
  den_l = sum_m e^{k u_m},  num_l = sum_m u_m e^{k u_m}.
Approximating e^z by a degree-J Chebyshev fit P(z) = sum_j d_j z^j on the
realized z-range turns both into polynomials in k with power-sum
coefficients: den = sum_j d_j k^j S_j, num = sum_j d_j k^j S_{j+1}, where
S_j = sum_m u_m^j.  This replaces the O(L^2) energy tensor with O(L*J)
work (J=6 gives ~1e-3 end-to-end error vs the 2e-2 gate).

Layout: fp16 everywhere on the elementwise path with e (expert) innermost
so every scalar_tensor_tensor op hits the DVE 4x perf mode; power sums via
a pairwise tree reduce; Horner evaluation of num|den jointly (num and den
share the multiply-by-k steps and the immediate d_j coefficients once u is
pre-scaled by 1/2 on the host).

Strategy: pure data parallel over batch on 8 cores; two launches with the
45-float routing reduction mediated on host (equivalent of the all-reduce).
"""

import numpy as np
import ml_dtypes
from contextlib import ExitStack

import bass_rust
import concourse.bass as bass
import concourse.mybir as mybir
import concourse.tile as tile
from concourse.bass_utils import run_bass_kernel_spmd

_MULTIWAIT_OK = ("InstNoOp", "InstAllEngineBarrier",
                 "InstEventSemaphore", "InstUnconditionalBranch")


def legalize_sync_waits(nc):
    """walrus codegen on this stack rejects >1 sync wait on most
    instructions; hoist extra waits onto same-engine NoOps."""
    for func in nc.m.functions:
        for block in func.blocks:
            il = block.instructions
            out = []
            for inst in il:
                tname = type(inst).__name__
                si = getattr(inst, "sync_info", None)
                waits = list(si.on_wait) if si is not None else []
                if tname not in _MULTIWAIT_OK and len(waits) > 1:
                    for k, w in enumerate(waits):
                        nop = mybir.InstNoOp(
                            name=f"{inst.name}-synop{k}", ins=[], outs=[])
                        nop.engine = inst.engine
                        nop.sync_info = bass_rust.SyncInfo(
                            on_wait=[w], on_update=[])
                        out.append(nop)
                    inst.sync_info = bass_rust.SyncInfo(
                        on_wait=[], on_update=list(inst.sync_info.on_update))
                out.append(inst)
            il.clear()
            il.extend(out)


B, C, L, E = 8192, 45, 21, 22
NCORES = 8
BC = B // NCORES          # rows per core
P = 128                   # SBUF partitions
NT = BC // P              # batch tiles per core
CL = C * L                # 945
EL = E * L                # 462
J = 5                     # exp-approx polynomial degree
F32 = mybir.dt.float32
F16 = mybir.dt.float16
BF16 = mybir.dt.bfloat16
AF = mybir.ActivationFunctionType
ALU = mybir.AluOpType
AX = mybir.AxisListType
BYP = ALU.bypass

# channel groups for the gating conv matmul: 8 groups of <=6 channels
GROUPS = [list(range(g, min(g + 6, C))) for g in range(0, C, 6)]
NG = len(GROUPS)          # 8
GROWS = 127               # rows per chunk in the host-packed transposed x


def _ap(base, extra_free):
    """Custom free-dim access pattern on a tile slice: keep the partition
    dim of `base`, replace the free dims."""
    return bass.AP(tensor=base.tensor, offset=base.offset,
                   ap=[base.ap[0]] + extra_free)


def _dram_ap(dram, offset, ap):
    base = dram[:, :] if len(dram.shape) > 1 else dram[:]
    return bass.AP(tensor=base.tensor, offset=base.offset + offset, ap=ap)


def build_gate_program():
    """Gating network. x arrives host-transposed as 8 row-chunks of 127
    (6 channels x 21 taps + a ones row for bias), bf16.  Conv + avg-pool
    ride the PE as block-diagonal matmuls into a single bf16 PSUM bank
    (double-buffered); max-pool on DVE; the MLP runs transposed (bias via
    per-partition activation bias) with per-branch PSUM banks so the two
    branches and adjacent tiles overlap; softmax skips the max-subtract
    (|z| <= 2 by construction). Output gate in fp16."""
    nc = bass.Bass()
    # packed constants: one bf16 block [wcat | wav | w1a | w2a | ident]
    # (column offsets 0/945/990/1015/1060) and one f32 block [b1c | b2c]
    NCB = CL + C + 25 + C + P
    xg = nc.declare_dram_parameter("xg", [NG * GROWS, BC], BF16,
                                   isOutput=False)
    cstb = nc.declare_dram_parameter("cstb", [P, NCB], BF16, isOutput=False)
    cstf = nc.declare_dram_parameter("cstf", [C, 2], F32, isOutput=False)
    gate_o = nc.declare_dram_parameter("gate", [BC, C], F16, isOutput=True)

    # per-group geometry: (chunk row base, data rows, out-col base, n chans)
    geo = []
    cb = 0
    for g, chans in enumerate(GROUPS):
        nch = len(chans)
        geo.append((g * GROWS, nch * L, cb, nch))
        cb += nch * L

    with tile.TileContext(nc) as tc, ExitStack() as ctx:
        singles = ctx.enter_context(tc.tile_pool(name="singles", bufs=1))
        xs = ctx.enter_context(tc.tile_pool(name="xs", bufs=3))
        cp = ctx.enter_context(tc.tile_pool(name="cp", bufs=2))
        hw = ctx.enter_context(tc.tile_pool(name="hw", bufs=2))
        sm = ctx.enter_context(tc.tile_pool(name="sm", bufs=3))
        ps = ctx.enter_context(tc.tile_pool(name="ps", bufs=2, space="PSUM"))
        psm = ctx.enter_context(tc.tile_pool(name="psm", bufs=1, space="PSUM"))

        # PE-read consts funnel through DVE (one-wait matmul constraint);
        # warm-up transpose advances PE's observed DVE clock past them.
        def dve_const(dram, p, n, dt):
            raw = singles.tile([p, n], dt, name="raw_" + dram.name)
            nc.sync.dma_start(out=raw, in_=dram[:, :])
            t = singles.tile([p, n], dt, name="sb_" + dram.name)
            nc.vector.tensor_copy(out=t, in_=raw)
            return t

        sb_cb = dve_const(cstb, P, NCB, BF16)
        sb_cf = dve_const(cstf, C, 2, F32)
        sb_wcat = sb_cb[0:GROWS, 0:CL]
        sb_wav = sb_cb[0:GROWS, CL:CL + C]
        sb_w1a = sb_cb[0:C, CL + C:CL + C + 25]
        sb_w2a = sb_cb[0:25, CL + C + 25:CL + 2 * C + 25]
        sb_id = sb_cb[0:P, CL + 2 * C + 25:NCB]
        sb_b1c = sb_cf[0:25, 0:1]
        sb_b2c = sb_cf[0:C, 1:2]
        ones_col = singles.tile([P, 1], BF16)
        nc.vector.memset(ones_col, 1.0)
        warm_ps = psm.tile([C, 4 * P], BF16, tag="hTq")
        nc.tensor.transpose(warm_ps[0:1, 0:P], ones_col, sb_id)

        QT = 2                      # tiles batched through one MLP pass
        for q in range(NT // QT):
            mxq = sm.tile([P, QT * 48], BF16, tag="mxq")
            avq = sm.tile([P, QT * C], BF16, tag="avq")
            for ti in range(QT):
                t = q * QT + ti
                # one DMA: 8 transposed chunks side by side (127, 8*128)
                xgt = xs.tile([GROWS, NG * P], BF16, tag="xgt")
                nc.sync.dma_start(
                    out=xgt[:, :],
                    in_=_dram_ap(xg, t * P,
                                 [[BC, GROWS], [GROWS * BC, NG], [1, P]]))

                # conv + avg into one PSUM tile: [0:945) conv, [945:990) avg
                tp = ps.tile([P, CL + C], F32, tag="tp")
                for g, (rbase, rdata, cbase, nch) in enumerate(geo):
                    lhs = xgt[0:rdata + 1, g * P:(g + 1) * P]
                    nc.tensor.matmul(
                        tp[:, cbase:cbase + nch * L], lhs,
                        sb_cb[0:rdata + 1, cbase:cbase + nch * L],
                        start=True, stop=True)
                    cav = sum(len(ch) for ch in GROUPS[:g])
                    nc.tensor.matmul(tp[:, CL + cav:CL + cav + nch], lhs,
                                     sb_cb[0:rdata + 1, CL + cav:CL + cav + nch],
                                     start=True, stop=True)

                # avg copy (ACT); max-pool via three PSUM reduces (DVE)
                nc.scalar.activation(out=avq[:, ti * C:(ti + 1) * C],
                                     in_=tp[:, CL:CL + C], func=AF.Copy)
                nc.vector.tensor_reduce(
                    out=mxq[:, ti * 48:ti * 48 + 24],
                    in_=_ap(tp[:, 0:504], [[126, 4], [21, 6], [1, L]]),
                    axis=AX.X, op=ALU.max)
                nc.vector.tensor_reduce(
                    out=mxq[:, ti * 48 + 24:ti * 48 + 42],
                    in_=_ap(tp[:, 504:882], [[126, 3], [21, 6], [1, L]]),
                    axis=AX.X, op=ALU.max)
                nc.vector.tensor_reduce(
                    out=mxq[:, ti * 48 + 42:ti * 48 + 45],
                    in_=_ap(tp[:, 882:945], [[21, 3], [1, L]]),
                    axis=AX.X, op=ALU.max)

            # quad MLP: 4 tiles share each matmul/tanh as (.., 512) passes
            zTs = []
            for br, hq in enumerate((mxq, avq)):
                step = 48 if br == 0 else C
                hTq_ps = psm.tile([C, QT * P], BF16, tag="hTq")
                for ti in range(QT):
                    nc.tensor.transpose(
                        hTq_ps[:, ti * P:(ti + 1) * P],
                        hq[:, ti * step:ti * step + C], sb_id)
                hTq = hw.tile([C, QT * P], BF16, tag=f"hTq{br}")
                if br == 0:
                    nc.scalar.activation(out=hTq, in_=hTq_ps, func=AF.Copy)
                else:
                    nc.vector.tensor_copy(out=hTq, in_=hTq_ps)
                pmlp = psm.tile([C, QT * P], F32, tag=f"pmlp{br}")
                nc.tensor.matmul(pmlp[0:25, :], sb_w1a, hTq,
                                 start=True, stop=True)
                t1Tq = hw.tile([25, QT * P], BF16, tag=f"t1Tq{br}")
                nc.scalar.activation(out=t1Tq, in_=pmlp[0:25, :],
                                     func=AF.Tanh, bias=sb_b1c)
                nc.tensor.matmul(pmlp, sb_w2a, t1Tq, start=True, stop=True)
                zT = hw.tile([C, QT * P], BF16, tag=f"zTq{br}")
                nc.scalar.activation(out=zT, in_=pmlp, func=AF.Tanh,
                                     bias=sb_b2c)
                zTs.append(zT)

            # sum branches in SBUF; transpose back per tile (PE transpose
            # does not accumulate in PSUM); softmax per tile
            zTsum = hw.tile([C, QT * P], BF16, tag="zTsum")
            nc.vector.tensor_add(out=zTsum, in0=zTs[0], in1=zTs[1])
            zsq = psm.tile([P, QT * 48], BF16, tag="zsq")
            for ti in range(QT):
                t = q * QT + ti
                nc.tensor.transpose(zsq[:, ti * 48:ti * 48 + C],
                                    zTsum[:, ti * P:(ti + 1) * P],
                                    sb_cb[0:C, CL + 2 * C + 25:CL + 2 * C + 25 + C])
                eg = sm.tile([P, C], F16, tag=f"eg{ti}")
                ssum = sm.tile([P, 1], F32, tag=f"ssum{ti}")
                nc.scalar.activation(out=eg, in_=zsq[:, ti * 48:ti * 48 + C],
                                     func=AF.Exp, accum_out=ssum)
                rs = sm.tile([P, 1], F32, tag=f"rs{ti}")
                nc.vector.reciprocal(out=rs, in_=ssum)
                gt = sm.tile([P, C], F16, tag=f"gt{ti}")
                nc.vector.tensor_scalar_mul(out=gt, in0=eg, scalar1=rs)
                nc.scalar.dma_start(out=gate_o[t * P:(t + 1) * P, :], in_=gt)
    legalize_sync_waits(nc)
    return nc


def build_attn_program(dj):
    """Rank-1 attention via the polynomial trick.  Inputs are fp16,
    l-major (element (l,e) at l*22+e), with u pre-scaled by 1/2 on the
    host so fp16 power sums cannot overflow; dj[j] = cheb_j * 2^j are the
    shared step immediates (the num half then computes num/2, folded
    into the 4*P_e gate coefficient together with the output-side 2x
    that the host strips off again).

    Engine notes: only plain TensorScalarPtr/TensorCopy get the DVE
    2x/4x modes (scalar_tensor_tensor gets none), and walrus limits TSP
    to 2 free dims, so the hot loop uses fp16 InstTensorTensor (2x_1p)
    with 3-free-dim access patterns:
    - power stack interleaved (l, j, e): the pairwise tree reduce over l
      runs on (l, row-contiguous) patterns and its last step lands
      S'_1..S'_7 directly in (j, e) order next to the memset S'_0;
    - per-step coefficient pairs [S'_j | S'_{j+1}]*dj are prebuilt by a
      single windowed TT against a repeated-dj constant; the num-half
      coefficients additionally absorb gate*4P_e so the Horner num
      output is already the gated numerator.
    J=5 keeps end-to-end error ~2e-3 against the 2e-2 gate."""
    nc = bass.Bass()
    xsg = nc.declare_dram_parameter("xsg", [BC, EL + E], F16, isOutput=False)
    # packed broadcast constants [a2v | gvv | djv | p2v | qvv]
    NCC = 2 * EL + (J + 1) * 2 * E + 2 * E
    cstc = nc.declare_dram_parameter("cstc", [NCC], F16, isOutput=False)
    o16 = nc.declare_dram_parameter("o16", [BC, 2 * EL], F16, isOutput=True)

    NJ = J + 1        # powers u'^1..u'^{J+1}
    ROW = NJ * E      # one l-row of the interleaved power stack

    with tile.TileContext(nc) as tc, ExitStack() as ctx:
        singles = ctx.enter_context(tc.tile_pool(name="singles", bufs=1))
        pstk = ctx.enter_context(tc.tile_pool(name="pstk", bufs=4))
        trp = ctx.enter_context(tc.tile_pool(name="trp", bufs=4))
        kp = ctx.enter_context(tc.tile_pool(name="kp", bufs=4))
        hp = ctx.enter_context(tc.tile_pool(name="hp", bufs=4))
        op = ctx.enter_context(tc.tile_pool(name="op", bufs=4))

        base = cstc[:]
        cB = singles.tile([P, NCC], F16, name="bc_cstc")
        nc.gpsimd.dma_start(
            out=cB, in_=bass.AP(tensor=base.tensor, offset=base.offset,
                                ap=[[0, P], [1, NCC]]))
        a2B = cB[:, 0:EL]
        gB = cB[:, EL:2 * EL]
        djB = cB[:, 2 * EL:2 * EL + (J + 1) * 2 * E]
        p2B = cB[:, 2 * EL + (J + 1) * 2 * E:NCC - E]
        qB = cB[:, NCC - E:NCC]

        for t in range(NT):
            ug = kp.tile([P, EL + E], F16, tag="ug")  # [u' | gate_sel]
            nc.sync.dma_start(out=ug, in_=xsg[t * P:(t + 1) * P, :])
            u0 = ug[:, 0:EL]
            gst = ug[:, EL:EL + E]

            # interleaved power stack (l, j, e); ACT copies u' to slot 1
            pst = pstk.tile([P, L * ROW], F16, tag="pst")

            def slot(j):          # (l, e) view of power j
                return _ap(pst[:, (j - 1) * E:], [[ROW, L], [1, E]])

            nc.scalar.activation(out=slot(1), in_=u0, func=AF.Copy)

            # kap = (2a)u' + g   (Pool)
            kt = kp.tile([P, EL], F16, tag="kt")
            nc.gpsimd.tensor_mul(out=kt, in0=u0, in1=a2B)
            kap = kp.tile([P, EL], F16, tag="kap")
            nc.gpsimd.tensor_add(out=kap, in0=kt, in1=gB)

            # powers u'^2..: squares on ACT, odd muls on DVE/Pool
            nc.scalar.activation(out=slot(2), in_=slot(1), func=AF.Square)
            nc.gpsimd.tensor_mul(out=slot(3), in0=slot(2), in1=slot(1))
            nc.scalar.activation(out=slot(4), in_=slot(2), func=AF.Square)
            if NJ >= 5:
                nc.gpsimd.tensor_mul(out=slot(5), in0=slot(3), in1=slot(2))
            if NJ >= 6:
                nc.scalar.activation(out=slot(6), in_=slot(3), func=AF.Square)
            if NJ >= 7:
                nc.gpsimd.tensor_mul(out=slot(7), in0=slot(6), in1=slot(1))

            # pairwise tree over l; (j,e) stays contiguous throughout, so
            # the final step writes S'_1.. straight into the S-stack
            Sp = trp.tile([P, (NJ + 1) * E], F16, tag="Sp")
            nc.gpsimd.memset(Sp[:, 0:E], float(L))
            t1 = trp.tile([P, 10 * ROW], F16, tag="t1")
            t2 = trp.tile([P, 5 * ROW], F16, tag="t2")
            t3 = trp.tile([P, 2 * ROW], F16, tag="t3")
            t4 = trp.tile([P, ROW], F16, tag="t4")
            t5 = trp.tile([P, ROW], F16, tag="t5")

            pR = lambda l0, n: _ap(pst[:, l0 * ROW:], [[ROW, n], [1, ROW]])
            tR = lambda tl, l0, n: _ap(tl[:, l0 * ROW:], [[ROW, n], [1, ROW]])
            nc.vector.tensor_add(out=tR(t1, 0, 10), in0=pR(0, 10),
                                 in1=pR(10, 10))
            nc.vector.tensor_add(out=tR(t2, 0, 5), in0=tR(t1, 0, 5),
                                 in1=tR(t1, 5, 5))
            nc.vector.tensor_add(out=tR(t3, 0, 2), in0=tR(t2, 0, 2),
                                 in1=tR(t2, 2, 2))
            nc.vector.tensor_add(out=tR(t4, 0, 1), in0=tR(t3, 0, 1),
                                 in1=tR(t3, 1, 1))
            nc.vector.tensor_add(out=tR(t5, 0, 1), in0=tR(t4, 0, 1),
                                 in1=tR(t2, 4, 1))
            nc.vector.tensor_add(out=_ap(Sp[:, E:], [[1, ROW]]),
                                 in0=tR(t5, 0, 1), in1=pR(20, 1))

            # coefficient pairs SS_j = dj * [S'_j | S'_{j+1}] in one
            # windowed TT; then scale the num half by gate*4P_e
            SS = trp.tile([P, (J + 1) * 2 * E], F16, tag="SS")
            nc.vector.tensor_mul(
                out=_ap(SS[:, 0:], [[2 * E, J + 1], [1, 2 * E]]),
                in0=_ap(Sp[:, 0:], [[E, J + 1], [1, 2 * E]]),
                in1=_ap(djB, [[2 * E, J + 1], [1, 2 * E]]))
            gp = kp.tile([P, E], F16, tag="gp")
            nc.gpsimd.tensor_mul(out=gp, in0=gst, in1=p2B)
            nc.vector.tensor_mul(
                out=_ap(SS[:, E:], [[2 * E, J + 1], [1, E]]),
                in0=_ap(SS[:, E:], [[2 * E, J + 1], [1, E]]),
                in1=_ap(gp[:, 0:E], [[0, J + 1], [1, E]]))

            # joint Horner on [den | gated-num] with 3-free-dim TT ops
            ra = hp.tile([P, 2 * EL], F16, tag="ra")
            rb = hp.tile([P, 2 * EL], F16, tag="rb")
            f44 = lambda tl: _ap(tl[:, 0:], [[EL, 2], [E, L], [1, E]])
            cj = lambda j: _ap(SS[:, j * 2 * E:], [[E, 2], [0, L], [1, E]])
            kB = _ap(kap[:, 0:EL], [[0, 2], [E, L], [1, E]])
            cur, other = ra, rb
            nc.vector.tensor_mul(out=f44(cur), in0=cj(J), in1=kB)
            for j in range(J - 1, -1, -1):
                nc.vector.tensor_add(out=f44(other), in0=cj(j), in1=f44(cur))
                cur, other = other, cur
                if j > 0:
                    nc.vector.tensor_mul(out=f44(other), in0=f44(cur), in1=kB)
                    cur, other = other, cur

            # at2 = 2*A = gated-num/den + 2*gate*Q (host halves A on the
            # way out); gt = at2 * u' = A * x
            rd32 = op.tile([P, EL], F32, tag="rd32")
            nc.vector.reciprocal(out=rd32, in_=cur[:, 0:EL])
            atm = op.tile([P, EL], F16, tag="atm")
            nc.gpsimd.tensor_mul(out=atm, in0=cur[:, EL:2 * EL], in1=rd32)
            gq = kp.tile([P, E], F16, tag="gq")
            nc.gpsimd.tensor_mul(out=gq, in0=gst, in1=qB)
            ot = op.tile([P, 2 * EL], F16, tag="ot")
            gqB = _ap(gq[:, 0:E], [[0, L], [1, E]])
            nc.vector.tensor_add(out=ot[:, 0:EL], in0=atm, in1=gqB)
            nc.vector.tensor_mul(out=ot[:, EL:2 * EL], in0=ot[:, 0:EL],
                                 in1=u0)
            nc.scalar.dma_start(out=o16[t * P:(t + 1) * P, :], in_=ot)
    legalize_sync_waits(nc)
    return nc


def _gate_params(inputs):
    gc_w, gc_b = inputs["gc_w"], inputs["gc_b"]
    wbar = gc_w.mean(0)
    bbar = gc_b.mean()
    NCB = CL + C + 25 + C + P
    cstb = np.zeros((P, NCB), np.float32)
    cb = 0
    for g, chans in enumerate(GROUPS):
        nch = len(chans)
        for k, c in enumerate(chans):
            cstb[k * L:(k + 1) * L, cb + k * L:cb + (k + 1) * L] = gc_w.T
            cstb[k * L:(k + 1) * L, CL + c] = wbar
            cstb[nch * L, CL + c] = bbar
        cstb[nch * L, cb:cb + nch * L] = np.tile(gc_b, nch)
        cb += nch * L
    cstb[0:C, CL + C:CL + C + 25] = inputs["w1"].T
    cstb[0:25, CL + C + 25:CL + 2 * C + 25] = inputs["w2"].T
    cstb[:, CL + 2 * C + 25:NCB] = np.eye(P)
    cstf = np.zeros((C, 2), np.float32)
    cstf[0:25, 0] = inputs["b1"]
    cstf[0:C, 1] = inputs["b2"]
    return cstb.astype(ml_dtypes.bfloat16), cstf


_CACHE = {}


def kernel(**inputs):
    inputs = {k: np.ascontiguousarray(np.asarray(v)) for k, v in inputs.items()}
    x = inputs["x"].astype(np.float32)              # (B, C, L)
    bf = ml_dtypes.bfloat16
    cores = list(range(NCORES))

    # ---- launch 1: gate -------------------------------------------------
    cstb, cstf = _gate_params(inputs)
    # host-marshaled transposed x: 8 chunks of (6ch x 21 + ones row) x B
    xt = x.reshape(B, CL).T                          # (945, B)
    xg = np.zeros((NG * GROWS, B), np.float32)
    cb = 0
    for g, chans in enumerate(GROUPS):
        nch = len(chans)
        xg[g * GROWS:g * GROWS + nch * L] = xt[cb:cb + nch * L]
        xg[g * GROWS + nch * L] = 1.0
        cb += nch * L
    xg = xg.astype(bf)

    if "gate" not in _CACHE:
        _CACHE["gate"] = build_gate_program()
    nc1 = _CACHE["gate"]
    maps1 = [{"xg": np.ascontiguousarray(xg[:, i * BC:(i + 1) * BC]),
              "cstb": cstb, "cstf": cstf} for i in cores]
    r1 = run_bass_kernel_spmd(nc1, maps1, cores).results
    gate16 = np.concatenate([np.asarray(r["gate"]) for r in r1], 0)  # (B,45)

    # ---- routing (host-mediated all-reduce) -----------------------------
    mean_gate = gate16.astype(np.float64).mean(0)
    sel = np.sort(np.argsort(-mean_gate, kind="stable")[:E])

    # ---- launch 2: attention -------------------------------------------
    wq, bq = inputs["wq"], inputs["bq"]
    wk, bk = inputs["wk"], inputs["bk"]
    wv, bv = inputs["wv"], inputs["bv"]
    wo, bo = inputs["wo"], inputs["bo"]
    alpha = (wq * wk).sum(1).astype(np.float64)
    gamma = (bq * wk).sum(1).astype(np.float64)
    pv = (wo * wv).sum(1).astype(np.float64)
    qv = ((wo * bv).sum(1) + bo).astype(np.float64)

    xsel = x[:, sel, :]                              # (B, E, L)
    umax = float(np.abs(xsel).max())
    zm = (np.abs(alpha).max() * umax + np.abs(gamma).max()) * umax
    cheb = np.polynomial.chebyshev.Chebyshev.interpolate(
        np.exp, J, domain=[-zm, zm])
    dc = cheb.convert(kind=np.polynomial.Polynomial).coef
    dj = [float(dc[j] * (2.0 ** j)) for j in range(J + 1)]

    key = tuple(np.round(dj, 12))
    if _CACHE.get("attn_key") != key:
        _CACHE["attn"] = build_attn_program(dj)
        _CACHE["attn_key"] = key
    nc2 = _CACHE["attn"]

    xsg = np.empty((B, EL + E), np.float16)          # [u' l-major | gate_sel]
    xsg[:, :EL] = (xsel.transpose(0, 2, 1) * np.float32(0.5)
                   ).astype(np.float16).reshape(B, EL)
    xsg[:, EL:] = gate16[:, sel]
    cstc = np.concatenate([
        np.tile((2 * alpha).astype(np.float16), L),
        np.tile(gamma.astype(np.float16), L),
        np.repeat(np.asarray(dj), 2 * E).astype(np.float16),
        (4 * pv).astype(np.float16),
        (2 * qv).astype(np.float16)]).astype(np.float16)
    maps2 = [{"xsg": xsg[i * BC:(i + 1) * BC], "cstc": cstc}
             for i in cores]
    r2 = run_bass_kernel_spmd(nc2, maps2, cores).results
    o16 = np.concatenate([np.asarray(r["o16"]) for r in r2], 0)  # (B, 924)

    # ---- host unshard / scatter (device emits 2*A and A*x) -------------
    at = (o16[:, :EL].astype(np.float32) * 0.5).reshape(
        B, L, E).transpose(0, 2, 1)
    gt = o16[:, EL:].astype(np.float32).reshape(B, L, E).transpose(0, 2, 1)
    cols = (np.repeat(sel * L, L) + np.tile(np.arange(L), E))
    A_full = np.zeros((B, CL), np.float32)
    G_full = np.zeros((B, CL), np.float32)
    A_full[:, cols] = at.reshape(B, EL)
    G_full[:, cols] = gt.reshape(B, EL)
    return G_full, A_full


# revision 21
# speedup vs baseline: 3.2307x; 1.0465x over previous
"""Trainium2 Bass kernel for grouped-attention MoE routing.

Math (derived from the nn.Module):
  gate  = softmax(mlp(maxpool(conv(x))) + mlp(avgpool(conv(x))))      (B,45)
  sel   = sorted(top22(mean_b gate))                                  (22,)
  Per expert e with u = x[:, sel[e], :]:
    energy[l,m] = (a_e*u_l + g_e) * u_m   (rank-1; scalars a,g from weights)
    attn = softmax_m(energy);  s_l = sum_m u_m attn[l,m]
    y_l  = P_e*s_l + Q_e;      A[:,sel[e],:] = y * gate[:,sel[e]]
  G = x * A (flat);  return (G, A_flat)

Key optimization: with k = a*u_l + g, the softmax row sums are
  den_l = sum_m e^{k u_m},  num_l = sum_m u_m e^{k u_m}.
Approximating e^z by a degree-J Chebyshev fit P(z) = sum_j d_j z^j on the
realized z-range turns both into polynomials in k with power-sum
coefficients: den = sum_j d_j k^j S_j, num = sum_j d_j k^j S_{j+1}, where
S_j = sum_m u_m^j.  This replaces the O(L^2) energy tensor with O(L*J)
work (J=6 gives ~1e-3 end-to-end error vs the 2e-2 gate).

Layout: fp16 everywhere on the elementwise path with e (expert) innermost
so every scalar_tensor_tensor op hits the DVE 4x perf mode; power sums via
a pairwise tree reduce; Horner evaluation of num|den jointly (num and den
share the multiply-by-k steps and the immediate d_j coefficients once u is
pre-scaled by 1/2 on the host).

Strategy: pure data parallel over batch on 8 cores; two launches with the
45-float routing reduction mediated on host (equivalent of the all-reduce).
"""

import numpy as np
import ml_dtypes
from contextlib import ExitStack

import bass_rust
import concourse.bass as bass
import concourse.mybir as mybir
import concourse.tile as tile
from concourse.bass_utils import run_bass_kernel_spmd

_MULTIWAIT_OK = ("InstNoOp", "InstAllEngineBarrier",
                 "InstEventSemaphore", "InstUnconditionalBranch")


def legalize_sync_waits(nc):
    """walrus codegen on this stack rejects >1 sync wait on most
    instructions; hoist extra waits onto same-engine NoOps."""
    for func in nc.m.functions:
        for block in func.blocks:
            il = block.instructions
            out = []
            for inst in il:
                tname = type(inst).__name__
                si = getattr(inst, "sync_info", None)
                waits = list(si.on_wait) if si is not None else []
                if tname not in _MULTIWAIT_OK and len(waits) > 1:
                    for k, w in enumerate(waits):
                        nop = mybir.InstNoOp(
                            name=f"{inst.name}-synop{k}", ins=[], outs=[])
                        nop.engine = inst.engine
                        nop.sync_info = bass_rust.SyncInfo(
                            on_wait=[w], on_update=[])
                        out.append(nop)
                    inst.sync_info = bass_rust.SyncInfo(
                        on_wait=[], on_update=list(inst.sync_info.on_update))
                out.append(inst)
            il.clear()
            il.extend(out)


B, C, L, E = 8192, 45, 21, 22
NCORES = 8
BC = B // NCORES          # rows per core
P = 128                   # SBUF partitions
NT = BC // P              # batch tiles per core
CL = C * L                # 945
EL = E * L                # 462
J = 5                     # exp-approx polynomial degree
F32 = mybir.dt.float32
F16 = mybir.dt.float16
BF16 = mybir.dt.bfloat16
AF = mybir.ActivationFunctionType
ALU = mybir.AluOpType
AX = mybir.AxisListType
BYP = ALU.bypass

# channel groups for the gating conv matmul: 8 groups of <=6 channels
GROUPS = [list(range(g, min(g + 6, C))) for g in range(0, C, 6)]
NG = len(GROUPS)          # 8
GROWS = 127               # rows per chunk in the host-packed transposed x


def _ap(base, extra_free):
    """Custom free-dim access pattern on a tile slice: keep the partition
    dim of `base`, replace the free dims."""
    return bass.AP(tensor=base.tensor, offset=base.offset,
                   ap=[base.ap[0]] + extra_free)


def _dram_ap(dram, offset, ap):
    base = dram[:, :] if len(dram.shape) > 1 else dram[:]
    return bass.AP(tensor=base.tensor, offset=base.offset + offset, ap=ap)


def build_gate_program():
    """Gating network. x arrives host-transposed as 8 row-chunks of 127
    (6 channels x 21 taps + a ones row for bias), bf16.  Conv + avg-pool
    ride the PE as block-diagonal matmuls into a single bf16 PSUM bank
    (double-buffered); max-pool on DVE; the MLP runs transposed (bias via
    per-partition activation bias) with per-branch PSUM banks so the two
    branches and adjacent tiles overlap; softmax skips the max-subtract
    (|z| <= 2 by construction). Output gate in fp16."""
    nc = bass.Bass()
    # packed constants: one bf16 block [wcat | wav | w1a | w2a | ident]
    # (column offsets 0/945/990/1015/1060) and one f32 block [b1c | b2c]
    NCB = CL + C + 25 + C + P
    xg = nc.declare_dram_parameter("xg", [NG * GROWS, BC], BF16,
                                   isOutput=False)
    cstb = nc.declare_dram_parameter("cstb", [P, NCB], BF16, isOutput=False)
    cstf = nc.declare_dram_parameter("cstf", [C, 2], F32, isOutput=False)
    gate_o = nc.declare_dram_parameter("gate", [BC, C], F16, isOutput=True)

    # per-group geometry: (chunk row base, data rows, out-col base, n chans)
    geo = []
    cb = 0
    for g, chans in enumerate(GROUPS):
        nch = len(chans)
        geo.append((g * GROWS, nch * L, cb, nch))
        cb += nch * L

    with tile.TileContext(nc) as tc, ExitStack() as ctx:
        singles = ctx.enter_context(tc.tile_pool(name="singles", bufs=1))
        xs = ctx.enter_context(tc.tile_pool(name="xs", bufs=3))
        cp = ctx.enter_context(tc.tile_pool(name="cp", bufs=2))
        hw = ctx.enter_context(tc.tile_pool(name="hw", bufs=2))
        sm = ctx.enter_context(tc.tile_pool(name="sm", bufs=3))
        ps = ctx.enter_context(tc.tile_pool(name="ps", bufs=2, space="PSUM"))
        psm = ctx.enter_context(tc.tile_pool(name="psm", bufs=1, space="PSUM"))

        # PE-read consts funnel through DVE (one-wait matmul constraint);
        # warm-up transpose advances PE's observed DVE clock past them.
        def dve_const(dram, p, n, dt):
            raw = singles.tile([p, n], dt, name="raw_" + dram.name)
            nc.sync.dma_start(out=raw, in_=dram[:, :])
            t = singles.tile([p, n], dt, name="sb_" + dram.name)
            nc.vector.tensor_copy(out=t, in_=raw)
            return t

        sb_cb = dve_const(cstb, P, NCB, BF16)
        sb_cf = dve_const(cstf, C, 2, F32)
        sb_wcat = sb_cb[0:GROWS, 0:CL]
        sb_wav = sb_cb[0:GROWS, CL:CL + C]
        sb_w1a = sb_cb[0:C, CL + C:CL + C + 25]
        sb_w2a = sb_cb[0:25, CL + C + 25:CL + 2 * C + 25]
        sb_id = sb_cb[0:P, CL + 2 * C + 25:NCB]
        sb_b1c = sb_cf[0:25, 0:1]
        sb_b2c = sb_cf[0:C, 1:2]
        ones_col = singles.tile([P, 1], BF16)
        nc.vector.memset(ones_col, 1.0)
        warm_ps = psm.tile([C, 4 * P], BF16, tag="hTq")
        nc.tensor.transpose(warm_ps[0:1, 0:P], ones_col, sb_id)

        QT = 2                      # tiles batched through one MLP pass
        for q in range(NT // QT):
            mxq = sm.tile([P, QT * 48], BF16, tag="mxq")
            avq = sm.tile([P, QT * C], BF16, tag="avq")
            for ti in range(QT):
                t = q * QT + ti
                # one DMA: 8 transposed chunks side by side (127, 8*128)
                xgt = xs.tile([GROWS, NG * P], BF16, tag="xgt")
                nc.sync.dma_start(
                    out=xgt[:, :],
                    in_=_dram_ap(xg, t * P,
                                 [[BC, GROWS], [GROWS * BC, NG], [1, P]]))

                # conv + avg into one PSUM tile: [0:945) conv, [945:990) avg
                tp = ps.tile([P, CL + C], F32, tag="tp")
                for g, (rbase, rdata, cbase, nch) in enumerate(geo):
                    lhs = xgt[0:rdata + 1, g * P:(g + 1) * P]
                    nc.tensor.matmul(
                        tp[:, cbase:cbase + nch * L], lhs,
                        sb_cb[0:rdata + 1, cbase:cbase + nch * L],
                        start=True, stop=True)
                    cav = sum(len(ch) for ch in GROUPS[:g])
                    nc.tensor.matmul(tp[:, CL + cav:CL + cav + nch], lhs,
                                     sb_cb[0:rdata + 1, CL + cav:CL + cav + nch],
                                     start=True, stop=True)

                # avg copy (ACT); max-pool via three PSUM reduces (DVE)
                nc.scalar.activation(out=avq[:, ti * C:(ti + 1) * C],
                                     in_=tp[:, CL:CL + C], func=AF.Copy)
                nc.vector.tensor_reduce(
                    out=mxq[:, ti * 48:ti * 48 + 24],
                    in_=_ap(tp[:, 0:504], [[126, 4], [21, 6], [1, L]]),
                    axis=AX.X, op=ALU.max)
                nc.vector.tensor_reduce(
                    out=mxq[:, ti * 48 + 24:ti * 48 + 42],
                    in_=_ap(tp[:, 504:882], [[126, 3], [21, 6], [1, L]]),
                    axis=AX.X, op=ALU.max)
                nc.vector.tensor_reduce(
                    out=mxq[:, ti * 48 + 42:ti * 48 + 45],
                    in_=_ap(tp[:, 882:945], [[21, 3], [1, L]]),
                    axis=AX.X, op=ALU.max)

            # quad MLP: 4 tiles share each matmul/tanh as (.., 512) passes
            zTs = []
            for br, hq in enumerate((mxq, avq)):
                step = 48 if br == 0 else C
                hTq_ps = psm.tile([C, QT * P], BF16, tag="hTq")
                for ti in range(QT):
                    nc.tensor.transpose(
                        hTq_ps[:, ti * P:(ti + 1) * P],
                        hq[:, ti * step:ti * step + C], sb_id)
                hTq = hw.tile([C, QT * P], BF16, tag=f"hTq{br}")
                if br == 0:
                    nc.scalar.activation(out=hTq, in_=hTq_ps, func=AF.Copy)
                else:
                    nc.vector.tensor_copy(out=hTq, in_=hTq_ps)
                pmlp = psm.tile([C, QT * P], F32, tag=f"pmlp{br}")
                nc.tensor.matmul(pmlp[0:25, :], sb_w1a, hTq,
                                 start=True, stop=True)
                t1Tq = hw.tile([25, QT * P], BF16, tag=f"t1Tq{br}")
                nc.scalar.activation(out=t1Tq, in_=pmlp[0:25, :],
                                     func=AF.Tanh, bias=sb_b1c)
                nc.tensor.matmul(pmlp, sb_w2a, t1Tq, start=True, stop=True)
                zT = hw.tile([C, QT * P], BF16, tag=f"zTq{br}")
                nc.scalar.activation(out=zT, in_=pmlp, func=AF.Tanh,
                                     bias=sb_b2c)
                zTs.append(zT)

            # sum branches in SBUF; transpose back per tile (PE transpose
            # does not accumulate in PSUM); softmax per tile
            zTsum = hw.tile([C, QT * P], BF16, tag="zTsum")
            nc.vector.tensor_add(out=zTsum, in0=zTs[0], in1=zTs[1])
            zsq = psm.tile([P, QT * 48], BF16, tag="zsq")
            for ti in range(QT):
                t = q * QT + ti
                nc.tensor.transpose(zsq[:, ti * 48:ti * 48 + C],
                                    zTsum[:, ti * P:(ti + 1) * P],
                                    sb_cb[0:C, CL + 2 * C + 25:CL + 2 * C + 25 + C])
                eg = sm.tile([P, C], F16, tag=f"eg{ti}")
                ssum = sm.tile([P, 1], F32, tag=f"ssum{ti}")
                nc.scalar.activation(out=eg, in_=zsq[:, ti * 48:ti * 48 + C],
                                     func=AF.Exp, accum_out=ssum)
                rs = sm.tile([P, 1], F32, tag=f"rs{ti}")
                nc.vector.reciprocal(out=rs, in_=ssum)
                gt = sm.tile([P, C], F16, tag=f"gt{ti}")
                nc.vector.tensor_scalar_mul(out=gt, in0=eg, scalar1=rs)
                nc.scalar.dma_start(out=gate_o[t * P:(t + 1) * P, :], in_=gt)
    legalize_sync_waits(nc)
    return nc


def build_attn_program(dj):
    """Rank-1 attention via the polynomial trick.  The host ships, per
    row and in l-major fp16 (element (l,e) at l*22+e): u' = x_sel/2
    (halved so fp16 power sums cannot overflow), kap = a*x_sel + g, and
    the selected gate row.  dj[j] = cheb_j * 2^j are shared step
    immediates.

    den and the gated numerator M = gp*num/2 + gq*den (gp = 4P_e*gate,
    gq = 2Q_e*gate) are evaluated jointly by one Horner pass over a
    duplicated-expert axis eh=44: the step-j coefficient is the
    contiguous 44-wide slice [dj*S'_j | dj*(gp*S'_{j+1}+gq*S'_j)] of a
    prebuilt stack, broadcast over l; then 2*A = M/den and A*x follow.
    Only plain TensorScalarPtr/TensorCopy get DVE 2x/4x modes and walrus
    limits TSP to 2 free dims, so the hot loop is fp16 InstTensorTensor
    (2x_1p) with 3-free-dim access patterns; power sums use a pairwise
    tree over l on an (l, j, e)-interleaved power stack."""
    nc = bass.Bass()
    W3 = 2 * EL + E
    xsg = nc.declare_dram_parameter("xsg", [BC, W3], F16, isOutput=False)
    # packed broadcast constants [djv | p2v | qvv]
    NCC = (J + 1) * 2 * E + 2 * E
    cstc = nc.declare_dram_parameter("cstc", [NCC], F16, isOutput=False)
    o16 = nc.declare_dram_parameter("o16", [BC, 2 * EL], F16, isOutput=True)

    NJ = J + 1        # powers u'^1..u'^{J+1}
    ROW = NJ * E      # one l-row of the interleaved power stack

    with tile.TileContext(nc) as tc, ExitStack() as ctx:
        singles = ctx.enter_context(tc.tile_pool(name="singles", bufs=1))
        pstk = ctx.enter_context(tc.tile_pool(name="pstk", bufs=4))
        trp = ctx.enter_context(tc.tile_pool(name="trp", bufs=4))
        kp = ctx.enter_context(tc.tile_pool(name="kp", bufs=4))
        hp = ctx.enter_context(tc.tile_pool(name="hp", bufs=4))
        op = ctx.enter_context(tc.tile_pool(name="op", bufs=4))

        base = cstc[:]
        cB = singles.tile([P, NCC], F16, name="bc_cstc")
        nc.gpsimd.dma_start(
            out=cB, in_=bass.AP(tensor=base.tensor, offset=base.offset,
                                ap=[[0, P], [1, NCC]]))
        djB = cB[:, 0:(J + 1) * 2 * E]
        p2B = cB[:, NCC - 2 * E:NCC - E]
        qB = cB[:, NCC - E:NCC]

        for t in range(NT):
            ug = kp.tile([P, W3], F16, tag="ug")     # [u' | kap | gate_sel]
            nc.sync.dma_start(out=ug, in_=xsg[t * P:(t + 1) * P, :])
            u0 = ug[:, 0:EL]
            kapv = ug[:, EL:2 * EL]
            gst = ug[:, 2 * EL:W3]

            # interleaved power stack (l, j, e); ACT copies u' to slot 1
            pst = pstk.tile([P, L * ROW], F16, tag="pst")

            def slot(j):          # (l, e) view of power j
                return _ap(pst[:, (j - 1) * E:], [[ROW, L], [1, E]])

            nc.scalar.activation(out=slot(1), in_=u0, func=AF.Copy)

            # powers u'^2..u'^{J+1}: squares on ACT, odd muls on DVE/Pool
            nc.scalar.activation(out=slot(2), in_=u0, func=AF.Square)
            nc.vector.tensor_mul(out=slot(3), in0=slot(2), in1=slot(1))
            nc.scalar.activation(out=slot(4), in_=slot(2), func=AF.Square)
            if NJ >= 5:
                nc.gpsimd.tensor_mul(out=slot(5), in0=slot(3), in1=slot(2))
            if NJ >= 6:
                nc.scalar.activation(out=slot(6), in_=slot(3), func=AF.Square)
            if NJ >= 7:
                nc.gpsimd.tensor_mul(out=slot(7), in0=slot(6), in1=slot(1))

            # pairwise tree over l; (j,e) stays contiguous throughout, so
            # the final step writes S'_1.. straight into the S-stack
            Sp = trp.tile([P, (NJ + 1) * E], F16, tag="Sp")
            nc.gpsimd.memset(Sp[:, 0:E], float(L))
            t1 = trp.tile([P, 10 * ROW], F16, tag="t1")
            t2 = trp.tile([P, 5 * ROW], F16, tag="t2")
            t3 = trp.tile([P, 2 * ROW], F16, tag="t3")
            t4 = trp.tile([P, ROW], F16, tag="t4")
            t5 = trp.tile([P, ROW], F16, tag="t5")

            pR = lambda l0, n: _ap(pst[:, l0 * ROW:], [[ROW, n], [1, ROW]])
            tR = lambda tl, l0, n: _ap(tl[:, l0 * ROW:], [[ROW, n], [1, ROW]])
            nc.vector.tensor_add(out=tR(t1, 0, 10), in0=pR(0, 10),
                                 in1=pR(10, 10))
            nc.vector.tensor_add(out=tR(t2, 0, 5), in0=tR(t1, 0, 5),
                                 in1=tR(t1, 5, 5))
            nc.vector.tensor_add(out=tR(t3, 0, 2), in0=tR(t2, 0, 2),
                                 in1=tR(t2, 2, 2))
            nc.gpsimd.tensor_add(out=tR(t4, 0, 1), in0=tR(t3, 0, 1),
                                 in1=tR(t3, 1, 1))
            nc.gpsimd.tensor_add(out=tR(t5, 0, 1), in0=tR(t4, 0, 1),
                                 in1=tR(t2, 4, 1))
            nc.gpsimd.tensor_add(out=_ap(Sp[:, E:], [[1, ROW]]),
                                 in0=tR(t5, 0, 1), in1=pR(20, 1))

            # coefficient pairs: [dj*S'_j | dj*(gp*S'_{j+1} + gq*S'_j)]
            gp = kp.tile([P, E], F16, tag="gp")
            nc.gpsimd.tensor_mul(out=gp, in0=gst, in1=p2B)
            gq = kp.tile([P, E], F16, tag="gq")
            nc.gpsimd.tensor_mul(out=gq, in0=gst, in1=qB)
            SS = trp.tile([P, (J + 1) * 2 * E], F16, tag="SS")
            tq = trp.tile([P, (J + 1) * E], F16, tag="tq")
            wJ = lambda tl, off: _ap(tl[:, off:], [[E, J + 1], [1, E]])
            wS = lambda off: _ap(SS[:, off:], [[2 * E, J + 1], [1, E]])
            wD = lambda off: _ap(djB[:, off:], [[2 * E, J + 1], [1, E]])
            gpB = _ap(gp[:, 0:E], [[0, J + 1], [1, E]])
            gqB = _ap(gq[:, 0:E], [[0, J + 1], [1, E]])
            nc.vector.tensor_mul(out=wS(0), in0=wJ(Sp, 0), in1=wD(0))
            nc.vector.tensor_mul(out=wS(E), in0=wJ(Sp, E), in1=gpB)
            nc.vector.tensor_mul(out=wJ(tq, 0), in0=wJ(Sp, 0), in1=gqB)
            nc.vector.tensor_add(out=wS(E), in0=wS(E), in1=wJ(tq, 0))
            nc.vector.tensor_mul(out=wS(E), in0=wS(E), in1=wD(E))

            # joint Horner on [den | M] with 3-free-dim TT ops
            ra = hp.tile([P, 2 * EL], F16, tag="ra")
            rb = hp.tile([P, 2 * EL], F16, tag="rb")
            f44 = lambda tl: _ap(tl[:, 0:], [[EL, 2], [E, L], [1, E]])
            cj = lambda j: _ap(SS[:, j * 2 * E:], [[E, 2], [0, L], [1, E]])
            kB = _ap(ug[:, EL:], [[0, 2], [E, L], [1, E]])
            cur, other = ra, rb
            nc.vector.tensor_mul(out=f44(cur), in0=cj(J), in1=kB)
            for j in range(J - 1, -1, -1):
                nc.vector.tensor_add(out=f44(other), in0=cj(j), in1=f44(cur))
                cur, other = other, cur
                if j > 0:
                    nc.vector.tensor_mul(out=f44(other), in0=f44(cur), in1=kB)
                    cur, other = other, cur

            # 2*A = M/den (host halves A on the way out); A*x = 2A * u'
            rd32 = op.tile([P, EL], F32, tag="rd32")
            nc.vector.reciprocal(out=rd32, in_=cur[:, 0:EL])
            ot = op.tile([P, 2 * EL], F16, tag="ot")
            nc.gpsimd.tensor_mul(out=ot[:, 0:EL], in0=cur[:, EL:2 * EL],
                                 in1=rd32)
            nc.gpsimd.tensor_mul(out=ot[:, EL:2 * EL], in0=ot[:, 0:EL],
                                 in1=u0)
            nc.scalar.dma_start(out=o16[t * P:(t + 1) * P, :], in_=ot)
    legalize_sync_waits(nc)
    return nc


def _gate_params(inputs):
    gc_w, gc_b = inputs["gc_w"], inputs["gc_b"]
    wbar = gc_w.mean(0)
    bbar = gc_b.mean()
    NCB = CL + C + 25 + C + P
    cstb = np.zeros((P, NCB), np.float32)
    cb = 0
    for g, chans in enumerate(GROUPS):
        nch = len(chans)
        for k, c in enumerate(chans):
            cstb[k * L:(k + 1) * L, cb + k * L:cb + (k + 1) * L] = gc_w.T
            cstb[k * L:(k + 1) * L, CL + c] = wbar
            cstb[nch * L, CL + c] = bbar
        cstb[nch * L, cb:cb + nch * L] = np.tile(gc_b, nch)
        cb += nch * L
    cstb[0:C, CL + C:CL + C + 25] = inputs["w1"].T
    cstb[0:25, CL + C + 25:CL + 2 * C + 25] = inputs["w2"].T
    cstb[:, CL + 2 * C + 25:NCB] = np.eye(P)
    cstf = np.zeros((C, 2), np.float32)
    cstf[0:25, 0] = inputs["b1"]
    cstf[0:C, 1] = inputs["b2"]
    return cstb.astype(ml_dtypes.bfloat16), cstf


_CACHE = {}


def kernel(**inputs):
    inputs = {k: np.ascontiguousarray(np.asarray(v)) for k, v in inputs.items()}
    x = inputs["x"].astype(np.float32)              # (B, C, L)
    bf = ml_dtypes.bfloat16
    cores = list(range(NCORES))

    # ---- launch 1: gate -------------------------------------------------
    cstb, cstf = _gate_params(inputs)
    # host-marshaled transposed x: 8 chunks of (6ch x 21 + ones row) x B
    xt = x.reshape(B, CL).T                          # (945, B)
    xg = np.zeros((NG * GROWS, B), np.float32)
    cb = 0
    for g, chans in enumerate(GROUPS):
        nch = len(chans)
        xg[g * GROWS:g * GROWS + nch * L] = xt[cb:cb + nch * L]
        xg[g * GROWS + nch * L] = 1.0
        cb += nch * L
    xg = xg.astype(bf)

    if "gate" not in _CACHE:
        _CACHE["gate"] = build_gate_program()
    nc1 = _CACHE["gate"]
    maps1 = [{"xg": np.ascontiguousarray(xg[:, i * BC:(i + 1) * BC]),
              "cstb": cstb, "cstf": cstf} for i in cores]
    r1 = run_bass_kernel_spmd(nc1, maps1, cores).results
    gate16 = np.concatenate([np.asarray(r["gate"]) for r in r1], 0)  # (B,45)

    # ---- routing (host-mediated all-reduce) -----------------------------
    mean_gate = gate16.astype(np.float64).mean(0)
    sel = np.sort(np.argsort(-mean_gate, kind="stable")[:E])

    # ---- launch 2: attention -------------------------------------------
    wq, bq = inputs["wq"], inputs["bq"]
    wk, bk = inputs["wk"], inputs["bk"]
    wv, bv = inputs["wv"], inputs["bv"]
    wo, bo = inputs["wo"], inputs["bo"]
    alpha = (wq * wk).sum(1).astype(np.float64)
    gamma = (bq * wk).sum(1).astype(np.float64)
    pv = (wo * wv).sum(1).astype(np.float64)
    qv = ((wo * bv).sum(1) + bo).astype(np.float64)

    xsel = x[:, sel, :]                              # (B, E, L)
    umax = float(np.abs(xsel).max())
    zm = (np.abs(alpha).max() * umax + np.abs(gamma).max()) * umax
    cheb = np.polynomial.chebyshev.Chebyshev.interpolate(
        np.exp, J, domain=[-zm, zm])
    dc = cheb.convert(kind=np.polynomial.Polynomial).coef
    dj = [float(dc[j] * (2.0 ** j)) for j in range(J + 1)]

    key = tuple(np.round(dj, 12))
    if _CACHE.get("attn_key") != key:
        _CACHE["attn"] = build_attn_program(dj)
        _CACHE["attn_key"] = key
    nc2 = _CACHE["attn"]

    xsg = np.empty((B, 2 * EL + E), np.float16)  # [u' | kap | gate] l-major
    xlm = np.ascontiguousarray(xsel.transpose(0, 2, 1).astype(np.float32))
    xsg[:, :EL] = (xlm * np.float32(0.5)).reshape(B, EL)
    xsg[:, EL:2 * EL] = (xlm * alpha.astype(np.float32)[None, None, :]
                         + gamma.astype(np.float32)[None, None, :]
                         ).reshape(B, EL)
    xsg[:, 2 * EL:] = gate16[:, sel]
    cstc = np.concatenate([
        np.repeat(np.asarray(dj), 2 * E).astype(np.float16),
        (4 * pv).astype(np.float16),
        (2 * qv).astype(np.float16)]).astype(np.float16)
    maps2 = [{"xsg": xsg[i * BC:(i + 1) * BC], "cstc": cstc}
             for i in cores]
    r2 = run_bass_kernel_spmd(nc2, maps2, cores).results
    o16 = np.concatenate([np.asarray(r["o16"]) for r in r2], 0)  # (B, 924)

    # ---- host unshard / scatter (device emits 2*A and A*x) -------------
    at = (o16[:, :EL].astype(np.float32) * 0.5).reshape(
        B, L, E).transpose(0, 2, 1)
    gt = o16[:, EL:].astype(np.float32).reshape(B, L, E).transpose(0, 2, 1)
    cols = (np.repeat(sel * L, L) + np.tile(np.arange(L), E))
    A_full = np.zeros((B, CL), np.float32)
    G_full = np.zeros((B, CL), np.float32)
    A_full[:, cols] = at.reshape(B, EL)
    G_full[:, cols] = gt.reshape(B, EL)
    return G_full, A_full


# revision 22
# speedup vs baseline: 3.5170x; 1.0886x over previous
"""Trainium2 Bass kernel for grouped-attention MoE routing.

Math (derived from the nn.Module):
  gate  = softmax(mlp(maxpool(conv(x))) + mlp(avgpool(conv(x))))      (B,45)
  sel   = sorted(top22(mean_b gate))                                  (22,)
  Per expert e with u = x[:, sel[e], :]:
    energy[l,m] = (a_e*u_l + g_e) * u_m   (rank-1; scalars a,g from weights)
    attn = softmax_m(energy);  s_l = sum_m u_m attn[l,m]
    y_l  = P_e*s_l + Q_e;      A[:,sel[e],:] = y * gate[:,sel[e]]
  G = x * A (flat);  return (G, A_flat)

Key optimization: with k = a*u_l + g, the softmax row sums are
  den_l = sum_m e^{k u_m},  num_l = sum_m u_m e^{k u_m}.
Approximating e^z by a degree-J Chebyshev fit P(z) = sum_j d_j z^j on the
realized z-range turns both into polynomials in k with power-sum
coefficients: den = sum_j d_j k^j S_j, num = sum_j d_j k^j S_{j+1}, where
S_j = sum_m u_m^j.  This replaces the O(L^2) energy tensor with O(L*J)
work (J=6 gives ~1e-3 end-to-end error vs the 2e-2 gate).

Layout: fp16 everywhere on the elementwise path with e (expert) innermost
so every scalar_tensor_tensor op hits the DVE 4x perf mode; power sums via
a pairwise tree reduce; Horner evaluation of num|den jointly (num and den
share the multiply-by-k steps and the immediate d_j coefficients once u is
pre-scaled by 1/2 on the host).

Strategy: pure data parallel over batch on 8 cores; two launches with the
45-float routing reduction mediated on host (equivalent of the all-reduce).
"""

import numpy as np
import ml_dtypes
from contextlib import ExitStack

import bass_rust
import concourse.bass as bass
import concourse.mybir as mybir
import concourse.tile as tile
from concourse.bass_utils import run_bass_kernel_spmd

_MULTIWAIT_OK = ("InstNoOp", "InstAllEngineBarrier",
                 "InstEventSemaphore", "InstUnconditionalBranch")


def legalize_sync_waits(nc):
    """walrus codegen on this stack rejects >1 sync wait on most
    instructions; hoist extra waits onto same-engine NoOps."""
    for func in nc.m.functions:
        for block in func.blocks:
            il = block.instructions
            out = []
            for inst in il:
                tname = type(inst).__name__
                si = getattr(inst, "sync_info", None)
                waits = list(si.on_wait) if si is not None else []
                if tname not in _MULTIWAIT_OK and len(waits) > 1:
                    for k, w in enumerate(waits):
                        nop = mybir.InstNoOp(
                            name=f"{inst.name}-synop{k}", ins=[], outs=[])
                        nop.engine = inst.engine
                        nop.sync_info = bass_rust.SyncInfo(
                            on_wait=[w], on_update=[])
                        out.append(nop)
                    inst.sync_info = bass_rust.SyncInfo(
                        on_wait=[], on_update=list(inst.sync_info.on_update))
                out.append(inst)
            il.clear()
            il.extend(out)


B, C, L, E = 8192, 45, 21, 22
NCORES = 8
BC = B // NCORES          # rows per core
P = 128                   # SBUF partitions
NT = BC // P              # batch tiles per core
CL = C * L                # 945
EL = E * L                # 462
J = 4                     # exp-approx polynomial degree
F32 = mybir.dt.float32
F16 = mybir.dt.float16
BF16 = mybir.dt.bfloat16
AF = mybir.ActivationFunctionType
ALU = mybir.AluOpType
AX = mybir.AxisListType
BYP = ALU.bypass

# channel groups for the gating conv matmul: 8 groups of <=6 channels
GROUPS = [list(range(g, min(g + 6, C))) for g in range(0, C, 6)]
NG = len(GROUPS)          # 8
GROWS = 127               # rows per chunk in the host-packed transposed x


def _ap(base, extra_free):
    """Custom free-dim access pattern on a tile slice: keep the partition
    dim of `base`, replace the free dims."""
    return bass.AP(tensor=base.tensor, offset=base.offset,
                   ap=[base.ap[0]] + extra_free)


def _dram_ap(dram, offset, ap):
    base = dram[:, :] if len(dram.shape) > 1 else dram[:]
    return bass.AP(tensor=base.tensor, offset=base.offset + offset, ap=ap)


def build_gate_program():
    """Gating network. x arrives host-transposed as 8 row-chunks of 127
    (6 channels x 21 taps + a ones row for bias), bf16.  Conv + avg-pool
    ride the PE as block-diagonal matmuls into a single bf16 PSUM bank
    (double-buffered); max-pool on DVE; the MLP runs transposed (bias via
    per-partition activation bias) with per-branch PSUM banks so the two
    branches and adjacent tiles overlap; softmax skips the max-subtract
    (|z| <= 2 by construction). Output gate in fp16."""
    nc = bass.Bass()
    # packed constants: one bf16 block [wcat | wav | w1a | w2a | ident]
    # (column offsets 0/945/990/1015/1060) and one f32 block [b1c | b2c]
    NCB = CL + C + 25 + C + P
    xg = nc.declare_dram_parameter("xg", [NG * GROWS, BC], BF16,
                                   isOutput=False)
    cstb = nc.declare_dram_parameter("cstb", [P, NCB], BF16, isOutput=False)
    cstf = nc.declare_dram_parameter("cstf", [C, 2], F32, isOutput=False)
    gate_o = nc.declare_dram_parameter("gate", [BC, C], F16, isOutput=True)

    # per-group geometry: (chunk row base, data rows, out-col base, n chans)
    geo = []
    cb = 0
    for g, chans in enumerate(GROUPS):
        nch = len(chans)
        geo.append((g * GROWS, nch * L, cb, nch))
        cb += nch * L

    with tile.TileContext(nc) as tc, ExitStack() as ctx:
        singles = ctx.enter_context(tc.tile_pool(name="singles", bufs=1))
        xs = ctx.enter_context(tc.tile_pool(name="xs", bufs=3))
        cp = ctx.enter_context(tc.tile_pool(name="cp", bufs=2))
        hw = ctx.enter_context(tc.tile_pool(name="hw", bufs=2))
        sm = ctx.enter_context(tc.tile_pool(name="sm", bufs=3))
        ps = ctx.enter_context(tc.tile_pool(name="ps", bufs=2, space="PSUM"))
        psm = ctx.enter_context(tc.tile_pool(name="psm", bufs=1, space="PSUM"))

        # PE-read consts funnel through DVE (one-wait matmul constraint);
        # warm-up transpose advances PE's observed DVE clock past them.
        def dve_const(dram, p, n, dt):
            raw = singles.tile([p, n], dt, name="raw_" + dram.name)
            nc.sync.dma_start(out=raw, in_=dram[:, :])
            t = singles.tile([p, n], dt, name="sb_" + dram.name)
            nc.vector.tensor_copy(out=t, in_=raw)
            return t

        sb_cb = dve_const(cstb, P, NCB, BF16)
        sb_cf = dve_const(cstf, C, 2, F32)
        sb_wcat = sb_cb[0:GROWS, 0:CL]
        sb_wav = sb_cb[0:GROWS, CL:CL + C]
        sb_w1a = sb_cb[0:C, CL + C:CL + C + 25]
        sb_w2a = sb_cb[0:25, CL + C + 25:CL + 2 * C + 25]
        sb_id = sb_cb[0:P, CL + 2 * C + 25:NCB]
        sb_b1c = sb_cf[0:25, 0:1]
        sb_b2c = sb_cf[0:C, 1:2]
        ones_col = singles.tile([P, 1], BF16)
        nc.vector.memset(ones_col, 1.0)
        warm_ps = psm.tile([C, 4 * P], BF16, tag="hTq")
        nc.tensor.transpose(warm_ps[0:1, 0:P], ones_col, sb_id)

        QT = 2                      # tiles batched through one MLP pass
        for q in range(NT // QT):
            mxq = sm.tile([P, QT * 48], BF16, tag="mxq")
            avq = sm.tile([P, QT * C], BF16, tag="avq")
            for ti in range(QT):
                t = q * QT + ti
                # one DMA: 8 transposed chunks side by side (127, 8*128)
                xgt = xs.tile([GROWS, NG * P], BF16, tag="xgt")
                nc.sync.dma_start(
                    out=xgt[:, :],
                    in_=_dram_ap(xg, t * P,
                                 [[BC, GROWS], [GROWS * BC, NG], [1, P]]))

                # conv + avg into one PSUM tile: [0:945) conv, [945:990) avg
                tp = ps.tile([P, CL + C], F32, tag="tp")
                for g, (rbase, rdata, cbase, nch) in enumerate(geo):
                    lhs = xgt[0:rdata + 1, g * P:(g + 1) * P]
                    nc.tensor.matmul(
                        tp[:, cbase:cbase + nch * L], lhs,
                        sb_cb[0:rdata + 1, cbase:cbase + nch * L],
                        start=True, stop=True)
                    cav = sum(len(ch) for ch in GROUPS[:g])
                    nc.tensor.matmul(tp[:, CL + cav:CL + cav + nch], lhs,
                                     sb_cb[0:rdata + 1, CL + cav:CL + cav + nch],
                                     start=True, stop=True)

                # avg copy (ACT); max-pool via three PSUM reduces (DVE)
                nc.scalar.activation(out=avq[:, ti * C:(ti + 1) * C],
                                     in_=tp[:, CL:CL + C], func=AF.Copy)
                nc.vector.tensor_reduce(
                    out=mxq[:, ti * 48:ti * 48 + 24],
                    in_=_ap(tp[:, 0:504], [[126, 4], [21, 6], [1, L]]),
                    axis=AX.X, op=ALU.max)
                nc.vector.tensor_reduce(
                    out=mxq[:, ti * 48 + 24:ti * 48 + 42],
                    in_=_ap(tp[:, 504:882], [[126, 3], [21, 6], [1, L]]),
                    axis=AX.X, op=ALU.max)
                nc.vector.tensor_reduce(
                    out=mxq[:, ti * 48 + 42:ti * 48 + 45],
                    in_=_ap(tp[:, 882:945], [[21, 3], [1, L]]),
                    axis=AX.X, op=ALU.max)

            # quad MLP: 4 tiles share each matmul/tanh as (.., 512) passes
            zTs = []
            for br, hq in enumerate((mxq, avq)):
                step = 48 if br == 0 else C
                hTq_ps = psm.tile([C, QT * P], BF16, tag="hTq")
                for ti in range(QT):
                    nc.tensor.transpose(
                        hTq_ps[:, ti * P:(ti + 1) * P],
                        hq[:, ti * step:ti * step + C], sb_id)
                hTq = hw.tile([C, QT * P], BF16, tag=f"hTq{br}")
                if br == 0:
                    nc.scalar.activation(out=hTq, in_=hTq_ps, func=AF.Copy)
                else:
                    nc.vector.tensor_copy(out=hTq, in_=hTq_ps)
                pmlp = psm.tile([C, QT * P], F32, tag=f"pmlp{br}")
                nc.tensor.matmul(pmlp[0:25, :], sb_w1a, hTq,
                                 start=True, stop=True)
                t1Tq = hw.tile([25, QT * P], BF16, tag=f"t1Tq{br}")
                nc.scalar.activation(out=t1Tq, in_=pmlp[0:25, :],
                                     func=AF.Tanh, bias=sb_b1c)
                nc.tensor.matmul(pmlp, sb_w2a, t1Tq, start=True, stop=True)
                zT = hw.tile([C, QT * P], BF16, tag=f"zTq{br}")
                nc.scalar.activation(out=zT, in_=pmlp, func=AF.Tanh,
                                     bias=sb_b2c)
                zTs.append(zT)

            # sum branches in SBUF; transpose back per tile (PE transpose
            # does not accumulate in PSUM); softmax per tile
            zTsum = hw.tile([C, QT * P], BF16, tag="zTsum")
            nc.vector.tensor_add(out=zTsum, in0=zTs[0], in1=zTs[1])
            zsq = psm.tile([P, QT * 48], BF16, tag="zsq")
            for ti in range(QT):
                t = q * QT + ti
                nc.tensor.transpose(zsq[:, ti * 48:ti * 48 + C],
                                    zTsum[:, ti * P:(ti + 1) * P],
                                    sb_cb[0:C, CL + 2 * C + 25:CL + 2 * C + 25 + C])
                eg = sm.tile([P, C], F16, tag=f"eg{ti}")
                ssum = sm.tile([P, 1], F32, tag=f"ssum{ti}")
                nc.scalar.activation(out=eg, in_=zsq[:, ti * 48:ti * 48 + C],
                                     func=AF.Exp, accum_out=ssum)
                rs = sm.tile([P, 1], F32, tag=f"rs{ti}")
                nc.vector.reciprocal(out=rs, in_=ssum)
                gt = sm.tile([P, C], F16, tag=f"gt{ti}")
                nc.vector.tensor_scalar_mul(out=gt, in0=eg, scalar1=rs)
                nc.scalar.dma_start(out=gate_o[t * P:(t + 1) * P, :], in_=gt)
    legalize_sync_waits(nc)
    return nc


def build_attn_program(dj):
    """Rank-1 attention via the polynomial trick.  The host ships, per
    row and in l-major fp16 (element (l,e) at l*22+e): u' = x_sel/2
    (halved so fp16 power sums cannot overflow), kap = a*x_sel + g, and
    the selected gate row.  dj[j] = cheb_j * 2^j are shared step
    immediates.

    den and the gated numerator M = gp*num/2 + gq*den (gp = 4P_e*gate,
    gq = 2Q_e*gate) are evaluated jointly by one Horner pass over a
    duplicated-expert axis eh=44: the step-j coefficient is the
    contiguous 44-wide slice [dj*S'_j | dj*(gp*S'_{j+1}+gq*S'_j)] of a
    prebuilt stack, broadcast over l; then 2*A = M/den and A*x follow.
    Only plain TensorScalarPtr/TensorCopy get DVE 2x/4x modes and walrus
    limits TSP to 2 free dims, so the hot loop is fp16 InstTensorTensor
    (2x_1p) with 3-free-dim access patterns; power sums use a pairwise
    tree over l on an (l, j, e)-interleaved power stack."""
    nc = bass.Bass()
    W3 = 2 * EL + E
    xsg = nc.declare_dram_parameter("xsg", [BC, W3], F16, isOutput=False)
    # packed broadcast constants [djv | p2v | qvv]
    NCC = (J + 1) * 2 * E + 2 * E
    cstc = nc.declare_dram_parameter("cstc", [NCC], F16, isOutput=False)
    o16 = nc.declare_dram_parameter("o16", [BC, 2 * EL], F16, isOutput=True)

    NJ = J + 1        # powers u'^1..u'^{J+1}
    ROW = NJ * E      # one l-row of the interleaved power stack

    with tile.TileContext(nc) as tc, ExitStack() as ctx:
        singles = ctx.enter_context(tc.tile_pool(name="singles", bufs=1))
        pstk = ctx.enter_context(tc.tile_pool(name="pstk", bufs=4))
        trp = ctx.enter_context(tc.tile_pool(name="trp", bufs=4))
        kp = ctx.enter_context(tc.tile_pool(name="kp", bufs=4))
        hp = ctx.enter_context(tc.tile_pool(name="hp", bufs=4))
        op = ctx.enter_context(tc.tile_pool(name="op", bufs=4))

        base = cstc[:]
        cB = singles.tile([P, NCC], F16, name="bc_cstc")
        nc.gpsimd.dma_start(
            out=cB, in_=bass.AP(tensor=base.tensor, offset=base.offset,
                                ap=[[0, P], [1, NCC]]))
        djB = cB[:, 0:(J + 1) * 2 * E]
        p2B = cB[:, NCC - 2 * E:NCC - E]
        qB = cB[:, NCC - E:NCC]

        for t in range(NT):
            ug = kp.tile([P, W3], F16, tag="ug")     # [u' | kap | gate_sel]
            nc.sync.dma_start(out=ug, in_=xsg[t * P:(t + 1) * P, :])
            u0 = ug[:, 0:EL]
            kapv = ug[:, EL:2 * EL]
            gst = ug[:, 2 * EL:W3]

            # interleaved power stack (l, j, e); ACT copies u' to slot 1
            pst = pstk.tile([P, L * ROW], F16, tag="pst")

            def slot(j):          # (l, e) view of power j
                return _ap(pst[:, (j - 1) * E:], [[ROW, L], [1, E]])

            nc.scalar.activation(out=slot(1), in_=u0, func=AF.Copy)

            # powers u'^2..u'^{J+1}: squares on ACT, odd muls on DVE/Pool
            nc.scalar.activation(out=slot(2), in_=u0, func=AF.Square)
            nc.vector.tensor_mul(out=slot(3), in0=slot(2), in1=slot(1))
            nc.scalar.activation(out=slot(4), in_=slot(2), func=AF.Square)
            if NJ >= 5:
                nc.gpsimd.tensor_mul(out=slot(5), in0=slot(3), in1=slot(2))
            if NJ >= 6:
                nc.scalar.activation(out=slot(6), in_=slot(3), func=AF.Square)
            if NJ >= 7:
                nc.gpsimd.tensor_mul(out=slot(7), in0=slot(6), in1=slot(1))

            # pairwise tree over l; (j,e) stays contiguous throughout, so
            # the final step writes S'_1.. straight into the S-stack
            Sp = trp.tile([P, (NJ + 1) * E], F16, tag="Sp")
            nc.gpsimd.memset(Sp[:, 0:E], float(L))
            t1 = trp.tile([P, 10 * ROW], F16, tag="t1")
            t2 = trp.tile([P, 5 * ROW], F16, tag="t2")
            t3 = trp.tile([P, 2 * ROW], F16, tag="t3")
            t4 = trp.tile([P, ROW], F16, tag="t4")
            t5 = trp.tile([P, ROW], F16, tag="t5")

            pR = lambda l0, n: _ap(pst[:, l0 * ROW:], [[ROW, n], [1, ROW]])
            tR = lambda tl, l0, n: _ap(tl[:, l0 * ROW:], [[ROW, n], [1, ROW]])
            nc.vector.tensor_add(out=tR(t1, 0, 10), in0=pR(0, 10),
                                 in1=pR(10, 10))
            nc.vector.tensor_add(out=tR(t2, 0, 5), in0=tR(t1, 0, 5),
                                 in1=tR(t1, 5, 5))
            nc.vector.tensor_add(out=tR(t3, 0, 2), in0=tR(t2, 0, 2),
                                 in1=tR(t2, 2, 2))
            nc.gpsimd.tensor_add(out=tR(t4, 0, 1), in0=tR(t3, 0, 1),
                                 in1=tR(t3, 1, 1))
            nc.gpsimd.tensor_add(out=tR(t5, 0, 1), in0=tR(t4, 0, 1),
                                 in1=tR(t2, 4, 1))
            nc.gpsimd.tensor_add(out=_ap(Sp[:, E:], [[1, ROW]]),
                                 in0=tR(t5, 0, 1), in1=pR(20, 1))

            # coefficient pairs: [dj*S'_j | dj*(gp*S'_{j+1} + gq*S'_j)]
            gp = kp.tile([P, E], F16, tag="gp")
            nc.gpsimd.tensor_mul(out=gp, in0=gst, in1=p2B)
            gq = kp.tile([P, E], F16, tag="gq")
            nc.gpsimd.tensor_mul(out=gq, in0=gst, in1=qB)
            SS = trp.tile([P, (J + 1) * 2 * E], F16, tag="SS")
            tq = trp.tile([P, (J + 1) * E], F16, tag="tq")
            wJ = lambda tl, off: _ap(tl[:, off:], [[E, J + 1], [1, E]])
            wS = lambda off: _ap(SS[:, off:], [[2 * E, J + 1], [1, E]])
            wD = lambda off: _ap(djB[:, off:], [[2 * E, J + 1], [1, E]])
            gpB = _ap(gp[:, 0:E], [[0, J + 1], [1, E]])
            gqB = _ap(gq[:, 0:E], [[0, J + 1], [1, E]])
            nc.vector.tensor_mul(out=wS(0), in0=wJ(Sp, 0), in1=wD(0))
            nc.vector.tensor_mul(out=wS(E), in0=wJ(Sp, E), in1=gpB)
            nc.vector.tensor_mul(out=wJ(tq, 0), in0=wJ(Sp, 0), in1=gqB)
            nc.vector.tensor_add(out=wS(E), in0=wS(E), in1=wJ(tq, 0))
            nc.vector.tensor_mul(out=wS(E), in0=wS(E), in1=wD(E))

            # joint Horner on [den | M] with 3-free-dim TT ops
            ra = hp.tile([P, 2 * EL], F16, tag="ra")
            rb = hp.tile([P, 2 * EL], F16, tag="rb")
            f44 = lambda tl: _ap(tl[:, 0:], [[EL, 2], [E, L], [1, E]])
            cj = lambda j: _ap(SS[:, j * 2 * E:], [[E, 2], [0, L], [1, E]])
            kB = _ap(ug[:, EL:], [[0, 2], [E, L], [1, E]])
            cur, other = ra, rb
            nc.vector.tensor_mul(out=f44(cur), in0=cj(J), in1=kB)
            for j in range(J - 1, -1, -1):
                nc.vector.tensor_add(out=f44(other), in0=cj(j), in1=f44(cur))
                cur, other = other, cur
                if j > 0:
                    nc.vector.tensor_mul(out=f44(other), in0=f44(cur), in1=kB)
                    cur, other = other, cur

            # 2*A = M/den (host halves A on the way out); A*x = 2A * u'
            rd32 = op.tile([P, EL], F32, tag="rd32")
            nc.vector.reciprocal(out=rd32, in_=cur[:, 0:EL])
            ot = op.tile([P, 2 * EL], F16, tag="ot")
            nc.gpsimd.tensor_mul(out=ot[:, 0:EL], in0=cur[:, EL:2 * EL],
                                 in1=rd32)
            nc.gpsimd.tensor_mul(out=ot[:, EL:2 * EL], in0=ot[:, 0:EL],
                                 in1=u0)
            nc.scalar.dma_start(out=o16[t * P:(t + 1) * P, :], in_=ot)
    legalize_sync_waits(nc)
    return nc


def _gate_params(inputs):
    gc_w, gc_b = inputs["gc_w"], inputs["gc_b"]
    wbar = gc_w.mean(0)
    bbar = gc_b.mean()
    NCB = CL + C + 25 + C + P
    cstb = np.zeros((P, NCB), np.float32)
    cb = 0
    for g, chans in enumerate(GROUPS):
        nch = len(chans)
        for k, c in enumerate(chans):
            cstb[k * L:(k + 1) * L, cb + k * L:cb + (k + 1) * L] = gc_w.T
            cstb[k * L:(k + 1) * L, CL + c] = wbar
            cstb[nch * L, CL + c] = bbar
        cstb[nch * L, cb:cb + nch * L] = np.tile(gc_b, nch)
        cb += nch * L
    cstb[0:C, CL + C:CL + C + 25] = inputs["w1"].T
    cstb[0:25, CL + C + 25:CL + 2 * C + 25] = inputs["w2"].T
    cstb[:, CL + 2 * C + 25:NCB] = np.eye(P)
    cstf = np.zeros((C, 2), np.float32)
    cstf[0:25, 0] = inputs["b1"]
    cstf[0:C, 1] = inputs["b2"]
    return cstb.astype(ml_dtypes.bfloat16), cstf


_CACHE = {}


def kernel(**inputs):
    inputs = {k: np.ascontiguousarray(np.asarray(v)) for k, v in inputs.items()}
    x = inputs["x"].astype(np.float32)              # (B, C, L)
    bf = ml_dtypes.bfloat16
    cores = list(range(NCORES))

    # ---- launch 1: gate -------------------------------------------------
    cstb, cstf = _gate_params(inputs)
    # host-marshaled transposed x: 8 chunks of (6ch x 21 + ones row) x B
    xt = x.reshape(B, CL).T                          # (945, B)
    xg = np.zeros((NG * GROWS, B), np.float32)
    cb = 0
    for g, chans in enumerate(GROUPS):
        nch = len(chans)
        xg[g * GROWS:g * GROWS + nch * L] = xt[cb:cb + nch * L]
        xg[g * GROWS + nch * L] = 1.0
        cb += nch * L
    xg = xg.astype(bf)

    if "gate" not in _CACHE:
        _CACHE["gate"] = build_gate_program()
    nc1 = _CACHE["gate"]
    maps1 = [{"xg": np.ascontiguousarray(xg[:, i * BC:(i + 1) * BC]),
              "cstb": cstb, "cstf": cstf} for i in cores]
    r1 = run_bass_kernel_spmd(nc1, maps1, cores).results
    gate16 = np.concatenate([np.asarray(r["gate"]) for r in r1], 0)  # (B,45)

    # ---- routing (host-mediated all-reduce) -----------------------------
    mean_gate = gate16.astype(np.float64).mean(0)
    sel = np.sort(np.argsort(-mean_gate, kind="stable")[:E])

    # ---- launch 2: attention -------------------------------------------
    wq, bq = inputs["wq"], inputs["bq"]
    wk, bk = inputs["wk"], inputs["bk"]
    wv, bv = inputs["wv"], inputs["bv"]
    wo, bo = inputs["wo"], inputs["bo"]
    alpha = (wq * wk).sum(1).astype(np.float64)
    gamma = (bq * wk).sum(1).astype(np.float64)
    pv = (wo * wv).sum(1).astype(np.float64)
    qv = ((wo * bv).sum(1) + bo).astype(np.float64)

    xsel = x[:, sel, :]                              # (B, E, L)
    umax = float(np.abs(xsel).max())
    zm = (np.abs(alpha).max() * umax + np.abs(gamma).max()) * umax
    cheb = np.polynomial.chebyshev.Chebyshev.interpolate(
        np.exp, J, domain=[-zm, zm])
    dc = cheb.convert(kind=np.polynomial.Polynomial).coef
    dj = [float(dc[j] * (2.0 ** j)) for j in range(J + 1)]

    key = tuple(np.round(dj, 12))
    if _CACHE.get("attn_key") != key:
        _CACHE["attn"] = build_attn_program(dj)
        _CACHE["attn_key"] = key
    nc2 = _CACHE["attn"]

    xsg = np.empty((B, 2 * EL + E), np.float16)  # [u' | kap | gate] l-major
    xlm = np.ascontiguousarray(xsel.transpose(0, 2, 1).astype(np.float32))
    xsg[:, :EL] = (xlm * np.float32(0.5)).reshape(B, EL)
    xsg[:, EL:2 * EL] = (xlm * alpha.astype(np.float32)[None, None, :]
                         + gamma.astype(np.float32)[None, None, :]
                         ).reshape(B, EL)
    xsg[:, 2 * EL:] = gate16[:, sel]
    cstc = np.concatenate([
        np.repeat(np.asarray(dj), 2 * E).astype(np.float16),
        (4 * pv).astype(np.float16),
        (2 * qv).astype(np.float16)]).astype(np.float16)
    maps2 = [{"xsg": xsg[i * BC:(i + 1) * BC], "cstc": cstc}
             for i in cores]
    r2 = run_bass_kernel_spmd(nc2, maps2, cores).results
    o16 = np.concatenate([np.asarray(r["o16"]) for r in r2], 0)  # (B, 924)

    # ---- host unshard / scatter (device emits 2*A and A*x) -------------
    at = (o16[:, :EL].astype(np.float32) * 0.5).reshape(
        B, L, E).transpose(0, 2, 1)
    gt = o16[:, EL:].astype(np.float32).reshape(B, L, E).transpose(0, 2, 1)
    cols = (np.repeat(sel * L, L) + np.tile(np.arange(L), E))
    A_full = np.zeros((B, CL), np.float32)
    G_full = np.zeros((B, CL), np.float32)
    A_full[:, cols] = at.reshape(B, EL)
    G_full[:, cols] = gt.reshape(B, EL)
    return G_full, A_full
